# revision 1
# baseline (speedup 1.0000x reference)
"""LocalGlobalTransformerEncoderBlock on 8 Trainium2 NeuronCores.

Sharding: core = (batch b = core//2, sequence half h = core%2). Each core
computes the full encoder block for 1024 query rows of one batch plus the
global token (sequence position 0). The per-core sequence is ROTATED by
1024*h so the core's query rows are always rotated rows [0, 1024), and
x[b, 0] (the global token) is appended as row 2048. The band attention uses
4 aligned 128-key chunks per 256-query block (window [256i-128, 256i+384)
mod 2048) with host-built multiplicative masks; the global token's full
2048-key attention runs in a dedicated path. All masks are derived from the
actual attn_mask/padding_mask inputs.

Self-contained: only imports from /opt/trn_rl_repo (the installed bass
runtime), numpy, and stdlib.
"""

import sys
from contextlib import ExitStack

if "/opt/trn_rl_repo" not in sys.path:
    sys.path.insert(0, "/opt/trn_rl_repo")

import numpy as np

import concourse.bass as bass
import concourse.bacc as bacc_mod
import concourse.mybir as mybir
import concourse.tile as tile
from concourse.masks import make_identity

P = 128
B, S, D, H, FF = 4, 2048, 512, 8, 2048
HD = D // H            # 64
DC = D // P            # 4 chunks of the model dim
FFC = FF // P          # 16 chunks of the FF dim
SK = S + 1             # 2049 keys (2048 rotated + appended global token)
SQ = 1024              # band queries per core
NT = SQ + 1            # 1025 output tokens (1024 band + 1 global)
QB = 256               # band query block
NQB = SQ // QB         # 4
NKC = 4                # aligned 128-key chunks per band window
NPAIR = H // 2         # 4 head-pair tiles (2 heads of 64 rows each)
EPS = 1e-5
NEG = -1e30

F32 = mybir.dt.float32
F32R = mybir.dt.float32r
AF = mybir.ActivationFunctionType
ALU = mybir.AluOpType


def _r(ap):
    """Reinterpret an fp32 AP as float32r for full-rate PE matmuls."""
    return ap.bitcast(F32R)


def _layernorm_transpose(nc, ln_pool, stat_pool, tp_psum, src_tiles, dst_T,
                         eps_t, ident, ntiles, tag):
    """LN over natural [rows, D] tiles, write transposed into dst_T chunks.

    src_tiles(t) -> (ap [rows, D], rows). dst_T: list of DC [P, *] tiles,
    written at cols [t*P, t*P+rows).
    """
    for t in range(ntiles):
        xt, rows = src_tiles(t)
        st = stat_pool.tile([P, 6], F32, tag=f"{tag}_bnst")
        nc.vector.bn_stats(st[:rows], xt)
        mv = stat_pool.tile([P, 2], F32, tag=f"{tag}_bnmv")
        nc.vector.bn_aggr(mv[:rows], st[:rows])
        rstd = stat_pool.tile([P, 1], F32, tag=f"{tag}_rstd")
        nc.scalar.activation(rstd[:rows], mv[:rows, 1:2], AF.Sqrt,
                             bias=eps_t[:rows], scale=1.0)
        nc.vector.reciprocal(rstd[:rows], rstd[:rows])
        z = ln_pool.tile([P, D], F32, tag=f"{tag}_z")
        nc.vector.tensor_scalar(z[:rows], xt, mv[:rows, 0:1],
                                rstd[:rows], op0=ALU.subtract, op1=ALU.mult)
        for d in range(DC):
            pt = tp_psum.tile([P, P], F32, tag=f"{tag}_tp")
            nc.tensor.transpose(pt[:, :rows], z[:rows, d * P : (d + 1) * P],
                                ident[:rows, :rows])
            nc.scalar.activation(dst_T[d][:, t * P : t * P + rows].bitcast(F32R),
                                 pt[:, :rows], AF.Copy)


def build_module():
    nc = bacc_mod.Bacc("TRN2", target_bir_lowering=False)

    x_nat = nc.dram_tensor("x_nat", [SK, D], F32, kind="ExternalInput")
    wq_pc = nc.dram_tensor("wq_pc", [P, DC, D], F32R, kind="ExternalInput")
    wk_pc = nc.dram_tensor("wk_pc", [P, DC, D], F32R, kind="ExternalInput")
    wv_pc = nc.dram_tensor("wv_pc", [P, DC, D], F32R, kind="ExternalInput")
    bq_pc = nc.dram_tensor("bq_pc", [P, DC], F32, kind="ExternalInput")
    bk_pc = nc.dram_tensor("bk_pc", [P, DC], F32, kind="ExternalInput")
    bv_row = nc.dram_tensor("bv_row", [1, D], F32R, kind="ExternalInput")
    wo_pc = nc.dram_tensor("wo_pc", [P, DC, D], F32R, kind="ExternalInput")
    bo_row = nc.dram_tensor("bo_row", [1, D], F32R, kind="ExternalInput")
    w1_pc = nc.dram_tensor("w1_pc", [P, DC, FF], F32R, kind="ExternalInput")
    b1_pc = nc.dram_tensor("b1_pc", [P, FFC], F32, kind="ExternalInput")
    w2_pc = nc.dram_tensor("w2_pc", [P, FFC, D], F32R, kind="ExternalInput")
    b2_row = nc.dram_tensor("b2_row", [1, D], F32R, kind="ExternalInput")
    mask_band = nc.dram_tensor("mask_band", [P, NQB, NKC, QB], F32, kind="ExternalInput")
    mask_gcol = nc.dram_tensor("mask_gcol", [1, NQB, QB], F32, kind="ExternalInput")
    mask_g = nc.dram_tensor("mask_g", [8, S], F32, kind="ExternalInput")
    ones_in = nc.dram_tensor("ones_in", [1, P], F32R, kind="ExternalInput")
    e2_in = nc.dram_tensor("e2_in", [P, 2], F32R, kind="ExternalInput")
    vones_in = nc.dram_tensor("vones_in", [P, 17, H], F32R, kind="ExternalInput")
    y_out = nc.dram_tensor("y", [NT, D], F32, kind="ExternalOutput")

    with tile.TileContext(nc) as tc, ExitStack() as ctx:
        persist = ctx.enter_context(tc.tile_pool(name="persist", bufs=1))
        ident = persist.tile([P, P], F32)
        make_identity(nc, ident)
        ones_row = persist.tile([1, P], F32R)
        nc.sync.dma_start(ones_row, ones_in[:])
        eps_t = persist.tile([P, 1], F32)
        nc.vector.memset(eps_t, EPS)
        oT = [persist.tile([P, NT], F32, name=f"oT{p}") for p in range(NPAIR)]

        with tc.tile_pool(name="attn_scope", bufs=1) as attn_scope:
            QT = [attn_scope.tile([P, NT], F32, name=f"QT{p}") for p in range(NPAIR)]
            KT = [attn_scope.tile([P, SK], F32, name=f"KT{p}") for p in range(NPAIR)]
            Vsb = attn_scope.tile([P, 17, H, HD + 1], F32)  # ones col interleaved
            lrow = attn_scope.tile([1, H * SQ], F32)
            nc.sync.dma_start(Vsb[:, :, :, HD].bitcast(F32R), vones_in[:])

            # ====== Phases A+B: LN1 -> zT, QKV projections ======
            with tc.tile_pool(name="zbuf", bufs=1) as z_scope, \
                 tc.tile_pool(name="wqkv", bufs=1) as w_scope, \
                 tc.tile_pool(name="ln1", bufs=3) as ln_pool, \
                 tc.tile_pool(name="st1", bufs=4) as stat_pool, \
                 tc.tile_pool(name="tp1", bufs=4, space="PSUM") as tp_psum:
                zT = [z_scope.tile([P, SK], F32, name=f"zT{d}") for d in range(DC)]
                wq_sb = w_scope.tile([P, DC, D], F32R)
                nc.sync.dma_start(wq_sb, wq_pc[:])
                wk_sb = w_scope.tile([P, DC, D], F32R)
                nc.sync.dma_start(wk_sb, wk_pc[:])
                wv_sb = w_scope.tile([P, DC, D], F32R)
                nc.sync.dma_start(wv_sb, wv_pc[:])
                bq_sb = w_scope.tile([P, DC], F32)
                nc.sync.dma_start(bq_sb, bq_pc[:])
                bk_sb = w_scope.tile([P, DC], F32)
                nc.sync.dma_start(bk_sb, bk_pc[:])
                bv_sb = w_scope.tile([1, D], F32R)
                nc.sync.dma_start(bv_sb, bv_row[:])

                def ln1_src(t, _pool=ln_pool):
                    rows = P if t < 16 else 1
                    xt = _pool.tile([P, D], F32, tag="xt")
                    nc.sync.dma_start(xt[:rows], x_nat[t * P : t * P + rows, :])
                    return xt[:rows], rows

                _layernorm_transpose(nc, ln_pool, stat_pool, tp_psum, ln1_src,
                                     zT, eps_t, ident, 17, "ln1")

                # ---- QKV projections (Phase B) ----
                with tc.tile_pool(name="qkv_ps", bufs=3, space="PSUM") as mm_psum:
                    q_blocks = [(0, 0, 512), (512, 512, 512), (S, SQ, 1)]
                    k_blocks = [(i * 512, i * 512, 512) for i in range(4)] + [(S, S, 1)]
                    for p in range(NPAIR):
                        for w_sb, b_sb, dst_T in ((wq_sb, bq_sb, QT), (wk_sb, bk_sb, KT)):
                            blocks = q_blocks if dst_T is QT else k_blocks
                            for src, dst, w in blocks:
                                # fp32r matmul needs moving dim >= 2: widen
                                # 1-col tails with the (real) preceding column
                                s0, w0, keep = (src, w, 0) if w > 1 else (src - 1, 2, 1)
                                ps = mm_psum.tile([P, 512], F32, tag="qk")
                                for d in range(DC):
                                    nc.tensor.matmul(ps[:, :w0],
                                                     _r(w_sb[:, d, p * P : (p + 1) * P]),
                                                     _r(zT[d][:, s0 : s0 + w0]),
                                                     start=(d == 0), stop=(d == DC - 1))
                                nc.scalar.activation(
                                    dst_T[p][:, dst : dst + w].bitcast(F32R),
                                    ps[:, keep : keep + w], AF.Identity,
                                    bias=b_sb[:, p : p + 1])
                    for t in range(17):
                        rows = P if t < 16 else 1
                        ps = mm_psum.tile([P, D], F32, tag="qk")
                        for d in range(DC):
                            nc.tensor.matmul(ps[:rows],
                                             _r(zT[d][:, t * P : t * P + rows]),
                                             _r(wv_sb[:, d, :]),
                                             start=(d == 0), stop=False)
                        nc.tensor.matmul(ps[:rows], _r(ones_row[:1, :rows]), _r(bv_sb),
                                         start=False, stop=True)
                        nc.scalar.activation(
                            Vsb[:rows, t, :, 0:HD].bitcast(F32R),
                            ps[:rows].rearrange("p (h e) -> p h e", h=H),
                            AF.Copy)

            # ====== Phase C: banded local attention ======
            with tc.tile_pool(name="bandmask", bufs=1) as m_scope, \
                 tc.tile_pool(name="sc_ps", bufs=2, space="PSUM") as sc_psum, \
                 tc.tile_pool(name="po_ps", bufs=2, space="PSUM") as po_psum, \
                 tc.tile_pool(name="gc_ps", bufs=1, space="PSUM") as gc_psum, \
                 tc.tile_pool(name="pT", bufs=3) as p_pool:
                mb_sb = m_scope.tile([P, NQB, NKC, QB], F32)
                nc.sync.dma_start(mb_sb, mask_band[:])
                mgc_sb = m_scope.tile([1, NQB, QB], F32)
                nc.sync.dma_start(mgc_sb, mask_gcol[:])
                # global-key column scores for all queries, per head
                pgall = [None] * H
                for h in range(H):
                    pr, sub = h // 2, (h % 2) * HD
                    pg = p_pool.tile([1, SQ], F32, tag=f"pg{h}", bufs=1)
                    for half in range(2):
                        sgc = gc_psum.tile([1, 512], F32, tag="sgc")
                        nc.tensor.matmul(
                            sgc, _r(KT[pr][sub : sub + HD, S : S + 1]),
                            _r(QT[pr][sub : sub + HD, half * 512 : (half + 1) * 512]),
                            start=True, stop=True)
                        nc.scalar.activation(
                            pg[0:1, half * 512 : (half + 1) * 512].bitcast(F32R),
                            sgc, AF.Exp)
                    nc.vector.tensor_tensor(pg.bitcast(F32R), pg,
                                            mgc_sb[0:1, :, :], ALU.mult)
                    pgall[h] = pg
                for i in range(NQB):
                    for h in range(H):
                        pr, sub = h // 2, (h % 2) * HD
                        q_ap = QT[pr][sub : sub + HD, i * QB : (i + 1) * QB]
                        sc = sc_psum.tile([P, NKC, QB], F32, tag="sc")
                        for c in range(NKC):
                            a = (2 * i - 1 + c) % 16
                            nc.tensor.matmul(sc[:, c, :],
                                             _r(KT[pr][sub : sub + HD, a * P : (a + 1) * P]),
                                             _r(q_ap), start=True, stop=True)
                        pT = p_pool.tile([P, NKC, QB], F32, tag="pT")
                        nc.scalar.activation(pT[:].bitcast(F32R), sc[:], AF.Exp)
                        nc.vector.tensor_tensor(pT[:].bitcast(F32R), pT[:], mb_sb[:, i, :, :], ALU.mult)
                        po = po_psum.tile([HD + 1, QB], F32, tag="po")
                        for c in range(NKC):
                            a = (2 * i - 1 + c) % 16
                            nc.tensor.matmul(po, _r(Vsb[:, a, h, :]), _r(pT[:, c, :]),
                                             start=(c == 0), stop=False)
                        nc.tensor.matmul(po, _r(Vsb[0:1, 16, h, :]),
                                         _r(pgall[h][0:1, i * QB : (i + 1) * QB]),
                                         start=False, stop=True)
                        nc.vector.tensor_copy(
                            oT[pr][sub : sub + HD, i * QB : (i + 1) * QB].bitcast(F32R),
                            po[0:HD, :])
                        nc.scalar.activation(
                            lrow[0:1, h * SQ + i * QB : h * SQ + (i + 1) * QB].bitcast(F32R),
                            po[HD : HD + 1, :], AF.Copy)

            # ====== Phase D: global-token full attention ======
            with tc.tile_pool(name="eg", bufs=2) as eg_pool, \
                 tc.tile_pool(name="eg1", bufs=1) as eg1_pool, \
                 tc.tile_pool(name="eg_ps", bufs=2, space="PSUM") as eg_psum, \
                 tc.tile_pool(name="tp2", bufs=2, space="PSUM") as tp_psum:
                mg_sb = eg1_pool.tile([8, S], F32)
                nc.sync.dma_start(mg_sb, mask_g[:])
                E2 = eg1_pool.tile([P, 2], F32R)
                nc.sync.dma_start(E2, e2_in[:])
                pgT = eg1_pool.tile([P, 16, 8], F32)
                for p in range(NPAIR):
                    kg = eg_pool.tile([P, S], F32, tag="kg")
                    nc.vector.tensor_scalar_mul(kg.bitcast(F32R), KT[p][:, 0:S], QT[p][:, SQ : SQ + 1])
                    sgp = eg_pool.tile([2, S], F32, tag="sgp")
                    for tcb in range(4):
                        ps = eg_psum.tile([2, 512], F32, tag="sgps")
                        nc.tensor.matmul(ps, _r(E2), _r(kg[:, tcb * 512 : (tcb + 1) * 512]),
                                         start=True, stop=True)
                        nc.scalar.activation(sgp[:, tcb * 512 : (tcb + 1) * 512],
                                             ps, AF.Copy)
                    nc.vector.tensor_tensor(sgp, sgp, mg_sb[0:2, :], ALU.add)
                    lgp = eg_pool.tile([2, 1], F32, tag="lgp")
                    nc.scalar.activation(sgp, sgp, AF.Exp, accum_out=lgp)
                    nc.vector.reciprocal(lgp, lgp)
                    nc.vector.tensor_scalar_mul(sgp, sgp, lgp)  # normalized probs
                    for c in range(16):
                        pt = tp_psum.tile([P, 8], F32, tag="pgt")
                        nc.tensor.transpose(pt[:, 0:2], sgp[0:2, c * P : (c + 1) * P],
                                            ident[0:2, 0:2])
                        nc.scalar.activation(
                            pgT[:, c, 2 * p : 2 * p + 2].bitcast(F32R),
                            pt[:, 0:2], AF.Copy)
                for g in range(2):
                    pog = eg_psum.tile([8, 4 * HD], F32, tag="pog")
                    for c in range(16):
                        nc.tensor.matmul(pog, _r(pgT[:, c, :]),
                                         _r(Vsb[:, c, 4 * g : 4 * g + 4, 0:HD]),
                                         start=(c == 0), stop=(c == 15))
                    pog_sb = eg_pool.tile([8, 4 * HD], F32, tag="pog_sb")
                    nc.scalar.activation(pog_sb, pog, AF.Copy)
                    # transpose so each head's diag block lands partition-aligned,
                    # then copy straight into oT's global-token column
                    for j in range(2):
                        ptj = tp_psum.tile([P, 8], F32, tag="ogt")
                        nc.tensor.transpose(ptj[:, 0:8],
                                            pog_sb[0:8, j * P : (j + 1) * P],
                                            ident[0:8, 0:8])
                        for hh in (2 * j, 2 * j + 1):
                            h = 4 * g + hh
                            rlo = (hh % 2) * HD
                            nc.scalar.activation(
                                oT[h // 2][rlo : rlo + HD, SQ : SQ + 1].bitcast(F32R),
                                ptj[rlo : rlo + HD, h : h + 1], AF.Copy)

            # ---- normalize band outputs by softmax sums ----
            # broadcast l across partitions via ones-column PE matmul, then
            # divide: oT[:, q] *= 1/l[head(q-row), q]
            with tc.tile_pool(name="lnorm", bufs=3) as norm_pool, \
                 tc.tile_pool(name="lnorm_ps", bufs=3, space="PSUM") as norm_psum:
                for p in range(NPAIR):
                    for seg in range(2):
                        lbi = norm_pool.tile([P, 512], F32, tag="lbi")
                        for s2 in range(2):
                            off = (2 * p + s2) * SQ + seg * 512
                            lbp = norm_psum.tile([HD, 512], F32, tag="lbp")
                            nc.tensor.matmul(
                                lbp, _r(ones_row[0:1, 0:HD]),
                                _r(lrow[0:1, off : off + 512]),
                                start=True, stop=True)
                            nc.vector.reciprocal(lbi[s2 * HD : (s2 + 1) * HD, :], lbp)
                        nc.vector.tensor_tensor(
                            oT[p][:, seg * 512 : (seg + 1) * 512].bitcast(F32R),
                            oT[p][:, seg * 512 : (seg + 1) * 512], lbi, ALU.mult)

        # ====== Phase E: out_proj + residual -> x1 ======
        x1_scope = ctx.enter_context(tc.tile_pool(name="x1_scope", bufs=1))
        x1 = x1_scope.tile([P, 9, D], F32)
        # prefetch FFN weights now so their DMA overlaps phases E+F
        ffw_pool = ctx.enter_context(tc.tile_pool(name="ffw", bufs=1))
        w1_sb = ffw_pool.tile([P, DC, FF], F32R)
        nc.sync.dma_start(w1_sb, w1_pc[:])
        b1_sb = ffw_pool.tile([P, FFC], F32)
        nc.sync.dma_start(b1_sb, b1_pc[:])
        w2_sb = ffw_pool.tile([P, FFC, D], F32R)
        nc.sync.dma_start(w2_sb, w2_pc[:])
        b2_sb = ffw_pool.tile([1, D], F32R)
        nc.sync.dma_start(b2_sb, b2_row[:])
        with tc.tile_pool(name="opj", bufs=3) as op_pool, \
             tc.tile_pool(name="opjw", bufs=1) as opw_pool, \
             tc.tile_pool(name="opj_ps", bufs=3, space="PSUM") as op_psum:
            wo_sb = opw_pool.tile([P, DC, D], F32R)
            nc.sync.dma_start(wo_sb, wo_pc[:])
            bo_sb = opw_pool.tile([1, D], F32R)
            nc.sync.dma_start(bo_sb, bo_row[:])
            for t in range(9):
                w = P if t < 8 else 1
                src_row = t * P if t < 8 else S
                xr = op_pool.tile([P, D], F32, tag="xr")
                nc.sync.dma_start(xr[:w], x_nat[src_row : src_row + w, :])
                ps = op_psum.tile([P, D], F32, tag="yps")
                for p in range(NPAIR):
                    nc.tensor.matmul(ps[:w], _r(oT[p][:, t * P : t * P + w]),
                                     _r(wo_sb[:, p, :]), start=(p == 0), stop=False)
                nc.tensor.matmul(ps[:w], _r(ones_row[:1, :w]), _r(bo_sb),
                                 start=False, stop=True)
                nc.vector.tensor_tensor(x1[:w, t, :], ps[:w], xr[:w], ALU.add)

        # ====== Phases F+G: LN2 -> z2T, FFN + residual -> y ======
        with tc.tile_pool(name="z2buf", bufs=1) as z2_scope:
            z2T = [z2_scope.tile([P, NT], F32, name=f"z2T{d}") for d in range(DC)]
            with tc.tile_pool(name="ln2", bufs=3) as ln_pool, \
                 tc.tile_pool(name="st2", bufs=4) as stat_pool, \
                 tc.tile_pool(name="tp3", bufs=4, space="PSUM") as tp_psum:

                def ln2_src(t):
                    rows = P if t < 8 else 1
                    return x1[:rows, t, :], rows

                _layernorm_transpose(nc, ln_pool, stat_pool, tp_psum, ln2_src,
                                     z2T, eps_t, ident, 9, "ln2")

            with tc.tile_pool(name="ffn", bufs=2) as ffn_pool, \
                 tc.tile_pool(name="ffo", bufs=3) as out_pool, \
                 tc.tile_pool(name="ffn_ps", bufs=2, space="PSUM") as h_psum, \
                 tc.tile_pool(name="y2_ps", bufs=2, space="PSUM") as y_psum:
                for t0, tw in [(0, 512), (512, 512), (SQ, 1)]:
                    hT = ffn_pool.tile([P, FFC, 512], F32, tag="hT")
                    s0, w0, keep = (t0, tw, 0) if tw > 1 else (t0 - 1, 2, 1)
                    for f in range(FFC):
                        ps = h_psum.tile([P, 512], F32, tag="h1")
                        for d in range(DC):
                            nc.tensor.matmul(ps[:, :w0],
                                             _r(w1_sb[:, d, f * P : (f + 1) * P]),
                                             _r(z2T[d][:, s0 : s0 + w0]),
                                             start=(d == 0), stop=(d == DC - 1))
                        nc.scalar.activation(hT[:, f, :tw].bitcast(F32R),
                                             ps[:, keep : keep + tw], AF.Gelu,
                                             bias=b1_sb[:, f : f + 1])
                    nsub = 4 if tw == 512 else 1
                    for stp in range(nsub):
                        sw = P if tw == 512 else 1
                        ps2 = y_psum.tile([P, D], F32, tag="y2")
                        for f in range(FFC):
                            nc.tensor.matmul(ps2[:sw],
                                             _r(hT[:, f, stp * P : stp * P + sw]),
                                             _r(w2_sb[:, f, :]),
                                             start=(f == 0), stop=False)
                        nc.tensor.matmul(ps2[:sw], _r(ones_row[:1, :sw]), _r(b2_sb),
                                         start=False, stop=True)
                        yt = out_pool.tile([P, D], F32, tag="yt")
                        tglob = t0 // P + stp
                        nc.vector.tensor_tensor(yt[:sw], ps2[:sw],
                                                x1[:sw, tglob, :], ALU.add)
                        nc.sync.dma_start(y_out[t0 + stp * P : t0 + stp * P + sw, :],
                                          yt[:sw])

    nc.finalize()
    return nc


def make_host_inputs(x, padding_mask, attn_mask, in_proj_w, in_proj_b, out_proj_w,
                     out_proj_b, ln1_g, ln1_b, ln2_g, ln2_b, ff_w1, ff_b1, ff_w2,
                     ff_b2):
    """Build the 8 per-core input maps (numpy only)."""
    f32 = np.float32
    x = np.asarray(x, f32)
    attn_mask = np.asarray(attn_mask, f32)
    padding_mask = np.asarray(padding_mask, bool)

    g1 = np.asarray(ln1_g, f32); b1 = np.asarray(ln1_b, f32)
    g2 = np.asarray(ln2_g, f32); b2 = np.asarray(ln2_b, f32)
    Wq, Wk, Wv = (np.asarray(in_proj_w[i * D:(i + 1) * D], f32) for i in range(3))
    bq0, bk0, bv0 = (np.asarray(in_proj_b[i * D:(i + 1) * D], f32) for i in range(3))
    sc = 1.0 / np.sqrt(HD)

    Wq_ = Wq * g1[None, :] * sc
    bq_ = (Wq @ b1 + bq0) * sc
    Wk_ = Wk * g1[None, :]
    bk_ = Wk @ b1 + bk0
    Wv_ = Wv * g1[None, :]
    bv_ = Wv @ b1 + bv0
    W1_ = np.asarray(ff_w1, f32) * g2[None, :]
    b1f = np.asarray(ff_w1, f32) @ b2 + np.asarray(ff_b1, f32)

    def pc(wt, nchunk):  # [Dout, Din] -> [P, nchunk, Dout] chunked on Din
        return np.ascontiguousarray(
            wt.T.reshape(nchunk, P, wt.shape[0]).transpose(1, 0, 2))

    shared = {
        "wq_pc": pc(Wq_, DC), "wk_pc": pc(Wk_, DC), "wv_pc": pc(Wv_, DC),
        "bq_pc": np.ascontiguousarray(bq_.reshape(DC, P).T),
        "bk_pc": np.ascontiguousarray(bk_.reshape(DC, P).T),
        "bv_row": bv_[None, :].copy(),
        "wo_pc": pc(np.asarray(out_proj_w, f32), DC),
        "bo_row": np.asarray(out_proj_b, f32)[None, :].copy(),
        "w1_pc": pc(W1_, DC),
        "b1_pc": np.ascontiguousarray(b1f.reshape(FFC, P).T),
        "w2_pc": pc(np.asarray(ff_w2, f32), FFC),
        "b2_row": np.asarray(ff_b2, f32)[None, :].copy(),
        "ones_in": np.ones((1, P), f32),
        "e2_in": np.concatenate([
            np.repeat(np.array([[1.0, 0.0]], f32), HD, axis=0),
            np.repeat(np.array([[0.0, 1.0]], f32), HD, axis=0)], axis=0),
        "vones_in": np.ones((P, 17, H), f32),
    }

    in_maps = []
    for core in range(8):
        b = core // 2
        h = core % 2
        rot = np.roll(x[b], -1024 * h, axis=0)
        x_nat = np.ascontiguousarray(np.concatenate([rot, x[b, 0:1]], axis=0))

        # additive mask for this batch -> multiplicative factor
        A = attn_mask + np.where(padding_mask[b], -np.inf, 0.0)[None, :]
        mfac = np.exp(np.minimum(A, 0.0)).astype(f32)  # exp(-inf)=0, exp(0)=1
        mfac[~np.isfinite(A)] = 0.0

        # band masks: [P(t), NQB(i), NKC(c), QB(r)]
        i_idx = np.arange(NQB)[:, None, None, None]
        c_idx = np.arange(NKC)[None, :, None, None]
        t_idx = np.arange(P)[None, None, :, None]
        r_idx = np.arange(QB)[None, None, None, :]
        a_idx = (2 * i_idx - 1 + c_idx) % 16
        k_rot = a_idx * P + t_idx
        q_rot = i_idx * QB + r_idx
        gq = (q_rot + 1024 * h) % S
        gk = (k_rot + 1024 * h) % S
        band = mfac[gq, gk]                       # [NQB, NKC, P, QB]
        mask_band = np.ascontiguousarray(band.transpose(2, 0, 1, 3))

        # global-key column mask: [1, NQB, QB]
        key0_rot = (0 - 1024 * h) % S
        gq2 = (np.arange(NQB)[:, None] * QB + np.arange(QB)[None, :] + 1024 * h) % S
        gcol = mfac[gq2, 0].copy()
        for i in range(NQB):
            chunks = {(2 * i - 1 + c) % 16 for c in range(NKC)}
            if key0_rot // P in chunks:
                gcol[i, :] = 0.0  # key 0 already inside this block's band window
        mask_gcol = np.ascontiguousarray(gcol[None, :, :])

        # global-query additive mask row, rotated, replicated across 8 heads
        Arow = A[0, (np.arange(S) + 1024 * h) % S]
        mask_g = np.ascontiguousarray(
            np.tile(np.maximum(Arow, NEG)[None, :], (8, 1)).astype(f32))

        m = dict(shared)
        m.update({
            "x_nat": x_nat,
            "mask_band": mask_band.astype(f32),
            "mask_gcol": mask_gcol.astype(f32),
            "mask_g": mask_g,
        })
        in_maps.append(m)
    return in_maps


def assemble_output(results):
    """results: list of 8 dicts with 'y' [NT, D] -> full [B, S, D]."""
    out = np.empty((B, S, D), np.float32)
    for b in range(B):
        y0 = results[2 * b]["y"]
        y1 = results[2 * b + 1]["y"]
        out[b, 0] = y0[SQ]
        out[b, 1:SQ] = y0[1:SQ]
        out[b, SQ:] = y1[0:SQ]
    return out


_CACHED_NC = None


def kernel(**inputs) -> np.ndarray:
    global _CACHED_NC
    from concourse.bass_utils import run_bass_kernel_spmd

    in_maps = make_host_inputs(**inputs)
    if _CACHED_NC is None:
        _CACHED_NC = build_module()
    res = run_bass_kernel_spmd(_CACHED_NC, in_maps, core_ids=list(range(8)))
    return assemble_output(res.results)


if __name__ == "__main__":
    nc = build_module()
    print("build + compile OK")



# revision 28
# speedup vs baseline: 1.0063x; 1.0063x over previous
"""LocalGlobalTransformerEncoderBlock on 8 Trainium2 NeuronCores.

Sharding: core = (batch b = core//2, sequence half h = core%2). Each core
computes the full encoder block for 1024 query rows of one batch plus the
global token (sequence position 0). The per-core sequence is ROTATED by
1024*h so the core's query rows are always rotated rows [0, 1024), and
x[b, 0] (the global token) is appended as row 2048.

v2 redesign vs the 481us baseline:
  - Band attention uses 3 unaligned 128-key chunks per 256-query block
    (window [256i-64, 256i+320) mod 2048) via a 64-col wrapped prefix on
    the transposed K / z buffers and half-shifted V key tiles.
  - Attention tensors (Q^T, K^T, V, probs, masks) and FFN weights/hidden
    are bf16: same PE rate as fp32r but half the SBUF/DMA and 2x DVE.
  - Softmax denominators are gathered into a [32, 256] tile so ONE
    partition-parallel reciprocal replaces 16 serial [64,512] ones.
  - The global-token path is per-head wide matmuls + one fused softmax,
    emitted interleaved with the band blocks so it hides under them.
  - Normalize/out_proj/LN2 are emitted per band block (software pipeline);
    FFN weights prefetch during attention; x is loaded once into the
    residual buffer.

Self-contained: only imports from /opt/trn_rl_repo (the installed bass
runtime), numpy, and stdlib.
"""

import sys
from contextlib import ExitStack

if "/opt/trn_rl_repo" not in sys.path:
    sys.path.insert(0, "/opt/trn_rl_repo")

import numpy as np

import concourse.bass as bass
import concourse.bacc as bacc_mod
import concourse.mybir as mybir
import concourse.tile as tile
from concourse.masks import make_identity

P = 128
B, S, D, H, FF = 4, 2048, 512, 8, 2048
HD = D // H            # 64
DC = D // P            # 4 chunks of the model dim
FFC = FF // P          # 16 chunks of the FF dim
SK = S + 1             # 2049 tokens (2048 rotated + appended global token)
SQ = 1024              # band queries per core
NT = SQ + 1            # 1025 output tokens
QB = 256               # band query block
NQB = SQ // QB         # 4
NKC = 3                # 128-key chunks per band window
KPF = 64               # wrapped key/token prefix columns
NPAIR = H // 2         # 4 head-pair tiles
EPS = 1e-5
NEG = -1e30

F32 = mybir.dt.float32
F32R = mybir.dt.float32r
BF16 = mybir.dt.bfloat16
AF = mybir.ActivationFunctionType
ALU = mybir.AluOpType
AXL = mybir.AxisListType


def _r(ap):
    """Reinterpret an fp32 AP as float32r for full-rate PE matmuls."""
    return ap.bitcast(F32R)


def build_module():
    nc = bacc_mod.Bacc("TRN2", target_bir_lowering=False)

    x_nat = nc.dram_tensor("x_nat", [SK, D], F32, kind="ExternalInput")
    wq_pc = nc.dram_tensor("wq_pc", [P, DC, D], F32R, kind="ExternalInput")
    wk_pc = nc.dram_tensor("wk_pc", [P, DC, D], F32R, kind="ExternalInput")
    wv_pc = nc.dram_tensor("wv_pc", [P, DC, D], F32R, kind="ExternalInput")
    bq_pc = nc.dram_tensor("bq_pc", [P, DC], F32, kind="ExternalInput")
    bk_pc = nc.dram_tensor("bk_pc", [P, DC], F32, kind="ExternalInput")
    bv_row = nc.dram_tensor("bv_row", [1, D], F32R, kind="ExternalInput")
    wo_pc = nc.dram_tensor("wo_pc", [P, DC, D], F32R, kind="ExternalInput")
    bo_row = nc.dram_tensor("bo_row", [1, D], F32R, kind="ExternalInput")
    w1_pc = nc.dram_tensor("w1_pc", [P, DC, FF], BF16, kind="ExternalInput")
    b1_pc = nc.dram_tensor("b1_pc", [P, FFC], F32, kind="ExternalInput")
    w2_pc = nc.dram_tensor("w2_pc", [P, FFC, D], BF16, kind="ExternalInput")
    b2_row = nc.dram_tensor("b2_row", [1, D], F32R, kind="ExternalInput")
    mask_band = nc.dram_tensor("mask_band", [P, NQB, NKC, QB], BF16, kind="ExternalInput")
    mask_gcol1 = nc.dram_tensor("mask_gcol1", [1, SQ], BF16, kind="ExternalInput")
    mask_grow = nc.dram_tensor("mask_grow", [1, S], F32R, kind="ExternalInput")
    ones_in = nc.dram_tensor("ones_in", [1, P], F32R, kind="ExternalInput")
    onesp_in = nc.dram_tensor("onesp_in", [P, P], F32R, kind="ExternalInput")
    y_out = nc.dram_tensor("y", [NT, D], F32, kind="ExternalOutput")

    with tile.TileContext(nc) as tc, ExitStack() as ctx:
        # ---- persistent state ----
        persist = ctx.enter_context(tc.tile_pool(name="persist", bufs=1))
        ident = persist.tile([P, P], F32)
        make_identity(nc, ident)
        ones_row = persist.tile([1, P], F32R)
        nc.sync.dma_start(ones_row, ones_in[:])
        eps_t = persist.tile([P, 1], F32)
        nc.vector.memset(eps_t, EPS)
        bo_sb = persist.tile([1, D], F32R)
        nc.sync.dma_start(bo_sb, bo_row[:])
        b1_sb = persist.tile([P, FFC], F32)
        nc.sync.dma_start(b1_sb, b1_pc[:])
        b2_sb = persist.tile([1, D], F32R)
        nc.sync.dma_start(b2_sb, b2_row[:])

        oT = [persist.tile([P, NT], F32, name=f"oT{p}") for p in range(NPAIR)]
        x1 = persist.tile([P, 9, D], F32)
        z2T = [persist.tile([P, NT], BF16, name=f"z2T{d}") for d in range(DC)]
        pgT = persist.tile([P, 16, 8], BF16)
        onesP = persist.tile([P, P], F32R)
        nc.sync.dma_start(onesP, onesp_in[:])
        idb8 = persist.tile([8, 8], BF16)
        nc.gpsimd.tensor_copy(idb8, ident[0:8, 0:8])

        # ---- PSUM pool: mm(2) + sm(2) + sc(2) + smb(2) = 8 banks ----
        ps = ctx.enter_context(tc.tile_pool(name="ps", bufs=2, space="PSUM"))

        # ---- attention state (lives through band phase) ----
        attn = ctx.enter_context(tc.tile_pool(name="attn", bufs=1))
        QT = [attn.tile([P, NT], BF16, name=f"QT{p}") for p in range(NPAIR)]
        KTx = [attn.tile([P, KPF + SK], BF16, name=f"KTx{p}") for p in range(NPAIR)]
        Vsbx = attn.tile([P, 17, H, HD + 1], BF16)
        nc.gpsimd.memset(Vsbx[:, :, :, HD], 1.0)
        # per-head global-key V rows / probs at quarter-partition bases
        Vg = attn.tile([P, H, HD + 1], BF16)
        pgh3 = [attn.tile([P, SQ], BF16, name=f"pgh3_{t}") for t in range(3)]
        k0h = [attn.tile([P, 1], BF16, name=f"k0h{h}") for h in range(H)]
        mb_sb = attn.tile([P, NQB, NKC, QB], BF16)
        mgc_sb = attn.tile([P, SQ], BF16)
        mgrow_sb = attn.tile([1, S], F32R)

        # x loads on the gpsimd queue; weights/masks on the sync queue
        nc.sync.dma_start(mb_sb, mask_band[:])
        nc.sync.dma_start(mgc_sb[0:1, :], mask_gcol1[:])
        nc.sync.dma_start(mgrow_sb, mask_grow[:])
        for mrow in (32, 64):
            nc.gpsimd.tensor_copy(mgc_sb[mrow : mrow + 1, :], mgc_sb[0:1, :])

        # ====== Phase A+B: LN1 -> zTx, V (interleaved), Q/K ======
        with tc.tile_pool(name="phA", bufs=1) as pha, \
             tc.tile_pool(name="lnz", bufs=2) as z_pool, \
             tc.tile_pool(name="st1", bufs=4) as stat_pool:
            zTx = [pha.tile([P, KPF + SK], F32, name=f"zTx{d}") for d in range(DC)]
            wv_sb = pha.tile([P, DC, D], F32R)
            wq_sb = pha.tile([P, DC, D], F32R)
            wk_sb = pha.tile([P, DC, D], F32R)
            bq_sb = pha.tile([P, DC], F32)
            bk_sb = pha.tile([P, DC], F32)
            bv_sb = pha.tile([1, D], F32R)

            nc.gpsimd.dma_start(
                x1[:, 0:4, :], x_nat[0:512, :].rearrange("(t p) d -> p t d", t=4))
            nc.gpsimd.dma_start(
                x1[:, 4:8, :], x_nat[512:1024, :].rearrange("(t p) d -> p t d", t=4))
            nc.gpsimd.dma_start(x1[0:1, 8, :], x_nat[S : S + 1, :])
            nc.sync.dma_start(wv_sb, wv_pc[:])
            nc.sync.dma_start(bv_sb, bv_row[:])
            nc.sync.dma_start(wq_sb, wq_pc[:])
            nc.sync.dma_start(bq_sb, bq_pc[:])
            nc.sync.dma_start(wk_sb, wk_pc[:])
            nc.sync.dma_start(bk_sb, bk_pc[:])

            def vproj(j):
                """V projection for shifted key tile j (tokens [128j-64,128j+64))."""
                rows = P if j < 16 else 1
                col0 = 128 * j if j < 16 else KPF + S
                pv = ps.tile([P, D], F32, tag="mm")
                for d in range(DC):
                    nc.tensor.matmul(pv[:rows], _r(zTx[d][:, col0 : col0 + rows]),
                                     _r(wv_sb[:, d, :]),
                                     start=(d == 0), stop=False)
                nc.tensor.matmul(pv[:rows], _r(ones_row[:1, :rows]), _r(bv_sb),
                                 start=False, stop=True)
                nc.scalar.activation(
                    Vsbx[:rows, j, :, 0:HD],
                    pv[:rows].rearrange("p (h e) -> p h e", h=H), AF.Copy)

            def ln1(t):
                rows = P if t < 16 else 1
                if t < 8:
                    src = x1[:rows, t, :]
                elif t < 16:
                    xt = z_pool.tile([P, D], F32, tag="xt")
                    nc.gpsimd.dma_start(xt, x_nat[t * P : (t + 1) * P, :])
                    src = xt[:rows]
                else:
                    src = x1[0:1, 8, :]
                st = stat_pool.tile([P, 6], F32, tag="bnst")
                nc.vector.bn_stats(st[:rows], src)
                mv = stat_pool.tile([P, 2], F32, tag="bnmv")
                nc.vector.bn_aggr(mv[:rows], st[:rows])
                rstd = stat_pool.tile([P, 1], F32, tag="rstd")
                nc.scalar.activation(rstd[:rows], mv[:rows, 1:2], AF.Sqrt,
                                     bias=eps_t[:rows], scale=1.0)
                nc.vector.reciprocal(rstd[:rows], rstd[:rows])
                z = z_pool.tile([P, D], F32, tag="z")
                eng = nc.vector if (t % 2 == 0) else nc.gpsimd
                eng.tensor_scalar(z[:rows], src, mv[:rows, 0:1], rstd[:rows],
                                  op0=ALU.subtract, op1=ALU.mult)
                for d in range(DC):
                    pt = ps.tile([P, QB], F32, tag="sm")
                    nc.tensor.transpose(pt[:, :rows], z[:rows, d * P : (d + 1) * P],
                                        ident[:rows, :rows])
                    nc.scalar.activation(
                        zTx[d][:, KPF + t * P : KPF + t * P + rows].bitcast(F32R),
                        pt[:, :rows], AF.Copy)
                    if t == 15:
                        # wrapped prefix: tokens 1984..2048 = local rows 64..128
                        nc.vector.tensor_copy(
                            zTx[d][:, 0:KPF].bitcast(F32R), pt[:, HD:P])

            for t in range(17):
                ln1(t)
                if 1 <= t <= 15:
                    vproj(t)
                if t == 15:
                    vproj(0)
            vproj(16)

            # ---- Q/K projections ----
            q_blocks = [(KPF, 0, 512), (KPF + 512, 512, 512), (KPF + S, SQ, 1)]
            k_blocks = [(KPF + i * 512, KPF + i * 512, 512) for i in range(4)] \
                + [(KPF + S, KPF + S, 1)]
            for p in range(NPAIR):
                for w_sb, b_sb, dst, blocks in (
                        (wq_sb, bq_sb, QT[p], q_blocks),
                        (wk_sb, bk_sb, KTx[p], k_blocks)):
                    for bi, (src, dcol, w) in enumerate(blocks):
                        s0, w0, keep = (src, w, 0) if w > 1 else (src - 1, 2, 1)
                        pq = ps.tile([P, 512], F32, tag="mm")
                        for d in range(DC):
                            nc.tensor.matmul(pq[:, :w0],
                                             _r(w_sb[:, d, p * P : (p + 1) * P]),
                                             _r(zTx[d][:, s0 : s0 + w0]),
                                             start=(d == 0), stop=(d == DC - 1))
                        if bi % 2 == 0:
                            nc.scalar.activation(dst[:, dcol : dcol + w],
                                                 pq[:, keep : keep + w], AF.Identity,
                                                 bias=b_sb[:, p : p + 1])
                        else:
                            nc.vector.tensor_scalar(dst[:, dcol : dcol + w],
                                                    pq[:, keep : keep + w],
                                                    b_sb[:, p : p + 1], None,
                                                    op0=ALU.add)
                nc.gpsimd.tensor_copy(KTx[p][:, 0:KPF], KTx[p][:, S : S + KPF])

        # phase A scratch released; prefetch out_proj + FFN1 weights now
        wpool = ctx.enter_context(tc.tile_pool(name="wpool", bufs=1))
        wo_sb = wpool.tile([P, DC, D], F32R)
        nc.sync.dma_start(wo_sb, wo_pc[:])
        w1_sb = wpool.tile([P, DC, FF], BF16)
        nc.sync.dma_start(w1_sb, w1_pc[:])

        # ====== Phase C/D/E/F interleaved: band + global + norm + out_proj + LN2
        with tc.tile_pool(name="bandp", bufs=3) as band_pool, \
             tc.tile_pool(name="ln2z", bufs=2) as z2_pool, \
             tc.tile_pool(name="st2", bufs=4) as stat2_pool:

            # band-phase scratch: quarter-partition l tiles, global-path
            # per-pair scratch, zero-padded stationaries
            ltA = band_pool.tile([P, NQB, QB], F32, bufs=1)
            ltB = band_pool.tile([P, NQB, QB], F32, bufs=1)
            lrA = band_pool.tile([P, NQB, QB], F32, bufs=1)
            lrB = band_pool.tile([P, NQB, QB], F32, bufs=1)
            lq = band_pool.tile([P, NQB, QB], F32, bufs=1)
            egpp = [band_pool.tile([2, S], BF16, tag=f"egpp{p}", bufs=1,
                                   name=f"egpp{p}") for p in range(NPAIR)]
            laP = [band_pool.tile([2, 4], F32, tag=f"laP{p}", bufs=1,
                                  name=f"laP{p}") for p in range(NPAIR)]
            egp8 = band_pool.tile([8, S], BF16, bufs=1)
            la8 = band_pool.tile([8, 4], F32, bufs=1)
            ga8 = band_pool.tile([8, 1], F32, bufs=1)
            larec = band_pool.tile([8, 1], F32, bufs=1)

            # col j of q2g[p] holds head (2p+j)'s global-query column in its
            # own 64 contraction rows; k0h[h] is the zero-padded global-key
            # column; Vg packs each head's global-key V row (+ ones col) at
            # quarter-partition bases so band PV matmuls stay legal
            q2g = [band_pool.tile([P, 2], BF16, tag=f"q2g{p}", bufs=1,
                                  name=f"q2g{p}") for p in range(NPAIR)]
            for pr in range(NPAIR):
                nc.gpsimd.memset(q2g[pr], 0.0)
                for j in range(2):
                    sub = j * HD
                    h = 2 * pr + j
                    nc.gpsimd.tensor_copy(
                        q2g[pr][sub : sub + HD, j : j + 1],
                        QT[pr][sub : sub + HD, SQ : SQ + 1])
                    nc.gpsimd.memset(k0h[h], 0.0)
                    nc.gpsimd.tensor_copy(
                        k0h[h][sub : sub + HD, 0:1],
                        KTx[pr][sub : sub + HD, KPF + S : KPF + S + 1])
                    m = 32 * (h % 3)
                    nc.gpsimd.tensor_copy(Vg[m : m + 1, h, :],
                                          Vsbx[0:1, 16, h, :])

            # global-KEY column scores for all band queries (all 8 heads)
            for h in range(H):
                pr = h // 2
                m = 32 * (h % 3)
                pgX = pgh3[h // 3]
                for half in range(2):
                    gq = ps.tile([P, 512], F32, tag="mm")
                    nc.tensor.matmul(gq[0:1, :], k0h[h],
                                     QT[pr][:, half * 512 : (half + 1) * 512],
                                     start=True, stop=True)
                    nc.scalar.activation(
                        pgX[m : m + 1, half * 512 : (half + 1) * 512],
                        gq[0:1, :], AF.Exp)
                nc.vector.tensor_tensor(pgX[m : m + 1, :], pgX[m : m + 1, :],
                                        mgc_sb[m : m + 1, :], ALU.mult)

            def ln2(t):
                rows = P if t < 8 else 1
                src = x1[:rows, t, :]
                st = stat2_pool.tile([P, 6], F32, tag="bnst2")
                nc.vector.bn_stats(st[:rows], src)
                mv = stat2_pool.tile([P, 2], F32, tag="bnmv2")
                nc.vector.bn_aggr(mv[:rows], st[:rows])
                rstd = stat2_pool.tile([P, 1], F32, tag="rstd2")
                nc.scalar.activation(rstd[:rows], mv[:rows, 1:2], AF.Sqrt,
                                     bias=eps_t[:rows], scale=1.0)
                nc.vector.reciprocal(rstd[:rows], rstd[:rows])
                z2 = z2_pool.tile([P, D], F32, tag="z2")
                nc.gpsimd.tensor_scalar(z2[:rows], src, mv[:rows, 0:1], rstd[:rows],
                                        op0=ALU.subtract, op1=ALU.mult)
                for d in range(DC):
                    pt = ps.tile([P, QB], F32, tag="sm")
                    nc.tensor.transpose(pt[:, :rows], z2[:rows, d * P : (d + 1) * P],
                                        ident[:rows, :rows])
                    nc.scalar.activation(z2T[d][:, t * P : t * P + rows],
                                         pt[:, :rows], AF.Copy)

            def out_proj(t):
                w = P if t < 8 else 1
                yp = ps.tile([P, D], F32, tag="mm")
                if w > 1:
                    for p in range(NPAIR):
                        nc.tensor.matmul(yp[:w], _r(oT[p][:, t * P : t * P + w]),
                                         _r(wo_sb[:, p, :]),
                                         start=(p == 0), stop=False)
                else:
                    for p in range(NPAIR):
                        nc.tensor.matmul(yp[:w], _r(oT[p][:, SQ : SQ + 1]),
                                         _r(wo_sb[:, p, :]),
                                         start=(p == 0), stop=False)
                nc.tensor.matmul(yp[:w], _r(ones_row[:1, :w]), _r(bo_sb),
                                 start=False, stop=True)
                nc.vector.tensor_tensor(x1[:w, t, :], yp[:w], x1[:w, t, :], ALU.add)

            def global_scores(pr, tcb):
                gs = ps.tile([P, 512], F32, tag="mm")
                nc.tensor.matmul(gs[0:2, :], q2g[pr],
                                 KTx[pr][:, tcb * 512 : (tcb + 1) * 512],
                                 start=True, stop=False)
                # additive key mask is head-independent -> rank-1 matmul add
                nc.tensor.matmul(gs[0:2, :], _r(onesP[0:1, 0:2]),
                                 mgrow_sb[0:1, tcb * 512 : (tcb + 1) * 512],
                                 start=False, stop=True)
                nc.scalar.activation(
                    egpp[pr][:, tcb * 512 : (tcb + 1) * 512],
                    gs[0:2, :], AF.Exp,
                    accum_out=laP[pr][:, tcb : tcb + 1])

            def global_gather():
                for pr in range(NPAIR):
                    nc.gpsimd.dma_start(egp8[2 * pr : 2 * pr + 2, :], egpp[pr][:])
                    nc.gpsimd.dma_start(la8[2 * pr : 2 * pr + 2, :], laP[pr][:])

            def global_transposes():
                nc.vector.tensor_reduce(ga8, la8, axis=AXL.X, op=ALU.add)
                nc.vector.reciprocal(larec, ga8)
                for c in range(16):
                    ptb = ps.tile([P, QB], BF16, tag="smb", bufs=2)
                    nc.tensor.transpose(ptb[:, 0:8], egp8[0:8, c * P : (c + 1) * P],
                                        idb8)
                    nc.scalar.activation(pgT[:, c, :], ptb[:, 0:8], AF.Copy)

            def global_pv():
                for g in range(2):
                    pog = ps.tile([P, 512], F32, tag="mm")
                    for c in range(16):
                        nc.tensor.matmul(pog[0:8, 0 : 4 * HD], pgT[:, c, :],
                                         Vsbx[:, c, 4 * g : 4 * g + 4, 0:HD],
                                         start=(c == 0), stop=(c == 15))
                    pog_sb = band_pool.tile([8, 4 * HD], F32, tag="pog_sb")
                    nc.scalar.activation(pog_sb, pog[0:8, 0 : 4 * HD], AF.Copy,
                                         scale=larec[0:8, 0:1])
                    for j in range(2):
                        ptj = ps.tile([P, QB], F32, tag="sm")
                        nc.tensor.transpose(ptj[:, 0:8],
                                            pog_sb[0:8, j * P : (j + 1) * P],
                                            ident[0:8, 0:8])
                        for hh in (2 * j, 2 * j + 1):
                            h = 4 * g + hh
                            rlo = (hh % 2) * HD
                            nc.scalar.activation(
                                oT[h // 2][rlo : rlo + HD, SQ : SQ + 1].bitcast(F32R),
                                ptj[rlo : rlo + HD, h : h + 1], AF.Copy)

            for i in range(NQB):
                for h in range(H):
                    pr, sub = h // 2, (h % 2) * HD
                    q_ap = QT[pr][sub : sub + HD, i * QB : (i + 1) * QB]
                    sc = ps.tile([P, NKC, QB], F32, tag="sc", bufs=1)
                    for c in range(NKC):
                        nc.tensor.matmul(
                            sc[:, c, :],
                            KTx[pr][sub : sub + HD,
                                    QB * i + c * P : QB * i + c * P + P],
                            q_ap, start=True, stop=True)
                    pT = band_pool.tile([P, NKC, QB], BF16, tag="pT")
                    nc.scalar.activation(pT, sc, AF.Exp)
                    nc.vector.tensor_tensor(pT, pT, mb_sb[:, i, :, :], ALU.mult)
                    po = ps.tile([P, QB], F32, tag="sm")
                    for c in range(NKC):
                        nc.tensor.matmul(po[0 : HD + 1, :], Vsbx[:, 2 * i + c, h, :],
                                         pT[:, c, :], start=(c == 0), stop=False)
                    m = 32 * (h % 3)
                    pgX = pgh3[h // 3]
                    nc.tensor.matmul(po[0 : HD + 1, :], Vg[m : m + 1, h, :],
                                     pgX[m : m + 1, i * QB : (i + 1) * QB],
                                     start=False, stop=True)
                    if h % 2 == 0:
                        nc.vector.tensor_copy(
                            oT[pr][sub : sub + HD, i * QB : (i + 1) * QB].bitcast(F32R),
                            po[0:HD, :])
                    else:
                        nc.scalar.activation(
                            oT[pr][sub : sub + HD, i * QB : (i + 1) * QB].bitcast(F32R),
                            po[0:HD, :], AF.Copy)
                    ml = 32 * (h % 4)
                    ltX = ltA if h < 4 else ltB
                    nc.scalar.activation(
                        ltX[ml : ml + 1, i, :],
                        po[HD : HD + 1, :], AF.Copy)

                # normalize block i: partition-parallel reciprocals over the
                # quarter-row l tiles (DVE allows base 96); matmul operands
                # cannot sit at base 96, so heads 3/7 get moved to lq first
                with nc.allow_low_precision(reason="fp32r-rounded softmax sums"):
                    nc.vector.reciprocal(lrA[:, i, :].bitcast(F32R),
                                         ltA[:, i, :])
                    nc.vector.reciprocal(lrB[:, i, :].bitcast(F32R),
                                         ltB[:, i, :])
                nc.gpsimd.tensor_copy(lq[0:1, i, :].bitcast(F32R),
                                      lrA[96:97, i, :])
                nc.gpsimd.tensor_copy(lq[32:33, i, :].bitcast(F32R),
                                      lrB[96:97, i, :])

                def lsrc(h):
                    if h % 4 == 3:
                        return lq, 32 * (h // 4)
                    return (lrA if h < 4 else lrB), 32 * (h % 4)

                for p in range(NPAIR):
                    for j in range(2):
                        lt_t, r = lsrc(2 * p + j)
                        lbc = ps.tile([P, QB], F32, tag="sm")
                        nc.tensor.matmul(lbc[0:HD, :],
                                         _r(onesP[r : r + 1, 0:HD]),
                                         _r(lt_t[r : r + 1, i, :]),
                                         start=True, stop=True)
                        rows = oT[p][j * HD : (j + 1) * HD,
                                     i * QB : (i + 1) * QB]
                        nc.vector.tensor_tensor(rows.bitcast(F32R), rows,
                                                lbc[0:HD, :], ALU.mult)

                # pipeline: out_proj + LN2 for the two finished token tiles
                for t in (2 * i, 2 * i + 1):
                    out_proj(t)
                    ln2(t)

                # interleave the global-token path under the band blocks
                if i == 0:
                    for tcb in range(4):
                        global_scores(0, tcb)
                        global_scores(1, tcb)
                elif i == 1:
                    for tcb in range(4):
                        global_scores(2, tcb)
                        global_scores(3, tcb)
                    global_gather()
                elif i == 2:
                    global_transposes()
                else:
                    global_pv()

            out_proj(8)
            ln2(8)

        # attention state released; fetch FFN2 weights under out_proj/FFN1
        ffw = ctx.enter_context(tc.tile_pool(name="ffw", bufs=1))
        w2_sb = ffw.tile([P, FFC, D], BF16)
        nc.sync.dma_start(w2_sb, w2_pc[:])

        # ====== Phase G: FFN + residual -> y ======
        with tc.tile_pool(name="ffn", bufs=1) as ffn_pool, \
             tc.tile_pool(name="ffo", bufs=3) as out_pool:
            for t0, tw in [(0, 512), (512, 512), (SQ, 1)]:
                hT = ffn_pool.tile([P, FFC, 512], BF16, tag="hT")
                s0, w0, keep = (t0, tw, 0) if tw > 1 else (t0 - 1, 2, 1)
                for f in range(FFC):
                    ph = ps.tile([P, 512], F32, tag="mm")
                    for d in range(DC):
                        nc.tensor.matmul(ph[:, :w0],
                                         w1_sb[:, d, f * P : (f + 1) * P],
                                         z2T[d][:, s0 : s0 + w0],
                                         start=(d == 0), stop=(d == DC - 1))
                    nc.scalar.activation(hT[:, f, :tw],
                                         ph[:, keep : keep + tw], AF.Gelu,
                                         bias=b1_sb[:, f : f + 1])
                nsub = 4 if tw == 512 else 1
                for stp in range(nsub):
                    sw = P if tw == 512 else 1
                    ps2 = ps.tile([P, D], F32, tag="mm")
                    for f in range(FFC):
                        nc.tensor.matmul(ps2[:sw],
                                         hT[:, f, stp * P : stp * P + sw],
                                         w2_sb[:, f, :],
                                         start=(f == 0), stop=False)
                    nc.tensor.matmul(ps2[:sw], _r(ones_row[:1, :sw]), _r(b2_sb),
                                     start=False, stop=True)
                    yt = out_pool.tile([P, D], F32, tag="yt")
                    tglob = t0 // P + stp
                    nc.vector.tensor_tensor(yt[:sw], ps2[:sw],
                                            x1[:sw, tglob, :], ALU.add)
                    nc.sync.dma_start(y_out[t0 + stp * P : t0 + stp * P + sw, :],
                                      yt[:sw])

    nc.finalize()
    return nc


def make_host_inputs(x, padding_mask, attn_mask, in_proj_w, in_proj_b, out_proj_w,
                     out_proj_b, ln1_g, ln1_b, ln2_g, ln2_b, ff_w1, ff_b1, ff_w2,
                     ff_b2):
    """Build the 8 per-core input maps (numpy only)."""
    import ml_dtypes
    f32 = np.float32
    bf16 = ml_dtypes.bfloat16
    x = np.asarray(x, f32)
    attn_mask = np.asarray(attn_mask, f32)
    padding_mask = np.asarray(padding_mask, bool)

    g1 = np.asarray(ln1_g, f32); b1 = np.asarray(ln1_b, f32)
    g2 = np.asarray(ln2_g, f32); b2 = np.asarray(ln2_b, f32)
    Wq, Wk, Wv = (np.asarray(in_proj_w[i * D:(i + 1) * D], f32) for i in range(3))
    bq0, bk0, bv0 = (np.asarray(in_proj_b[i * D:(i + 1) * D], f32) for i in range(3))
    sc = 1.0 / np.sqrt(HD)

    Wq_ = Wq * g1[None, :] * sc
    bq_ = (Wq @ b1 + bq0) * sc
    Wk_ = Wk * g1[None, :]
    bk_ = Wk @ b1 + bk0
    Wv_ = Wv * g1[None, :]
    bv_ = Wv @ b1 + bv0
    W1_ = np.asarray(ff_w1, f32) * g2[None, :]
    b1f = np.asarray(ff_w1, f32) @ b2 + np.asarray(ff_b1, f32)

    def pc(wt, nchunk):  # [Dout, Din] -> [P, nchunk, Dout] chunked on Din
        return np.ascontiguousarray(
            wt.T.reshape(nchunk, P, wt.shape[0]).transpose(1, 0, 2))

    shared = {
        "wq_pc": pc(Wq_, DC), "wk_pc": pc(Wk_, DC), "wv_pc": pc(Wv_, DC),
        "bq_pc": np.ascontiguousarray(bq_.reshape(DC, P).T),
        "bk_pc": np.ascontiguousarray(bk_.reshape(DC, P).T),
        "bv_row": bv_[None, :].copy(),
        "wo_pc": pc(np.asarray(out_proj_w, f32), DC),
        "bo_row": np.asarray(out_proj_b, f32)[None, :].copy(),
        "w1_pc": pc(W1_, DC).astype(bf16),
        "b1_pc": np.ascontiguousarray(b1f.reshape(FFC, P).T),
        "w2_pc": pc(np.asarray(ff_w2, f32), FFC).astype(bf16),
        "b2_row": np.asarray(ff_b2, f32)[None, :].copy(),
        "ones_in": np.ones((1, P), f32),
        "onesp_in": np.ones((P, P), f32),
    }

    in_maps = []
    for core in range(8):
        b = core // 2
        h = core % 2
        rot = np.roll(x[b], -1024 * h, axis=0)
        x_nat = np.ascontiguousarray(np.concatenate([rot, x[b, 0:1]], axis=0))

        # additive mask for this batch -> multiplicative factor
        A = attn_mask + np.where(padding_mask[b], -np.inf, 0.0)[None, :]
        mfac = np.exp(np.minimum(A, 0.0)).astype(f32)  # exp(-inf)=0, exp(0)=1
        mfac[~np.isfinite(A)] = 0.0

        # band masks: [P(t), NQB(i), NKC(c), QB(r)];
        # chunk c of block i covers rotated keys [256i - 64 + 128c, +128)
        i_idx = np.arange(NQB)[:, None, None, None]
        c_idx = np.arange(NKC)[None, :, None, None]
        t_idx = np.arange(P)[None, None, :, None]
        r_idx = np.arange(QB)[None, None, None, :]
        k_rot = (QB * i_idx - KPF + P * c_idx + t_idx) % S
        q_rot = i_idx * QB + r_idx
        gq = (q_rot + 1024 * h) % S
        gk = (k_rot + 1024 * h) % S
        band = mfac[gq, gk]                       # [NQB, NKC, P, QB]
        mask_band = np.ascontiguousarray(band.transpose(2, 0, 1, 3)).astype(bf16)

        # global-key column mask, zeroed when key0 falls inside the window
        key0_rot = (0 - 1024 * h) % S
        gq2 = (np.arange(NQB)[:, None] * QB + np.arange(QB)[None, :] + 1024 * h) % S
        gcol = mfac[gq2, 0].copy()
        for i in range(NQB):
            off = (key0_rot - (QB * i - KPF)) % S
            if off < NKC * P:
                gcol[i, :] = 0.0  # key 0 already inside this block's band window
        mask_gcol1 = np.ascontiguousarray(gcol.reshape(1, -1)).astype(bf16)

        # global-query additive mask row, in SHIFTED key order:
        # score col kappa <-> rotated key (kappa - 64) mod S
        kap = np.arange(S)
        k_act = (((kap - KPF) % S) + 1024 * h) % S
        mask_grow = np.ascontiguousarray(
            np.maximum(A[0, k_act], NEG)[None, :].astype(f32))

        m = dict(shared)
        m.update({
            "x_nat": x_nat,
            "mask_band": mask_band,
            "mask_gcol1": mask_gcol1,
            "mask_grow": mask_grow,
        })
        in_maps.append(m)
    return in_maps


def assemble_output(results):
    """results: list of 8 dicts with 'y' [NT, D] -> full [B, S, D]."""
    out = np.empty((B, S, D), np.float32)
    for b in range(B):
        y0 = results[2 * b]["y"]
        y1 = results[2 * b + 1]["y"]
        out[b, 0] = y0[SQ]
        out[b, 1:SQ] = y0[1:SQ]
        out[b, SQ:] = y1[0:SQ]
    return out


_CACHED_NC = None


def kernel(**inputs) -> np.ndarray:
    global _CACHED_NC
    from concourse.bass_utils import run_bass_kernel_spmd

    in_maps = make_host_inputs(**inputs)
    if _CACHED_NC is None:
        _CACHED_NC = build_module()
    res = run_bass_kernel_spmd(_CACHED_NC, in_maps, core_ids=list(range(8)))
    return assemble_output(res.results)


if __name__ == "__main__":
    nc = build_module()
    print("build + compile OK")


# revision 32
# speedup vs baseline: 1.1213x; 1.1143x over previous
"""LocalGlobalTransformerEncoderBlock on 8 Trainium2 NeuronCores.

Sharding: core = (batch b = core//2, sequence half h = core%2). Each core
computes the full encoder block for 1024 query rows of one batch plus the
global token (sequence position 0). The per-core sequence is ROTATED by
1024*h so the core's query rows are always rotated rows [0, 1024), and
x[b, 0] (the global token) is appended as row 2048.

v2 redesign vs the 481us baseline:
  - Band attention uses 3 unaligned 128-key chunks per 256-query block
    (window [256i-64, 256i+320) mod 2048) via a 64-col wrapped prefix on
    the transposed K / z buffers and half-shifted V key tiles.
  - Attention tensors (Q^T, K^T, V, probs, masks) and FFN weights/hidden
    are bf16: same PE rate as fp32r but half the SBUF/DMA and 2x DVE.
  - Softmax denominators are gathered into a [32, 256] tile so ONE
    partition-parallel reciprocal replaces 16 serial [64,512] ones.
  - The global-token path is per-head wide matmuls + one fused softmax,
    emitted interleaved with the band blocks so it hides under them.
  - Normalize/out_proj/LN2 are emitted per band block (software pipeline);
    FFN weights prefetch during attention; x is loaded once into the
    residual buffer.

Self-contained: only imports from /opt/trn_rl_repo (the installed bass
runtime), numpy, and stdlib.
"""

import sys
from contextlib import ExitStack

if "/opt/trn_rl_repo" not in sys.path:
    sys.path.insert(0, "/opt/trn_rl_repo")

import numpy as np

import concourse.bass as bass
import concourse.bacc as bacc_mod
import concourse.mybir as mybir
import concourse.tile as tile
from concourse.masks import make_identity

P = 128
B, S, D, H, FF = 4, 2048, 512, 8, 2048
HD = D // H            # 64
DC = D // P            # 4 chunks of the model dim
FFC = FF // P          # 16 chunks of the FF dim
SK = S + 1             # 2049 tokens (2048 rotated + appended global token)
SQ = 1024              # band queries per core
NT = SQ + 1            # 1025 output tokens
QB = 256               # band query block
NQB = SQ // QB         # 4
NKC = 3                # 128-key chunks per band window
KPF = 64               # wrapped key/token prefix columns
NPAIR = H // 2         # 4 head-pair tiles
EPS = 1e-5
NEG = -1e30

F32 = mybir.dt.float32
F32R = mybir.dt.float32r
BF16 = mybir.dt.bfloat16
AF = mybir.ActivationFunctionType
ALU = mybir.AluOpType
AXL = mybir.AxisListType


def _r(ap):
    """Reinterpret an fp32 AP as float32r for full-rate PE matmuls."""
    return ap.bitcast(F32R)


def build_module():
    nc = bacc_mod.Bacc("TRN2", target_bir_lowering=False)

    x_nat = nc.dram_tensor("x_nat", [SK, D], F32, kind="ExternalInput")
    wq_pc = nc.dram_tensor("wq_pc", [P, DC, D], BF16, kind="ExternalInput")
    wk_pc = nc.dram_tensor("wk_pc", [P, DC, D], BF16, kind="ExternalInput")
    wv_pc = nc.dram_tensor("wv_pc", [P, DC, D], BF16, kind="ExternalInput")
    bq_pc = nc.dram_tensor("bq_pc", [P, DC], F32, kind="ExternalInput")
    bk_pc = nc.dram_tensor("bk_pc", [P, DC], F32, kind="ExternalInput")
    bv_row = nc.dram_tensor("bv_row", [1, D], F32R, kind="ExternalInput")
    wo_pc = nc.dram_tensor("wo_pc", [P, DC, D], F32R, kind="ExternalInput")
    bo_row = nc.dram_tensor("bo_row", [1, D], F32R, kind="ExternalInput")
    w1_pc = nc.dram_tensor("w1_pc", [P, DC, FF], BF16, kind="ExternalInput")
    b1_pc = nc.dram_tensor("b1_pc", [P, FFC], F32, kind="ExternalInput")
    w2_pc = nc.dram_tensor("w2_pc", [P, FFC, D], BF16, kind="ExternalInput")
    b2_row = nc.dram_tensor("b2_row", [1, D], F32R, kind="ExternalInput")
    mask_band = nc.dram_tensor("mask_band", [P, NQB, NKC, QB], BF16, kind="ExternalInput")
    mask_gcol1 = nc.dram_tensor("mask_gcol1", [1, SQ], BF16, kind="ExternalInput")
    mask_grow = nc.dram_tensor("mask_grow", [1, S], F32R, kind="ExternalInput")
    ones_in = nc.dram_tensor("ones_in", [1, P], F32R, kind="ExternalInput")
    onesp_in = nc.dram_tensor("onesp_in", [P, P], F32R, kind="ExternalInput")
    y_out = nc.dram_tensor("y", [NT, D], F32, kind="ExternalOutput")

    with tile.TileContext(nc) as tc, ExitStack() as ctx:
        # ---- persistent state ----
        persist = ctx.enter_context(tc.tile_pool(name="persist", bufs=1))
        ident = persist.tile([P, P], F32)
        make_identity(nc, ident)
        ones_row = persist.tile([1, P], F32R)
        nc.sync.dma_start(ones_row, ones_in[:])
        eps_t = persist.tile([P, 1], F32)
        nc.vector.memset(eps_t, EPS)
        bo_sb = persist.tile([1, D], F32R)
        nc.sync.dma_start(bo_sb, bo_row[:])
        b1_sb = persist.tile([P, FFC], F32)
        nc.sync.dma_start(b1_sb, b1_pc[:])
        b2_sb = persist.tile([1, D], F32R)
        nc.sync.dma_start(b2_sb, b2_row[:])

        oT = [persist.tile([P, NT], F32, name=f"oT{p}") for p in range(NPAIR)]
        x1 = persist.tile([P, 9, D], F32)
        z2T = [persist.tile([P, NT], BF16, name=f"z2T{d}") for d in range(DC)]
        pgT = persist.tile([P, 16, 8], BF16)
        onesP = persist.tile([P, P], F32R)
        nc.sync.dma_start(onesP, onesp_in[:])
        idb8 = persist.tile([8, 8], BF16)
        nc.gpsimd.tensor_copy(idb8, ident[0:8, 0:8])

        # ---- PSUM pool: mm(2) + sm(2) + sc(2) + smb(2) = 8 banks ----
        ps = ctx.enter_context(tc.tile_pool(name="ps", bufs=2, space="PSUM"))

        # ---- attention state (lives through band phase) ----
        attn = ctx.enter_context(tc.tile_pool(name="attn", bufs=1))
        QT = [attn.tile([P, NT], BF16, name=f"QT{p}") for p in range(NPAIR)]
        KTx = [attn.tile([P, KPF + SK], BF16, name=f"KTx{p}") for p in range(NPAIR)]
        Vsbx = attn.tile([P, 17, H, HD + 1], BF16)
        nc.gpsimd.memset(Vsbx[:, :, :, HD], 1.0)
        # per-head global-key V rows / probs at quarter-partition bases
        Vg = attn.tile([P, H, HD + 1], BF16)
        pgh3 = [attn.tile([P, SQ], BF16, name=f"pgh3_{t}") for t in range(3)]
        k0h = [attn.tile([P, 1], BF16, name=f"k0h{h}") for h in range(H)]
        mb_sb = attn.tile([P, NQB, NKC, QB], BF16)
        mgc_sb = attn.tile([P, SQ], BF16)
        mgrow_sb = attn.tile([1, S], F32R)

        wo_sb = attn.tile([P, DC, D], F32R)
        w1_sb = attn.tile([P, DC, FF], BF16)

        # x loads on the gpsimd queue; weights/masks on the sync queue
        nc.sync.dma_start(mb_sb, mask_band[:])
        nc.sync.dma_start(mgc_sb[0:1, :], mask_gcol1[:])
        nc.sync.dma_start(mgrow_sb, mask_grow[:])
        for mrow in (32, 64):
            nc.vector.tensor_copy(mgc_sb[mrow : mrow + 1, :], mgc_sb[0:1, :])

        # ====== Phase A+B: LN1 -> zTx, V (interleaved), Q/K ======
        with tc.tile_pool(name="phA", bufs=1) as pha, \
             tc.tile_pool(name="lnz", bufs=2) as z_pool, \
             tc.tile_pool(name="st1", bufs=4) as stat_pool:
            zTx = [pha.tile([P, KPF + SK], BF16, name=f"zTx{d}") for d in range(DC)]
            wv_sb = pha.tile([P, DC, D], BF16)
            wq_sb = pha.tile([P, DC, D], BF16)
            wk_sb = pha.tile([P, DC, D], BF16)
            bq_sb = pha.tile([P, DC], F32)
            bk_sb = pha.tile([P, DC], F32)
            bv_sb = pha.tile([1, D], F32R)

            nc.gpsimd.dma_start(
                x1[:, 0:4, :], x_nat[0:512, :].rearrange("(t p) d -> p t d", t=4))
            nc.gpsimd.dma_start(
                x1[:, 4:8, :], x_nat[512:1024, :].rearrange("(t p) d -> p t d", t=4))
            nc.gpsimd.dma_start(x1[0:1, 8, :], x_nat[S : S + 1, :])
            nc.sync.dma_start(wv_sb, wv_pc[:])
            nc.sync.dma_start(bv_sb, bv_row[:])
            nc.sync.dma_start(wq_sb, wq_pc[:])
            nc.sync.dma_start(bq_sb, bq_pc[:])
            nc.sync.dma_start(wk_sb, wk_pc[:])
            nc.sync.dma_start(bk_sb, bk_pc[:])
            nc.sync.dma_start(wo_sb, wo_pc[:])
            nc.sync.dma_start(w1_sb, w1_pc[:])

            def vproj(j):
                """V projection for shifted key tile j (tokens [128j-64,128j+64))."""
                rows = P if j < 16 else 1
                col0 = 128 * j if j < 16 else KPF + S
                pv = ps.tile([P, D], F32, tag="mm")
                for d in range(DC):
                    nc.tensor.matmul(pv[:rows], zTx[d][:, col0 : col0 + rows],
                                     wv_sb[:, d, :],
                                     start=(d == 0), stop=False)
                nc.tensor.matmul(pv[:rows], _r(ones_row[:1, :rows]), _r(bv_sb),
                                 start=False, stop=True)
                nc.scalar.activation(
                    Vsbx[:rows, j, :, 0:HD],
                    pv[:rows].rearrange("p (h e) -> p h e", h=H), AF.Copy)

            def ln1(t):
                rows = P if t < 16 else 1
                if t < 8:
                    src = x1[:rows, t, :]
                elif t < 16:
                    xt = z_pool.tile([P, D], F32, tag="xt")
                    nc.gpsimd.dma_start(xt, x_nat[t * P : (t + 1) * P, :])
                    src = xt[:rows]
                else:
                    src = x1[0:1, 8, :]
                st = stat_pool.tile([P, 6], F32, tag="bnst")
                nc.vector.bn_stats(st[:rows], src)
                mv = stat_pool.tile([P, 2], F32, tag="bnmv")
                nc.vector.bn_aggr(mv[:rows], st[:rows])
                rstd = stat_pool.tile([P, 1], F32, tag="rstd")
                nc.scalar.activation(rstd[:rows], mv[:rows, 1:2], AF.Sqrt,
                                     bias=eps_t[:rows], scale=1.0)
                nc.vector.reciprocal(rstd[:rows], rstd[:rows])
                z = z_pool.tile([P, D], F32, tag="z")
                nc.vector.tensor_scalar(z[:rows], src, mv[:rows, 0:1],
                                        rstd[:rows],
                                        op0=ALU.subtract, op1=ALU.mult)
                for d in range(DC):
                    pt = ps.tile([P, QB], F32, tag="sm")
                    nc.tensor.transpose(pt[:, :rows], z[:rows, d * P : (d + 1) * P],
                                        ident[:rows, :rows])
                    nc.scalar.activation(
                        zTx[d][:, KPF + t * P : KPF + t * P + rows],
                        pt[:, :rows], AF.Copy)
                    if t == 15:
                        # wrapped prefix: tokens 1984..2048 = local rows 64..128
                        nc.vector.tensor_copy(zTx[d][:, 0:KPF], pt[:, HD:P])

            for t in range(17):
                ln1(t)
                if 1 <= t <= 15:
                    vproj(t)
                if t == 15:
                    vproj(0)
            vproj(16)

            # ---- Q/K projections ----
            q_blocks = [(KPF, 0, 512), (KPF + 512, 512, 512), (KPF + S, SQ, 1)]
            k_blocks = [(KPF + i * 512, KPF + i * 512, 512) for i in range(4)] \
                + [(KPF + S, KPF + S, 1)]
            for p in range(NPAIR):
                for w_sb, b_sb, dst, blocks in (
                        (wq_sb, bq_sb, QT[p], q_blocks),
                        (wk_sb, bk_sb, KTx[p], k_blocks)):
                    for bi, (src, dcol, w) in enumerate(blocks):
                        s0, w0, keep = (src, w, 0) if w > 1 else (src - 1, 2, 1)
                        pq = ps.tile([P, 512], F32, tag="mm")
                        for d in range(DC):
                            nc.tensor.matmul(pq[:, :w0],
                                             w_sb[:, d, p * P : (p + 1) * P],
                                             zTx[d][:, s0 : s0 + w0],
                                             start=(d == 0), stop=(d == DC - 1))
                        if bi % 2 == 0:
                            nc.scalar.activation(dst[:, dcol : dcol + w],
                                                 pq[:, keep : keep + w], AF.Identity,
                                                 bias=b_sb[:, p : p + 1])
                        else:
                            nc.vector.tensor_scalar(dst[:, dcol : dcol + w],
                                                    pq[:, keep : keep + w],
                                                    b_sb[:, p : p + 1], None,
                                                    op0=ALU.add)
                nc.vector.tensor_copy(KTx[p][:, 0:KPF], KTx[p][:, S : S + KPF])

        # ====== Phase C/D/E/F interleaved: band + global + norm + out_proj + LN2
        with tc.tile_pool(name="bandp", bufs=3) as band_pool, \
             tc.tile_pool(name="ln2z", bufs=2) as z2_pool, \
             tc.tile_pool(name="st2", bufs=4) as stat2_pool:

            # band-phase scratch: quarter-partition l tiles, global-path
            # per-pair scratch, zero-padded stationaries
            ltA = band_pool.tile([P, NQB, QB], F32, bufs=1)
            ltB = band_pool.tile([P, NQB, QB], F32, bufs=1)
            lrA = band_pool.tile([P, NQB, QB], F32, bufs=1)
            lrB = band_pool.tile([P, NQB, QB], F32, bufs=1)
            lq = band_pool.tile([P, NQB, QB], F32, bufs=1)
            egpp = [band_pool.tile([2, S], BF16, tag=f"egpp{p}", bufs=1,
                                   name=f"egpp{p}") for p in range(NPAIR)]
            laP = [band_pool.tile([2, 4], F32, tag=f"laP{p}", bufs=1,
                                  name=f"laP{p}") for p in range(NPAIR)]
            egp8 = band_pool.tile([8, S], BF16, bufs=1)
            la8 = band_pool.tile([8, 4], F32, bufs=1)
            ga8 = band_pool.tile([8, 1], F32, bufs=1)
            larec = band_pool.tile([8, 1], F32, bufs=1)

            # col j of q2g[p] holds head (2p+j)'s global-query column in its
            # own 64 contraction rows; k0h[h] is the zero-padded global-key
            # column; Vg packs each head's global-key V row (+ ones col) at
            # quarter-partition bases so band PV matmuls stay legal
            q2g = [band_pool.tile([P, 2], BF16, tag=f"q2g{p}", bufs=1,
                                  name=f"q2g{p}") for p in range(NPAIR)]
            for pr in range(NPAIR):
                nc.gpsimd.memset(q2g[pr], 0.0)
                for j in range(2):
                    sub = j * HD
                    h = 2 * pr + j
                    nc.gpsimd.tensor_copy(
                        q2g[pr][sub : sub + HD, j : j + 1],
                        QT[pr][sub : sub + HD, SQ : SQ + 1])
                    nc.gpsimd.memset(k0h[h], 0.0)
                    nc.gpsimd.tensor_copy(
                        k0h[h][sub : sub + HD, 0:1],
                        KTx[pr][sub : sub + HD, KPF + S : KPF + S + 1])
                    m = 32 * (h % 3)
                    nc.gpsimd.tensor_copy(Vg[m : m + 1, h, :],
                                          Vsbx[0:1, 16, h, :])

            # global-KEY column scores for all band queries (all 8 heads)
            for h in range(H):
                pr = h // 2
                m = 32 * (h % 3)
                pgX = pgh3[h // 3]
                for half in range(2):
                    gq = ps.tile([P, 512], F32, tag="mm")
                    nc.tensor.matmul(gq[0:1, :], k0h[h],
                                     QT[pr][:, half * 512 : (half + 1) * 512],
                                     start=True, stop=True)
                    nc.scalar.activation(
                        pgX[m : m + 1, half * 512 : (half + 1) * 512],
                        gq[0:1, :], AF.Exp)
                nc.vector.tensor_tensor(pgX[m : m + 1, :], pgX[m : m + 1, :],
                                        mgc_sb[m : m + 1, :], ALU.mult)

            def ln2(t):
                rows = P if t < 8 else 1
                src = x1[:rows, t, :]
                st = stat2_pool.tile([P, 6], F32, tag="bnst2")
                nc.vector.bn_stats(st[:rows], src)
                mv = stat2_pool.tile([P, 2], F32, tag="bnmv2")
                nc.vector.bn_aggr(mv[:rows], st[:rows])
                rstd = stat2_pool.tile([P, 1], F32, tag="rstd2")
                nc.scalar.activation(rstd[:rows], mv[:rows, 1:2], AF.Sqrt,
                                     bias=eps_t[:rows], scale=1.0)
                nc.vector.reciprocal(rstd[:rows], rstd[:rows])
                z2 = z2_pool.tile([P, D], F32, tag="z2")
                nc.vector.tensor_scalar(z2[:rows], src, mv[:rows, 0:1],
                                        rstd[:rows],
                                        op0=ALU.subtract, op1=ALU.mult)
                for d in range(DC):
                    pt = ps.tile([P, QB], F32, tag="sm")
                    nc.tensor.transpose(pt[:, :rows], z2[:rows, d * P : (d + 1) * P],
                                        ident[:rows, :rows])
                    nc.scalar.activation(z2T[d][:, t * P : t * P + rows],
                                         pt[:, :rows], AF.Copy)

            def out_proj(t):
                w = P if t < 8 else 1
                yp = ps.tile([P, D], F32, tag="mm")
                if w > 1:
                    for p in range(NPAIR):
                        nc.tensor.matmul(yp[:w], _r(oT[p][:, t * P : t * P + w]),
                                         _r(wo_sb[:, p, :]),
                                         start=(p == 0), stop=False)
                else:
                    for p in range(NPAIR):
                        nc.tensor.matmul(yp[:w], _r(oT[p][:, SQ : SQ + 1]),
                                         _r(wo_sb[:, p, :]),
                                         start=(p == 0), stop=False)
                nc.tensor.matmul(yp[:w], _r(ones_row[:1, :w]), _r(bo_sb),
                                 start=False, stop=True)
                nc.vector.tensor_tensor(x1[:w, t, :], yp[:w], x1[:w, t, :], ALU.add)

            def global_scores(pr, tcb):
                gs = ps.tile([P, 512], F32, tag="mm")
                nc.tensor.matmul(gs[0:2, :], q2g[pr],
                                 KTx[pr][:, tcb * 512 : (tcb + 1) * 512],
                                 start=True, stop=False)
                # additive key mask is head-independent -> rank-1 matmul add
                nc.tensor.matmul(gs[0:2, :], _r(onesP[0:1, 0:2]),
                                 mgrow_sb[0:1, tcb * 512 : (tcb + 1) * 512],
                                 start=False, stop=True)
                nc.scalar.activation(
                    egpp[pr][:, tcb * 512 : (tcb + 1) * 512],
                    gs[0:2, :], AF.Exp,
                    accum_out=laP[pr][:, tcb : tcb + 1])

            def global_gather():
                for pr in range(NPAIR):
                    nc.gpsimd.dma_start(egp8[2 * pr : 2 * pr + 2, :], egpp[pr][:])
                    nc.gpsimd.dma_start(la8[2 * pr : 2 * pr + 2, :], laP[pr][:])

            def global_transposes():
                nc.vector.tensor_reduce(ga8, la8, axis=AXL.X, op=ALU.add)
                nc.vector.reciprocal(larec, ga8)
                for c in range(16):
                    ptb = ps.tile([P, QB], BF16, tag="smb", bufs=2)
                    nc.tensor.transpose(ptb[:, 0:8], egp8[0:8, c * P : (c + 1) * P],
                                        idb8)
                    nc.scalar.activation(pgT[:, c, :], ptb[:, 0:8], AF.Copy)

            def global_pv():
                for g in range(2):
                    pog = ps.tile([P, 512], F32, tag="mm")
                    for c in range(16):
                        nc.tensor.matmul(pog[0:8, 0 : 4 * HD], pgT[:, c, :],
                                         Vsbx[:, c, 4 * g : 4 * g + 4, 0:HD],
                                         start=(c == 0), stop=(c == 15))
                    pog_sb = band_pool.tile([8, 4 * HD], F32, tag="pog_sb")
                    nc.scalar.activation(pog_sb, pog[0:8, 0 : 4 * HD], AF.Copy,
                                         scale=larec[0:8, 0:1])
                    for j in range(2):
                        ptj = ps.tile([P, QB], F32, tag="sm")
                        nc.tensor.transpose(ptj[:, 0:8],
                                            pog_sb[0:8, j * P : (j + 1) * P],
                                            ident[0:8, 0:8])
                        for hh in (2 * j, 2 * j + 1):
                            h = 4 * g + hh
                            rlo = (hh % 2) * HD
                            nc.scalar.activation(
                                oT[h // 2][rlo : rlo + HD, SQ : SQ + 1].bitcast(F32R),
                                ptj[rlo : rlo + HD, h : h + 1], AF.Copy)

            for i in range(NQB):
                for h in range(H):
                    pr, sub = h // 2, (h % 2) * HD
                    q_ap = QT[pr][sub : sub + HD, i * QB : (i + 1) * QB]
                    sc = ps.tile([P, NKC, QB], F32, tag="sc", bufs=1)
                    for c in range(NKC):
                        nc.tensor.matmul(
                            sc[:, c, :],
                            KTx[pr][sub : sub + HD,
                                    QB * i + c * P : QB * i + c * P + P],
                            q_ap, start=True, stop=True)
                    pT = band_pool.tile([P, NKC, QB], BF16, tag="pT")
                    nc.scalar.activation(pT, sc, AF.Exp)
                    nc.vector.tensor_tensor(pT, pT, mb_sb[:, i, :, :], ALU.mult)
                    po = ps.tile([P, QB], F32, tag="sm")
                    for c in range(NKC):
                        nc.tensor.matmul(po[0 : HD + 1, :], Vsbx[:, 2 * i + c, h, :],
                                         pT[:, c, :], start=(c == 0), stop=False)
                    m = 32 * (h % 3)
                    pgX = pgh3[h // 3]
                    nc.tensor.matmul(po[0 : HD + 1, :], Vg[m : m + 1, h, :],
                                     pgX[m : m + 1, i * QB : (i + 1) * QB],
                                     start=False, stop=True)
                    if h % 2 == 0:
                        nc.vector.tensor_copy(
                            oT[pr][sub : sub + HD, i * QB : (i + 1) * QB].bitcast(F32R),
                            po[0:HD, :])
                    else:
                        nc.scalar.activation(
                            oT[pr][sub : sub + HD, i * QB : (i + 1) * QB].bitcast(F32R),
                            po[0:HD, :], AF.Copy)
                    ml = 32 * (h % 4)
                    ltX = ltA if h < 4 else ltB
                    nc.scalar.activation(
                        ltX[ml : ml + 1, i, :],
                        po[HD : HD + 1, :], AF.Copy)

                # normalize block i: partition-parallel reciprocals over the
                # quarter-row l tiles (DVE allows base 96); matmul operands
                # cannot sit at base 96, so heads 3/7 get moved to lq first
                with nc.allow_low_precision(reason="fp32r-rounded softmax sums"):
                    nc.vector.reciprocal(lrA[:, i, :].bitcast(F32R),
                                         ltA[:, i, :])
                    nc.vector.reciprocal(lrB[:, i, :].bitcast(F32R),
                                         ltB[:, i, :])
                nc.vector.tensor_copy(lq[0:1, i, :].bitcast(F32R),
                                      lrA[96:97, i, :])
                nc.vector.tensor_copy(lq[32:33, i, :].bitcast(F32R),
                                      lrB[96:97, i, :])

                def lsrc(h):
                    if h % 4 == 3:
                        return lq, 32 * (h // 4)
                    return (lrA if h < 4 else lrB), 32 * (h % 4)

                for p in range(NPAIR):
                    for j in range(2):
                        lt_t, r = lsrc(2 * p + j)
                        lbc = ps.tile([P, QB], F32, tag="sm")
                        nc.tensor.matmul(lbc[0:HD, :],
                                         _r(onesP[r : r + 1, 0:HD]),
                                         _r(lt_t[r : r + 1, i, :]),
                                         start=True, stop=True)
                        rows = oT[p][j * HD : (j + 1) * HD,
                                     i * QB : (i + 1) * QB]
                        nc.vector.tensor_tensor(rows.bitcast(F32R), rows,
                                                lbc[0:HD, :], ALU.mult)

                # pipeline: out_proj + LN2 for the two finished token tiles
                for t in (2 * i, 2 * i + 1):
                    out_proj(t)
                    ln2(t)

                # interleave the global-token path under the band blocks
                if i == 0:
                    for tcb in range(4):
                        global_scores(0, tcb)
                        global_scores(1, tcb)
                elif i == 1:
                    for tcb in range(4):
                        global_scores(2, tcb)
                        global_scores(3, tcb)
                    global_gather()
                elif i == 2:
                    global_transposes()
                else:
                    global_pv()

            out_proj(8)
            ln2(8)

        # band scratch released; fetch FFN2 weights under out_proj/FFN1
        ffw = ctx.enter_context(tc.tile_pool(name="ffw", bufs=1))
        w2_sb = ffw.tile([P, FFC, D], BF16)
        nc.sync.dma_start(w2_sb, w2_pc[:])

        # ====== Phase G: FFN + residual -> y ======
        with tc.tile_pool(name="ffn", bufs=1) as ffn_pool, \
             tc.tile_pool(name="ffo", bufs=3) as out_pool:
            for t0, tw in [(0, 512), (512, 512), (SQ, 1)]:
                hT = ffn_pool.tile([P, FFC, 512], BF16, tag="hT")
                s0, w0, keep = (t0, tw, 0) if tw > 1 else (t0 - 1, 2, 1)
                for f in range(FFC):
                    ph = ps.tile([P, 512], F32, tag="mm")
                    for d in range(DC):
                        nc.tensor.matmul(ph[:, :w0],
                                         w1_sb[:, d, f * P : (f + 1) * P],
                                         z2T[d][:, s0 : s0 + w0],
                                         start=(d == 0), stop=(d == DC - 1))
                    nc.scalar.activation(hT[:, f, :tw],
                                         ph[:, keep : keep + tw], AF.Gelu,
                                         bias=b1_sb[:, f : f + 1])
                nsub = 4 if tw == 512 else 1
                for stp in range(nsub):
                    sw = P if tw == 512 else 1
                    ps2 = ps.tile([P, D], F32, tag="mm")
                    for f in range(FFC):
                        nc.tensor.matmul(ps2[:sw],
                                         hT[:, f, stp * P : stp * P + sw],
                                         w2_sb[:, f, :],
                                         start=(f == 0), stop=False)
                    nc.tensor.matmul(ps2[:sw], _r(ones_row[:1, :sw]), _r(b2_sb),
                                     start=False, stop=True)
                    yt = out_pool.tile([P, D], F32, tag="yt")
                    tglob = t0 // P + stp
                    nc.vector.tensor_tensor(yt[:sw], ps2[:sw],
                                            x1[:sw, tglob, :], ALU.add)
                    nc.gpsimd.dma_start(
                        y_out[t0 + stp * P : t0 + stp * P + sw, :], yt[:sw])

    nc.finalize()
    return nc


def make_host_inputs(x, padding_mask, attn_mask, in_proj_w, in_proj_b, out_proj_w,
                     out_proj_b, ln1_g, ln1_b, ln2_g, ln2_b, ff_w1, ff_b1, ff_w2,
                     ff_b2):
    """Build the 8 per-core input maps (numpy only)."""
    import ml_dtypes
    f32 = np.float32
    bf16 = ml_dtypes.bfloat16
    x = np.asarray(x, f32)
    attn_mask = np.asarray(attn_mask, f32)
    padding_mask = np.asarray(padding_mask, bool)

    g1 = np.asarray(ln1_g, f32); b1 = np.asarray(ln1_b, f32)
    g2 = np.asarray(ln2_g, f32); b2 = np.asarray(ln2_b, f32)
    Wq, Wk, Wv = (np.asarray(in_proj_w[i * D:(i + 1) * D], f32) for i in range(3))
    bq0, bk0, bv0 = (np.asarray(in_proj_b[i * D:(i + 1) * D], f32) for i in range(3))
    sc = 1.0 / np.sqrt(HD)

    Wq_ = Wq * g1[None, :] * sc
    bq_ = (Wq @ b1 + bq0) * sc
    Wk_ = Wk * g1[None, :]
    bk_ = Wk @ b1 + bk0
    Wv_ = Wv * g1[None, :]
    bv_ = Wv @ b1 + bv0
    W1_ = np.asarray(ff_w1, f32) * g2[None, :]
    b1f = np.asarray(ff_w1, f32) @ b2 + np.asarray(ff_b1, f32)

    def pc(wt, nchunk):  # [Dout, Din] -> [P, nchunk, Dout] chunked on Din
        return np.ascontiguousarray(
            wt.T.reshape(nchunk, P, wt.shape[0]).transpose(1, 0, 2))

    shared = {
        "wq_pc": pc(Wq_, DC).astype(bf16), "wk_pc": pc(Wk_, DC).astype(bf16),
        "wv_pc": pc(Wv_, DC).astype(bf16),
        "bq_pc": np.ascontiguousarray(bq_.reshape(DC, P).T),
        "bk_pc": np.ascontiguousarray(bk_.reshape(DC, P).T),
        "bv_row": bv_[None, :].copy(),
        "wo_pc": pc(np.asarray(out_proj_w, f32), DC),
        "bo_row": np.asarray(out_proj_b, f32)[None, :].copy(),
        "w1_pc": pc(W1_, DC).astype(bf16),
        "b1_pc": np.ascontiguousarray(b1f.reshape(FFC, P).T),
        "w2_pc": pc(np.asarray(ff_w2, f32), FFC).astype(bf16),
        "b2_row": np.asarray(ff_b2, f32)[None, :].copy(),
        "ones_in": np.ones((1, P), f32),
        "onesp_in": np.ones((P, P), f32),
    }

    in_maps = []
    for core in range(8):
        b = core // 2
        h = core % 2
        rot = np.roll(x[b], -1024 * h, axis=0)
        x_nat = np.ascontiguousarray(np.concatenate([rot, x[b, 0:1]], axis=0))

        # additive mask for this batch -> multiplicative factor
        A = attn_mask + np.where(padding_mask[b], -np.inf, 0.0)[None, :]
        mfac = np.exp(np.minimum(A, 0.0)).astype(f32)  # exp(-inf)=0, exp(0)=1
        mfac[~np.isfinite(A)] = 0.0

        # band masks: [P(t), NQB(i), NKC(c), QB(r)];
        # chunk c of block i covers rotated keys [256i - 64 + 128c, +128)
        i_idx = np.arange(NQB)[:, None, None, None]
        c_idx = np.arange(NKC)[None, :, None, None]
        t_idx = np.arange(P)[None, None, :, None]
        r_idx = np.arange(QB)[None, None, None, :]
        k_rot = (QB * i_idx - KPF + P * c_idx + t_idx) % S
        q_rot = i_idx * QB + r_idx
        gq = (q_rot + 1024 * h) % S
        gk = (k_rot + 1024 * h) % S
        band = mfac[gq, gk]                       # [NQB, NKC, P, QB]
        mask_band = np.ascontiguousarray(band.transpose(2, 0, 1, 3)).astype(bf16)

        # global-key column mask, zeroed when key0 falls inside the window
        key0_rot = (0 - 1024 * h) % S
        gq2 = (np.arange(NQB)[:, None] * QB + np.arange(QB)[None, :] + 1024 * h) % S
        gcol = mfac[gq2, 0].copy()
        for i in range(NQB):
            off = (key0_rot - (QB * i - KPF)) % S
            if off < NKC * P:
                gcol[i, :] = 0.0  # key 0 already inside this block's band window
        mask_gcol1 = np.ascontiguousarray(gcol.reshape(1, -1)).astype(bf16)

        # global-query additive mask row, in SHIFTED key order:
        # score col kappa <-> rotated key (kappa - 64) mod S
        kap = np.arange(S)
        k_act = (((kap - KPF) % S) + 1024 * h) % S
        mask_grow = np.ascontiguousarray(
            np.maximum(A[0, k_act], NEG)[None, :].astype(f32))

        m = dict(shared)
        m.update({
            "x_nat": x_nat,
            "mask_band": mask_band,
            "mask_gcol1": mask_gcol1,
            "mask_grow": mask_grow,
        })
        in_maps.append(m)
    return in_maps


def assemble_output(results):
    """results: list of 8 dicts with 'y' [NT, D] -> full [B, S, D]."""
    out = np.empty((B, S, D), np.float32)
    for b in range(B):
        y0 = results[2 * b]["y"]
        y1 = results[2 * b + 1]["y"]
        out[b, 0] = y0[SQ]
        out[b, 1:SQ] = y0[1:SQ]
        out[b, SQ:] = y1[0:SQ]
    return out


_CACHED_NC = None


def kernel(**inputs) -> np.ndarray:
    global _CACHED_NC
    from concourse.bass_utils import run_bass_kernel_spmd

    in_maps = make_host_inputs(**inputs)
    if _CACHED_NC is None:
        _CACHED_NC = build_module()
    res = run_bass_kernel_spmd(_CACHED_NC, in_maps, core_ids=list(range(8)))
    return assemble_output(res.results)


if __name__ == "__main__":
    nc = build_module()
    print("build + compile OK")


# revision 37
# speedup vs baseline: 1.2429x; 1.1084x over previous
"""LocalGlobalTransformerEncoderBlock on 8 Trainium2 NeuronCores.

Sharding: core = (batch b = core//2, sequence half h = core%2). Each core
computes the full encoder block for 1024 query rows of one batch plus the
global token (sequence position 0). The per-core sequence is ROTATED by
1024*h so the core's query rows are always rotated rows [0, 1024), and
x[b, 0] (the global token) is appended as row 2048.

v2 redesign vs the 481us baseline:
  - Band attention uses 3 unaligned 128-key chunks per 256-query block
    (window [256i-64, 256i+320) mod 2048) via a 64-col wrapped prefix on
    the transposed K / z buffers and half-shifted V key tiles.
  - Attention tensors (Q^T, K^T, V, probs, masks) and FFN weights/hidden
    are bf16: same PE rate as fp32r but half the SBUF/DMA and 2x DVE.
  - Softmax denominators are gathered into a [32, 256] tile so ONE
    partition-parallel reciprocal replaces 16 serial [64,512] ones.
  - The global-token path is per-head wide matmuls + one fused softmax,
    emitted interleaved with the band blocks so it hides under them.
  - Normalize/out_proj/LN2 are emitted per band block (software pipeline);
    FFN weights prefetch during attention; x is loaded once into the
    residual buffer.

Self-contained: only imports from /opt/trn_rl_repo (the installed bass
runtime), numpy, and stdlib.
"""

import sys
from contextlib import ExitStack

if "/opt/trn_rl_repo" not in sys.path:
    sys.path.insert(0, "/opt/trn_rl_repo")

import numpy as np

import concourse.bass as bass
import concourse.bacc as bacc_mod
import concourse.mybir as mybir
import concourse.tile as tile
from concourse.masks import make_identity

P = 128
B, S, D, H, FF = 4, 2048, 512, 8, 2048
HD = D // H            # 64
DC = D // P            # 4 chunks of the model dim
FFC = FF // P          # 16 chunks of the FF dim
SK = S + 1             # 2049 tokens (2048 rotated + appended global token)
SQ = 1024              # band queries per core
NT = SQ + 1            # 1025 output tokens
QB = 256               # band query block
NQB = SQ // QB         # 4
NKC = 3                # 128-key chunks per band window
KPF = 64               # wrapped key/token prefix columns
NPAIR = H // 2         # 4 head-pair tiles
EPS = 1e-5
NEG = -1e30

F32 = mybir.dt.float32
F32R = mybir.dt.float32r
BF16 = mybir.dt.bfloat16
AF = mybir.ActivationFunctionType
ALU = mybir.AluOpType
AXL = mybir.AxisListType


def _r(ap):
    """Reinterpret an fp32 AP as float32r for full-rate PE matmuls."""
    return ap.bitcast(F32R)


def build_module():
    nc = bacc_mod.Bacc("TRN2", target_bir_lowering=False)

    x_nat = nc.dram_tensor("x_nat", [SK, D], F32, kind="ExternalInput")
    wq_pc = nc.dram_tensor("wq_pc", [P, DC, D], BF16, kind="ExternalInput")
    wk_pc = nc.dram_tensor("wk_pc", [P, DC, D], BF16, kind="ExternalInput")
    wv_pc = nc.dram_tensor("wv_pc", [P, DC, D], BF16, kind="ExternalInput")
    bq_pc = nc.dram_tensor("bq_pc", [P, DC], F32, kind="ExternalInput")
    bk_pc = nc.dram_tensor("bk_pc", [P, DC], F32, kind="ExternalInput")
    bv_row = nc.dram_tensor("bv_row", [1, D], F32R, kind="ExternalInput")
    wo_pc = nc.dram_tensor("wo_pc", [P, DC, D], F32R, kind="ExternalInput")
    bo_row = nc.dram_tensor("bo_row", [1, D], F32R, kind="ExternalInput")
    w1_pc = nc.dram_tensor("w1_pc", [P, DC, FF], BF16, kind="ExternalInput")
    b1_pc = nc.dram_tensor("b1_pc", [P, FFC], F32, kind="ExternalInput")
    w2_pc = nc.dram_tensor("w2_pc", [P, FFC, D], BF16, kind="ExternalInput")
    b2_row = nc.dram_tensor("b2_row", [1, D], F32R, kind="ExternalInput")
    mask_band = nc.dram_tensor("mask_band", [P, NQB, NKC, QB], BF16, kind="ExternalInput")
    mask_gcol1 = nc.dram_tensor("mask_gcol1", [1, SQ], BF16, kind="ExternalInput")
    mask_grow = nc.dram_tensor("mask_grow", [1, S], F32R, kind="ExternalInput")
    ones_in = nc.dram_tensor("ones_in", [1, P], F32R, kind="ExternalInput")
    onesp_in = nc.dram_tensor("onesp_in", [P, P], F32R, kind="ExternalInput")
    y_out = nc.dram_tensor("y", [NT, D], F32, kind="ExternalOutput")

    with tile.TileContext(nc) as tc, ExitStack() as ctx:
        # ---- persistent state ----
        persist = ctx.enter_context(tc.tile_pool(name="persist", bufs=1))
        x1 = persist.tile([P, 9, D], F32)
        nc.gpsimd.dma_start(
            x1[:, 0:4, :], x_nat[0:512, :].rearrange("(t p) d -> p t d", t=4))
        nc.gpsimd.dma_start(
            x1[:, 4:8, :], x_nat[512:1024, :].rearrange("(t p) d -> p t d", t=4))
        nc.gpsimd.dma_start(x1[0:1, 8, :], x_nat[S : S + 1, :])
        ident = persist.tile([P, P], F32)
        make_identity(nc, ident)
        ones_row = persist.tile([1, P], F32R)
        nc.sync.dma_start(ones_row, ones_in[:])
        eps_t = persist.tile([P, 1], F32)
        nc.vector.memset(eps_t, EPS)
        bo_sb = persist.tile([1, D], F32R)
        nc.sync.dma_start(bo_sb, bo_row[:])
        b1_sb = persist.tile([P, FFC], F32)
        nc.sync.dma_start(b1_sb, b1_pc[:])
        b2_sb = persist.tile([1, D], F32R)
        nc.sync.dma_start(b2_sb, b2_row[:])

        oT = [persist.tile([P, NT], F32, name=f"oT{p}") for p in range(NPAIR)]
        z2T = persist.tile([P, DC, NT], BF16)
        pgT = persist.tile([P, 16, 8], BF16)
        onesP = persist.tile([P, P], F32R)
        nc.sync.dma_start(onesP, onesp_in[:])
        onesPb = persist.tile([P, HD], BF16)
        nc.gpsimd.memset(onesPb, 1.0)

        # ---- PSUM pool: mm(2) + sm(2) + sc(2) + smb(2) = 8 banks ----
        ps = ctx.enter_context(tc.tile_pool(name="ps", bufs=2, space="PSUM"))

        # ---- attention state (lives through band phase) ----
        attn = ctx.enter_context(tc.tile_pool(name="attn", bufs=1))
        QT = [attn.tile([P, NT], BF16, name=f"QT{p}") for p in range(NPAIR)]
        KTx = [attn.tile([P, KPF + SK], BF16, name=f"KTx{p}") for p in range(NPAIR)]
        Vsbx = attn.tile([P, 17, H, HD + 1], BF16)
        nc.gpsimd.memset(Vsbx[:, :, :, HD], 1.0)
        # per-head global-key V rows / probs at quarter-partition bases
        Vg = attn.tile([P, H, HD + 1], BF16)
        pgh3 = [attn.tile([P, SQ], BF16, name=f"pgh3_{t}") for t in range(3)]
        k0h = [attn.tile([P, 1], BF16, name=f"k0h{h}") for h in range(H)]
        mb_sb = attn.tile([P, NQB, NKC, QB], BF16)
        mgc_sb = attn.tile([P, SQ], BF16)
        mgrow_sb = attn.tile([1, S], F32R)

        wo_sb = attn.tile([P, DC, D], F32R)
        w1_sb = attn.tile([P, DC, FF], BF16)

        # x loads on the gpsimd queue; weights/masks on the sync queue
        nc.sync.dma_start(mb_sb, mask_band[:])
        nc.sync.dma_start(mgc_sb[0:1, :], mask_gcol1[:])
        nc.sync.dma_start(mgrow_sb, mask_grow[:])
        for mrow in (32, 64):
            nc.vector.tensor_copy(mgc_sb[mrow : mrow + 1, :], mgc_sb[0:1, :])

        # ====== Phase A+B: LN1 -> zTx, V (interleaved), Q/K ======
        with tc.tile_pool(name="phA", bufs=1) as pha, \
             tc.tile_pool(name="lnz", bufs=2) as z_pool, \
             tc.tile_pool(name="st1", bufs=4) as stat_pool:
            zTx = pha.tile([P, DC, KPF + SK], BF16)
            wv_sb = pha.tile([P, DC, D], BF16)
            wq_sb = pha.tile([P, DC, D], BF16)
            wk_sb = pha.tile([P, DC, D], BF16)
            bq_sb = pha.tile([P, DC], F32)
            bk_sb = pha.tile([P, DC], F32)
            bv_sb = pha.tile([1, D], F32R)

            nc.sync.dma_start(wv_sb, wv_pc[:])
            nc.sync.dma_start(bv_sb, bv_row[:])
            nc.sync.dma_start(wq_sb, wq_pc[:])
            nc.sync.dma_start(bq_sb, bq_pc[:])
            nc.sync.dma_start(wk_sb, wk_pc[:])
            nc.sync.dma_start(bk_sb, bk_pc[:])
            nc.sync.dma_start(wo_sb, wo_pc[:])
            nc.sync.dma_start(w1_sb, w1_pc[:])

            def vproj(j):
                """V projection for shifted key tile j (tokens [128j-64,128j+64))."""
                rows = P if j < 16 else 1
                col0 = 128 * j if j < 16 else KPF + S
                pv = ps.tile([P, D], F32, tag="mm")
                for d in range(DC):
                    nc.tensor.matmul(pv[:rows], zTx[:, d, col0 : col0 + rows],
                                     wv_sb[:, d, :],
                                     start=(d == 0), stop=False)
                nc.tensor.matmul(pv[:rows], _r(ones_row[:1, :rows]), _r(bv_sb),
                                 start=False, stop=True)
                nc.scalar.activation(
                    Vsbx[:rows, j, :, 0:HD],
                    pv[:rows].rearrange("p (h e) -> p h e", h=H), AF.Copy)

            def ln1(t):
                rows = P if t < 16 else 1
                if t < 8:
                    src = x1[:rows, t, :]
                elif t < 16:
                    xt = z_pool.tile([P, D], F32, tag="xt")
                    nc.gpsimd.dma_start(xt, x_nat[t * P : (t + 1) * P, :])
                    src = xt[:rows]
                else:
                    src = x1[0:1, 8, :]
                st = stat_pool.tile([P, 6], F32, tag="bnst")
                nc.vector.bn_stats(st[:rows], src)
                mv = stat_pool.tile([P, 2], F32, tag="bnmv")
                nc.vector.bn_aggr(mv[:rows], st[:rows])
                rstd = stat_pool.tile([P, 1], F32, tag="rstd")
                nc.scalar.activation(rstd[:rows], mv[:rows, 1:2], AF.Sqrt,
                                     bias=eps_t[:rows], scale=1.0)
                nc.vector.reciprocal(rstd[:rows], rstd[:rows])
                z = z_pool.tile([P, D], F32, tag="z")
                nc.vector.tensor_scalar(z[:rows], src, mv[:rows, 0:1],
                                        rstd[:rows],
                                        op0=ALU.subtract, op1=ALU.mult)
                ptt = ps.tile([P, DC, P], F32, tag="mm")
                for d in range(DC):
                    nc.tensor.transpose(ptt[:, d, :rows],
                                        z[:rows, d * P : (d + 1) * P],
                                        ident[:rows, :rows])
                nc.scalar.activation(
                    zTx[:, :, KPF + t * P : KPF + t * P + rows],
                    ptt[:, :, :rows], AF.Copy)
                if t == 15:
                    # wrapped prefix: tokens 1984..2048 = local rows 64..128
                    nc.vector.tensor_copy(zTx[:, :, 0:KPF], ptt[:, :, HD:P])

            for t in range(17):
                ln1(t)
                if 1 <= t <= 15:
                    vproj(t)
                if t == 15:
                    vproj(0)
            vproj(16)

            # ---- Q/K projections ----
            q_blocks = [(KPF, 0, 512), (KPF + 512, 512, 512), (KPF + S, SQ, 1)]
            k_blocks = [(KPF + i * 512, KPF + i * 512, 512) for i in range(4)] \
                + [(KPF + S, KPF + S, 1)]
            for p in range(NPAIR):
                for w_sb, b_sb, dst, blocks in (
                        (wq_sb, bq_sb, QT[p], q_blocks),
                        (wk_sb, bk_sb, KTx[p], k_blocks)):
                    for bi, (src, dcol, w) in enumerate(blocks):
                        s0, w0, keep = (src, w, 0) if w > 1 else (src - 1, 2, 1)
                        pq = ps.tile([P, 512], F32, tag="mm")
                        for d in range(DC):
                            nc.tensor.matmul(pq[:, :w0],
                                             w_sb[:, d, p * P : (p + 1) * P],
                                             zTx[:, d, s0 : s0 + w0],
                                             start=(d == 0), stop=(d == DC - 1))
                        if bi % 2 == 0:
                            nc.scalar.activation(dst[:, dcol : dcol + w],
                                                 pq[:, keep : keep + w], AF.Identity,
                                                 bias=b_sb[:, p : p + 1])
                        else:
                            nc.vector.tensor_scalar(dst[:, dcol : dcol + w],
                                                    pq[:, keep : keep + w],
                                                    b_sb[:, p : p + 1], None,
                                                    op0=ALU.add)
                nc.vector.tensor_copy(KTx[p][:, 0:KPF], KTx[p][:, S : S + KPF])

        # ====== Phase C/D/E/F interleaved: band + global + norm + out_proj + LN2
        with tc.tile_pool(name="bandp", bufs=3) as band_pool, \
             tc.tile_pool(name="ln2z", bufs=2) as z2_pool, \
             tc.tile_pool(name="st2", bufs=4) as stat2_pool:

            # band-phase scratch: quarter-partition l tiles, global-path
            # per-pair scratch, zero-padded stationaries
            ltA = band_pool.tile([P, NQB, QB], BF16, bufs=1)
            ltB = band_pool.tile([P, NQB, QB], BF16, bufs=1)
            lrA = band_pool.tile([P, NQB, QB], BF16, bufs=1)
            lrB = band_pool.tile([P, NQB, QB], BF16, bufs=1)
            lq = band_pool.tile([P, NQB, QB], BF16, bufs=1)
            egpp = [band_pool.tile([2, S], F32, tag=f"egpp{p}", bufs=1,
                                   name=f"egpp{p}") for p in range(NPAIR)]
            laP = [band_pool.tile([2, 4], F32, tag=f"laP{p}", bufs=1,
                                  name=f"laP{p}") for p in range(NPAIR)]
            egp8 = band_pool.tile([8, S], F32, bufs=1)
            la8 = band_pool.tile([8, 4], F32, bufs=1)
            ga8 = band_pool.tile([8, 1], F32, bufs=1)
            larec = band_pool.tile([8, 1], F32, bufs=1)

            # col j of q2g[p] holds head (2p+j)'s global-query column in its
            # own 64 contraction rows; k0h[h] is the zero-padded global-key
            # column; Vg packs each head's global-key V row (+ ones col) at
            # quarter-partition bases so band PV matmuls stay legal
            q2g = [band_pool.tile([P, 2], BF16, tag=f"q2g{p}", bufs=1,
                                  name=f"q2g{p}") for p in range(NPAIR)]
            for pr in range(NPAIR):
                nc.gpsimd.memset(q2g[pr], 0.0)
                for j in range(2):
                    sub = j * HD
                    h = 2 * pr + j
                    nc.gpsimd.tensor_copy(
                        q2g[pr][sub : sub + HD, j : j + 1],
                        QT[pr][sub : sub + HD, SQ : SQ + 1])
                    nc.gpsimd.memset(k0h[h], 0.0)
                    nc.gpsimd.tensor_copy(
                        k0h[h][sub : sub + HD, 0:1],
                        KTx[pr][sub : sub + HD, KPF + S : KPF + S + 1])
                    m = 32 * (h % 3)
                    nc.gpsimd.tensor_copy(Vg[m : m + 1, h, :],
                                          Vsbx[0:1, 16, h, :])

            # global-KEY column scores for all band queries (all 8 heads)
            for h in range(H):
                pr = h // 2
                m = 32 * (h % 3)
                pgX = pgh3[h // 3]
                for half in range(2):
                    gq = ps.tile([P, 512], F32, tag="mm")
                    nc.tensor.matmul(gq[0:1, :], k0h[h],
                                     QT[pr][:, half * 512 : (half + 1) * 512],
                                     start=True, stop=True)
                    nc.scalar.activation(
                        pgX[m : m + 1, half * 512 : (half + 1) * 512],
                        gq[0:1, :], AF.Exp)
                nc.vector.tensor_tensor(pgX[m : m + 1, :], pgX[m : m + 1, :],
                                        mgc_sb[m : m + 1, :], ALU.mult)

            def ln2(t):
                rows = P if t < 8 else 1
                src = x1[:rows, t, :]
                st = stat2_pool.tile([P, 6], F32, tag="bnst2")
                nc.vector.bn_stats(st[:rows], src)
                mv = stat2_pool.tile([P, 2], F32, tag="bnmv2")
                nc.vector.bn_aggr(mv[:rows], st[:rows])
                rstd = stat2_pool.tile([P, 1], F32, tag="rstd2")
                nc.scalar.activation(rstd[:rows], mv[:rows, 1:2], AF.Sqrt,
                                     bias=eps_t[:rows], scale=1.0)
                nc.vector.reciprocal(rstd[:rows], rstd[:rows])
                z2 = z2_pool.tile([P, D], F32, tag="z2")
                nc.vector.tensor_scalar(z2[:rows], src, mv[:rows, 0:1],
                                        rstd[:rows],
                                        op0=ALU.subtract, op1=ALU.mult)
                ptt = ps.tile([P, DC, P], F32, tag="mm")
                for d in range(DC):
                    nc.tensor.transpose(ptt[:, d, :rows],
                                        z2[:rows, d * P : (d + 1) * P],
                                        ident[:rows, :rows])
                nc.scalar.activation(z2T[:, :, t * P : t * P + rows],
                                     ptt[:, :, :rows], AF.Copy)

            def out_proj(t):
                w = P if t < 8 else 1
                yp = ps.tile([P, D], F32, tag="mm")
                if w > 1:
                    for p in range(NPAIR):
                        nc.tensor.matmul(yp[:w], _r(oT[p][:, t * P : t * P + w]),
                                         _r(wo_sb[:, p, :]),
                                         start=(p == 0), stop=False)
                else:
                    for p in range(NPAIR):
                        nc.tensor.matmul(yp[:w], _r(oT[p][:, SQ : SQ + 1]),
                                         _r(wo_sb[:, p, :]),
                                         start=(p == 0), stop=False)
                nc.tensor.matmul(yp[:w], _r(ones_row[:1, :w]), _r(bo_sb),
                                 start=False, stop=True)
                nc.vector.tensor_tensor(x1[:w, t, :], yp[:w], x1[:w, t, :], ALU.add)

            def global_scores(pr, tcb):
                gs = ps.tile([P, 512], F32, tag="mm")
                nc.tensor.matmul(gs[0:2, :], q2g[pr],
                                 KTx[pr][:, tcb * 512 : (tcb + 1) * 512],
                                 start=True, stop=False)
                # additive key mask is head-independent -> rank-1 matmul add
                nc.tensor.matmul(gs[0:2, :], _r(onesP[0:1, 0:2]),
                                 mgrow_sb[0:1, tcb * 512 : (tcb + 1) * 512],
                                 start=False, stop=True)
                nc.scalar.activation(
                    egpp[pr][:, tcb * 512 : (tcb + 1) * 512],
                    gs[0:2, :], AF.Exp,
                    accum_out=laP[pr][:, tcb : tcb + 1])

            def global_gather():
                for pr in range(NPAIR):
                    nc.gpsimd.dma_start(egp8[2 * pr : 2 * pr + 2, :], egpp[pr][:])
                    nc.gpsimd.dma_start(la8[2 * pr : 2 * pr + 2, :], laP[pr][:])

            def global_transposes():
                nc.vector.tensor_reduce(ga8, la8, axis=AXL.X, op=ALU.add)
                nc.vector.reciprocal(larec, ga8)
                for c in range(16):
                    ptb = ps.tile([P, QB], F32, tag="sm")
                    nc.tensor.transpose(ptb[:, 0:8], egp8[0:8, c * P : (c + 1) * P],
                                        ident[0:8, 0:8])
                    nc.scalar.activation(pgT[:, c, :], ptb[:, 0:8], AF.Copy)

            def global_pv():
                for g in range(2):
                    pog = ps.tile([P, 512], F32, tag="mm")
                    for c in range(16):
                        nc.tensor.matmul(pog[0:8, 0 : 4 * HD], pgT[:, c, :],
                                         Vsbx[:, c, 4 * g : 4 * g + 4, 0:HD],
                                         start=(c == 0), stop=(c == 15))
                    pog_sb = band_pool.tile([8, 4 * HD], F32, tag="pog_sb")
                    nc.scalar.activation(pog_sb, pog[0:8, 0 : 4 * HD], AF.Copy,
                                         scale=larec[0:8, 0:1])
                    for j in range(2):
                        ptj = ps.tile([P, QB], F32, tag="sm")
                        nc.tensor.transpose(ptj[:, 0:8],
                                            pog_sb[0:8, j * P : (j + 1) * P],
                                            ident[0:8, 0:8])
                        for hh in (2 * j, 2 * j + 1):
                            h = 4 * g + hh
                            rlo = (hh % 2) * HD
                            nc.scalar.activation(
                                oT[h // 2][rlo : rlo + HD, SQ : SQ + 1].bitcast(F32R),
                                ptj[rlo : rlo + HD, h : h + 1], AF.Copy)

            for i in range(NQB):
                for h in range(H):
                    pr, sub = h // 2, (h % 2) * HD
                    q_ap = QT[pr][sub : sub + HD, i * QB : (i + 1) * QB]
                    sc = ps.tile([P, NKC, QB], F32, tag="sc", bufs=2)
                    for c in range(NKC):
                        nc.tensor.matmul(
                            sc[:, c, :],
                            KTx[pr][sub : sub + HD,
                                    QB * i + c * P : QB * i + c * P + P],
                            q_ap, start=True, stop=True)
                    pT = band_pool.tile([P, NKC, QB], BF16, tag="pT")
                    nc.scalar.activation(pT, sc, AF.Exp)
                    nc.vector.tensor_tensor(pT, pT, mb_sb[:, i, :, :], ALU.mult)
                    po = ps.tile([P, QB], F32, tag="sm")
                    for c in range(NKC):
                        nc.tensor.matmul(po[0 : HD + 1, :], Vsbx[:, 2 * i + c, h, :],
                                         pT[:, c, :], start=(c == 0), stop=False)
                    m = 32 * (h % 3)
                    pgX = pgh3[h // 3]
                    nc.tensor.matmul(po[0 : HD + 1, :], Vg[m : m + 1, h, :],
                                     pgX[m : m + 1, i * QB : (i + 1) * QB],
                                     start=False, stop=True)
                    if h % 2 == 0:
                        nc.vector.tensor_copy(
                            oT[pr][sub : sub + HD, i * QB : (i + 1) * QB].bitcast(F32R),
                            po[0:HD, :])
                    else:
                        nc.scalar.activation(
                            oT[pr][sub : sub + HD, i * QB : (i + 1) * QB].bitcast(F32R),
                            po[0:HD, :], AF.Copy)
                    ml = 32 * (h % 4)
                    ltX = ltA if h < 4 else ltB
                    nc.scalar.activation(
                        ltX[ml : ml + 1, i, :],
                        po[HD : HD + 1, :], AF.Copy)

                # normalize block i: partition-parallel reciprocals over the
                # quarter-row l tiles (DVE allows base 96); matmul operands
                # cannot sit at base 96, so heads 3/7 get moved to lq first
                with nc.allow_low_precision(reason="bf16 softmax sums"):
                    nc.vector.reciprocal(lrA[:, i, :], ltA[:, i, :])
                    nc.vector.reciprocal(lrB[:, i, :], ltB[:, i, :])
                nc.vector.tensor_copy(lq[0:1, i, :], lrA[96:97, i, :])
                nc.vector.tensor_copy(lq[32:33, i, :], lrB[96:97, i, :])

                def lsrc(h):
                    if h % 4 == 3:
                        return lq, 32 * (h // 4)
                    return (lrA if h < 4 else lrB), 32 * (h % 4)

                for p in range(NPAIR):
                    for j in range(2):
                        lt_t, r = lsrc(2 * p + j)
                        lbc = ps.tile([P, QB], F32, tag="sm")
                        nc.tensor.matmul(lbc[0:HD, :],
                                         onesPb[r : r + 1, :],
                                         lt_t[r : r + 1, i, :],
                                         start=True, stop=True)
                        rows = oT[p][j * HD : (j + 1) * HD,
                                     i * QB : (i + 1) * QB]
                        nc.vector.tensor_tensor(rows.bitcast(F32R), rows,
                                                lbc[0:HD, :], ALU.mult)

                # pipeline: out_proj + LN2 for the two finished token tiles
                for t in (2 * i, 2 * i + 1):
                    out_proj(t)
                    ln2(t)

                # interleave the global-token path under the band blocks
                if i == 0:
                    for tcb in range(4):
                        global_scores(0, tcb)
                        global_scores(1, tcb)
                elif i == 1:
                    for tcb in range(4):
                        global_scores(2, tcb)
                        global_scores(3, tcb)
                    global_gather()
                elif i == 2:
                    global_transposes()
                else:
                    global_pv()

            out_proj(8)
            ln2(8)

        # band scratch released; fetch FFN2 weights under out_proj/FFN1
        ffw = ctx.enter_context(tc.tile_pool(name="ffw", bufs=1))
        w2_sb = ffw.tile([P, FFC, D], BF16)
        nc.sync.dma_start(w2_sb, w2_pc[:])

        # ====== Phase G: FFN + residual -> y ======
        with tc.tile_pool(name="ffn", bufs=1) as ffn_pool, \
             tc.tile_pool(name="ffo", bufs=3) as out_pool:
            for t0, tw in [(0, 512), (512, 512), (SQ, 1)]:
                hT = ffn_pool.tile([P, FFC, 512], BF16, tag="hT")
                s0, w0, keep = (t0, tw, 0) if tw > 1 else (t0 - 1, 2, 1)
                for f in range(FFC):
                    ph = ps.tile([P, 512], F32, tag="mm")
                    for d in range(DC):
                        nc.tensor.matmul(ph[:, :w0],
                                         w1_sb[:, d, f * P : (f + 1) * P],
                                         z2T[:, d, s0 : s0 + w0],
                                         start=(d == 0), stop=(d == DC - 1))
                    nc.scalar.activation(hT[:, f, :tw],
                                         ph[:, keep : keep + tw], AF.Gelu,
                                         bias=b1_sb[:, f : f + 1])
                nsub = 4 if tw == 512 else 1
                for stp in range(nsub):
                    sw = P if tw == 512 else 1
                    ps2 = ps.tile([P, D], F32, tag="mm")
                    for f in range(FFC):
                        nc.tensor.matmul(ps2[:sw],
                                         hT[:, f, stp * P : stp * P + sw],
                                         w2_sb[:, f, :],
                                         start=(f == 0), stop=False)
                    nc.tensor.matmul(ps2[:sw], _r(ones_row[:1, :sw]), _r(b2_sb),
                                     start=False, stop=True)
                    yt = out_pool.tile([P, D], F32, tag="yt")
                    tglob = t0 // P + stp
                    nc.vector.tensor_tensor(yt[:sw], ps2[:sw],
                                            x1[:sw, tglob, :], ALU.add)
                    nc.gpsimd.dma_start(
                        y_out[t0 + stp * P : t0 + stp * P + sw, :], yt[:sw])

    nc.finalize()
    return nc


def make_host_inputs(x, padding_mask, attn_mask, in_proj_w, in_proj_b, out_proj_w,
                     out_proj_b, ln1_g, ln1_b, ln2_g, ln2_b, ff_w1, ff_b1, ff_w2,
                     ff_b2):
    """Build the 8 per-core input maps (numpy only)."""
    import ml_dtypes
    f32 = np.float32
    bf16 = ml_dtypes.bfloat16
    x = np.asarray(x, f32)
    attn_mask = np.asarray(attn_mask, f32)
    padding_mask = np.asarray(padding_mask, bool)

    g1 = np.asarray(ln1_g, f32); b1 = np.asarray(ln1_b, f32)
    g2 = np.asarray(ln2_g, f32); b2 = np.asarray(ln2_b, f32)
    Wq, Wk, Wv = (np.asarray(in_proj_w[i * D:(i + 1) * D], f32) for i in range(3))
    bq0, bk0, bv0 = (np.asarray(in_proj_b[i * D:(i + 1) * D], f32) for i in range(3))
    sc = 1.0 / np.sqrt(HD)

    Wq_ = Wq * g1[None, :] * sc
    bq_ = (Wq @ b1 + bq0) * sc
    Wk_ = Wk * g1[None, :]
    bk_ = Wk @ b1 + bk0
    Wv_ = Wv * g1[None, :]
    bv_ = Wv @ b1 + bv0
    W1_ = np.asarray(ff_w1, f32) * g2[None, :]
    b1f = np.asarray(ff_w1, f32) @ b2 + np.asarray(ff_b1, f32)

    def pc(wt, nchunk):  # [Dout, Din] -> [P, nchunk, Dout] chunked on Din
        return np.ascontiguousarray(
            wt.T.reshape(nchunk, P, wt.shape[0]).transpose(1, 0, 2))

    shared = {
        "wq_pc": pc(Wq_, DC).astype(bf16), "wk_pc": pc(Wk_, DC).astype(bf16),
        "wv_pc": pc(Wv_, DC).astype(bf16),
        "bq_pc": np.ascontiguousarray(bq_.reshape(DC, P).T),
        "bk_pc": np.ascontiguousarray(bk_.reshape(DC, P).T),
        "bv_row": bv_[None, :].copy(),
        "wo_pc": pc(np.asarray(out_proj_w, f32), DC),
        "bo_row": np.asarray(out_proj_b, f32)[None, :].copy(),
        "w1_pc": pc(W1_, DC).astype(bf16),
        "b1_pc": np.ascontiguousarray(b1f.reshape(FFC, P).T),
        "w2_pc": pc(np.asarray(ff_w2, f32), FFC).astype(bf16),
        "b2_row": np.asarray(ff_b2, f32)[None, :].copy(),
        "ones_in": np.ones((1, P), f32),
        "onesp_in": np.ones((P, P), f32),
    }

    in_maps = []
    for core in range(8):
        b = core // 2
        h = core % 2
        rot = np.roll(x[b], -1024 * h, axis=0)
        x_nat = np.ascontiguousarray(np.concatenate([rot, x[b, 0:1]], axis=0))

        # additive mask for this batch -> multiplicative factor
        A = attn_mask + np.where(padding_mask[b], -np.inf, 0.0)[None, :]
        mfac = np.exp(np.minimum(A, 0.0)).astype(f32)  # exp(-inf)=0, exp(0)=1
        mfac[~np.isfinite(A)] = 0.0

        # band masks: [P(t), NQB(i), NKC(c), QB(r)];
        # chunk c of block i covers rotated keys [256i - 64 + 128c, +128)
        i_idx = np.arange(NQB)[:, None, None, None]
        c_idx = np.arange(NKC)[None, :, None, None]
        t_idx = np.arange(P)[None, None, :, None]
        r_idx = np.arange(QB)[None, None, None, :]
        k_rot = (QB * i_idx - KPF + P * c_idx + t_idx) % S
        q_rot = i_idx * QB + r_idx
        gq = (q_rot + 1024 * h) % S
        gk = (k_rot + 1024 * h) % S
        band = mfac[gq, gk]                       # [NQB, NKC, P, QB]
        mask_band = np.ascontiguousarray(band.transpose(2, 0, 1, 3)).astype(bf16)

        # global-key column mask, zeroed when key0 falls inside the window
        key0_rot = (0 - 1024 * h) % S
        gq2 = (np.arange(NQB)[:, None] * QB + np.arange(QB)[None, :] + 1024 * h) % S
        gcol = mfac[gq2, 0].copy()
        for i in range(NQB):
            off = (key0_rot - (QB * i - KPF)) % S
            if off < NKC * P:
                gcol[i, :] = 0.0  # key 0 already inside this block's band window
        mask_gcol1 = np.ascontiguousarray(gcol.reshape(1, -1)).astype(bf16)

        # global-query additive mask row, in SHIFTED key order:
        # score col kappa <-> rotated key (kappa - 64) mod S
        kap = np.arange(S)
        k_act = (((kap - KPF) % S) + 1024 * h) % S
        mask_grow = np.ascontiguousarray(
            np.maximum(A[0, k_act], NEG)[None, :].astype(f32))

        m = dict(shared)
        m.update({
            "x_nat": x_nat,
            "mask_band": mask_band,
            "mask_gcol1": mask_gcol1,
            "mask_grow": mask_grow,
        })
        in_maps.append(m)
    return in_maps


def assemble_output(results):
    """results: list of 8 dicts with 'y' [NT, D] -> full [B, S, D]."""
    out = np.empty((B, S, D), np.float32)
    for b in range(B):
        y0 = results[2 * b]["y"]
        y1 = results[2 * b + 1]["y"]
        out[b, 0] = y0[SQ]
        out[b, 1:SQ] = y0[1:SQ]
        out[b, SQ:] = y1[0:SQ]
    return out


_CACHED_NC = None


def kernel(**inputs) -> np.ndarray:
    global _CACHED_NC
    from concourse.bass_utils import run_bass_kernel_spmd

    in_maps = make_host_inputs(**inputs)
    if _CACHED_NC is None:
        _CACHED_NC = build_module()
    res = run_bass_kernel_spmd(_CACHED_NC, in_maps, core_ids=list(range(8)))
    return assemble_output(res.results)


if __name__ == "__main__":
    nc = build_module()
    print("build + compile OK")


# revision 38
# speedup vs baseline: 1.2939x; 1.0410x over previous
"""LocalGlobalTransformerEncoderBlock on 8 Trainium2 NeuronCores.

Sharding: core = (batch b = core//2, sequence half h = core%2). Each core
computes the full encoder block for 1024 query rows of one batch plus the
global token (sequence position 0). The per-core sequence is ROTATED by
1024*h so the core's query rows are always rotated rows [0, 1024), and
x[b, 0] (the global token) is appended as row 2048.

v2 redesign vs the 481us baseline:
  - Band attention uses 3 unaligned 128-key chunks per 256-query block
    (window [256i-64, 256i+320) mod 2048) via a 64-col wrapped prefix on
    the transposed K / z buffers and half-shifted V key tiles.
  - Attention tensors (Q^T, K^T, V, probs, masks) and FFN weights/hidden
    are bf16: same PE rate as fp32r but half the SBUF/DMA and 2x DVE.
  - Softmax denominators are gathered into a [32, 256] tile so ONE
    partition-parallel reciprocal replaces 16 serial [64,512] ones.
  - The global-token path is per-head wide matmuls + one fused softmax,
    emitted interleaved with the band blocks so it hides under them.
  - Normalize/out_proj/LN2 are emitted per band block (software pipeline);
    FFN weights prefetch during attention; x is loaded once into the
    residual buffer.

Self-contained: only imports from /opt/trn_rl_repo (the installed bass
runtime), numpy, and stdlib.
"""

import sys
from contextlib import ExitStack

if "/opt/trn_rl_repo" not in sys.path:
    sys.path.insert(0, "/opt/trn_rl_repo")

import numpy as np

import concourse.bass as bass
import concourse.bacc as bacc_mod
import concourse.mybir as mybir
import concourse.tile as tile
from concourse.masks import make_identity

P = 128
B, S, D, H, FF = 4, 2048, 512, 8, 2048
HD = D // H            # 64
DC = D // P            # 4 chunks of the model dim
FFC = FF // P          # 16 chunks of the FF dim
SK = S + 1             # 2049 tokens (2048 rotated + appended global token)
SQ = 1024              # band queries per core
NT = SQ + 1            # 1025 output tokens
QB = 256               # band query block
NQB = SQ // QB         # 4
NKC = 3                # 128-key chunks per band window
KPF = 64               # wrapped key/token prefix columns
NPAIR = H // 2         # 4 head-pair tiles
EPS = 1e-5
NEG = -1e30

F32 = mybir.dt.float32
F32R = mybir.dt.float32r
BF16 = mybir.dt.bfloat16
AF = mybir.ActivationFunctionType
ALU = mybir.AluOpType
AXL = mybir.AxisListType


def _r(ap):
    """Reinterpret an fp32 AP as float32r for full-rate PE matmuls."""
    return ap.bitcast(F32R)


def build_module():
    nc = bacc_mod.Bacc("TRN2", target_bir_lowering=False)

    x_nat = nc.dram_tensor("x_nat", [SK, D], F32, kind="ExternalInput")
    wq_pc = nc.dram_tensor("wq_pc", [P, DC, D], BF16, kind="ExternalInput")
    wk_pc = nc.dram_tensor("wk_pc", [P, DC, D], BF16, kind="ExternalInput")
    wv_pc = nc.dram_tensor("wv_pc", [P, DC, D], BF16, kind="ExternalInput")
    bq_pc = nc.dram_tensor("bq_pc", [P, DC], F32, kind="ExternalInput")
    bk_pc = nc.dram_tensor("bk_pc", [P, DC], F32, kind="ExternalInput")
    bv_row = nc.dram_tensor("bv_row", [1, D], F32R, kind="ExternalInput")
    wo_pc = nc.dram_tensor("wo_pc", [P, DC, D], F32R, kind="ExternalInput")
    bo_row = nc.dram_tensor("bo_row", [1, D], F32R, kind="ExternalInput")
    w1_pc = nc.dram_tensor("w1_pc", [P, DC, FF], BF16, kind="ExternalInput")
    b1_pc = nc.dram_tensor("b1_pc", [P, FFC], F32, kind="ExternalInput")
    w2_pc = nc.dram_tensor("w2_pc", [P, FFC, D], BF16, kind="ExternalInput")
    b2_row = nc.dram_tensor("b2_row", [1, D], F32R, kind="ExternalInput")
    mask_band = nc.dram_tensor("mask_band", [P, NQB, NKC, QB], BF16, kind="ExternalInput")
    mask_gcol1 = nc.dram_tensor("mask_gcol1", [1, SQ], BF16, kind="ExternalInput")
    mask_grow = nc.dram_tensor("mask_grow", [1, S], F32R, kind="ExternalInput")
    ones_in = nc.dram_tensor("ones_in", [1, P], F32R, kind="ExternalInput")
    onesp_in = nc.dram_tensor("onesp_in", [P, P], F32R, kind="ExternalInput")
    y_out = nc.dram_tensor("y", [NT, D], F32, kind="ExternalOutput")

    with tile.TileContext(nc) as tc, ExitStack() as ctx:
        # ---- persistent state ----
        persist = ctx.enter_context(tc.tile_pool(name="persist", bufs=1))
        x1 = persist.tile([P, 9, D], F32)
        nc.gpsimd.dma_start(x1[:, 0, :], x_nat[0:P, :])
        nc.gpsimd.dma_start(
            x1[:, 1:4, :], x_nat[P:512, :].rearrange("(t p) d -> p t d", t=3))
        nc.gpsimd.dma_start(
            x1[:, 4:8, :], x_nat[512:1024, :].rearrange("(t p) d -> p t d", t=4))
        nc.gpsimd.dma_start(x1[0:1, 8, :], x_nat[S : S + 1, :])
        ident = persist.tile([P, P], F32)
        make_identity(nc, ident)
        ones_row = persist.tile([1, P], F32R)
        nc.sync.dma_start(ones_row, ones_in[:])
        eps_t = persist.tile([P, 1], F32)
        nc.vector.memset(eps_t, EPS)
        bo_sb = persist.tile([1, D], F32R)
        nc.sync.dma_start(bo_sb, bo_row[:])
        b1_sb = persist.tile([P, FFC], F32)
        nc.sync.dma_start(b1_sb, b1_pc[:])
        b2_sb = persist.tile([1, D], F32R)
        nc.sync.dma_start(b2_sb, b2_row[:])

        oT = [persist.tile([P, NT], F32, name=f"oT{p}") for p in range(NPAIR)]
        z2T = persist.tile([P, DC, NT], BF16)
        pgT = persist.tile([P, 16, 8], BF16)
        onesP = persist.tile([P, P], F32R)
        nc.sync.dma_start(onesP, onesp_in[:])
        onesPb = persist.tile([P, HD], BF16)
        nc.gpsimd.memset(onesPb, 1.0)

        # ---- PSUM pool: mm(2) + sm(2) + sc(2) + smb(2) = 8 banks ----
        ps = ctx.enter_context(tc.tile_pool(name="ps", bufs=2, space="PSUM"))

        # ---- attention state (lives through band phase) ----
        attn = ctx.enter_context(tc.tile_pool(name="attn", bufs=1))
        QT = [attn.tile([P, NT], BF16, name=f"QT{p}") for p in range(NPAIR)]
        KTx = [attn.tile([P, KPF + SK], BF16, name=f"KTx{p}") for p in range(NPAIR)]
        Vsbx = attn.tile([P, 17, H, HD + 1], BF16)
        nc.gpsimd.memset(Vsbx[:, :, :, HD], 1.0)
        # per-head global-key V rows / probs at quarter-partition bases
        Vg = attn.tile([P, H, HD + 1], BF16)
        pgh3 = [attn.tile([P, SQ], BF16, name=f"pgh3_{t}") for t in range(3)]
        k0h = [attn.tile([P, 1], BF16, name=f"k0h{h}") for h in range(H)]
        mb_sb = attn.tile([P, NQB, NKC, QB], BF16)
        mgc_sb = attn.tile([P, SQ], BF16)
        mgrow_sb = attn.tile([1, S], F32R)

        wo_sb = attn.tile([P, DC, D], F32R)
        w1_sb = attn.tile([P, DC, FF], BF16)

        # x loads on the gpsimd queue; weights/masks on the sync queue
        nc.sync.dma_start(mb_sb, mask_band[:])
        nc.sync.dma_start(mgc_sb[0:1, :], mask_gcol1[:])
        nc.sync.dma_start(mgrow_sb, mask_grow[:])
        for mrow in (32, 64):
            nc.vector.tensor_copy(mgc_sb[mrow : mrow + 1, :], mgc_sb[0:1, :])

        # ====== Phase A+B: LN1 -> zTx, V (interleaved), Q/K ======
        with tc.tile_pool(name="phA", bufs=1) as pha, \
             tc.tile_pool(name="lnz", bufs=2) as z_pool, \
             tc.tile_pool(name="st1", bufs=4) as stat_pool:
            zTx = pha.tile([P, DC, KPF + SK], BF16)
            wv_sb = pha.tile([P, DC, D], BF16)
            wq_sb = pha.tile([P, DC, D], BF16)
            wk_sb = pha.tile([P, DC, D], BF16)
            bq_sb = pha.tile([P, DC], F32)
            bk_sb = pha.tile([P, DC], F32)
            bv_sb = pha.tile([1, D], F32R)

            nc.sync.dma_start(wv_sb, wv_pc[:])
            nc.sync.dma_start(bv_sb, bv_row[:])
            nc.sync.dma_start(wq_sb, wq_pc[:])
            nc.sync.dma_start(bq_sb, bq_pc[:])
            nc.sync.dma_start(wk_sb, wk_pc[:])
            nc.sync.dma_start(bk_sb, bk_pc[:])
            nc.sync.dma_start(wo_sb, wo_pc[:])
            nc.sync.dma_start(w1_sb, w1_pc[:])

            def vproj(j):
                """V projection for shifted key tile j (tokens [128j-64,128j+64))."""
                rows = P if j < 16 else 1
                col0 = 128 * j if j < 16 else KPF + S
                pv = ps.tile([P, D], F32, tag="mm")
                for d in range(DC):
                    nc.tensor.matmul(pv[:rows], zTx[:, d, col0 : col0 + rows],
                                     wv_sb[:, d, :],
                                     start=(d == 0), stop=False)
                nc.tensor.matmul(pv[:rows], _r(ones_row[:1, :rows]), _r(bv_sb),
                                 start=False, stop=True)
                nc.scalar.activation(
                    Vsbx[:rows, j, :, 0:HD],
                    pv[:rows].rearrange("p (h e) -> p h e", h=H), AF.Copy)

            def ln1(t):
                rows = P if t < 16 else 1
                if t < 8:
                    src = x1[:rows, t, :]
                elif t < 16:
                    xt = z_pool.tile([P, D], F32, tag="xt")
                    nc.gpsimd.dma_start(xt, x_nat[t * P : (t + 1) * P, :])
                    src = xt[:rows]
                else:
                    src = x1[0:1, 8, :]
                st = stat_pool.tile([P, 6], F32, tag="bnst")
                nc.vector.bn_stats(st[:rows], src)
                mv = stat_pool.tile([P, 2], F32, tag="bnmv")
                nc.vector.bn_aggr(mv[:rows], st[:rows])
                rstd = stat_pool.tile([P, 1], F32, tag="rstd")
                nc.scalar.activation(rstd[:rows], mv[:rows, 1:2], AF.Sqrt,
                                     bias=eps_t[:rows], scale=1.0)
                nc.vector.reciprocal(rstd[:rows], rstd[:rows])
                z = z_pool.tile([P, D], F32, tag="z")
                nc.vector.tensor_scalar(z[:rows], src, mv[:rows, 0:1],
                                        rstd[:rows],
                                        op0=ALU.subtract, op1=ALU.mult)
                ptt = ps.tile([P, DC, P], F32, tag="mm")
                for d in range(DC):
                    nc.tensor.transpose(ptt[:, d, :rows],
                                        z[:rows, d * P : (d + 1) * P],
                                        ident[:rows, :rows])
                nc.scalar.activation(
                    zTx[:, :, KPF + t * P : KPF + t * P + rows],
                    ptt[:, :, :rows], AF.Copy)
                if t == 15:
                    # wrapped prefix: tokens 1984..2048 = local rows 64..128
                    nc.vector.tensor_copy(zTx[:, :, 0:KPF], ptt[:, :, HD:P])

            for t in range(17):
                ln1(t)
                if 1 <= t <= 15:
                    vproj(t)
                if t == 15:
                    vproj(0)
            vproj(16)

            # ---- Q/K projections ----
            q_blocks = [(KPF, 0, 512), (KPF + 512, 512, 512), (KPF + S, SQ, 1)]
            k_blocks = [(KPF + i * 512, KPF + i * 512, 512) for i in range(4)] \
                + [(KPF + S, KPF + S, 1)]
            for p in range(NPAIR):
                for w_sb, b_sb, dst, blocks in (
                        (wq_sb, bq_sb, QT[p], q_blocks),
                        (wk_sb, bk_sb, KTx[p], k_blocks)):
                    for bi, (src, dcol, w) in enumerate(blocks):
                        s0, w0, keep = (src, w, 0) if w > 1 else (src - 1, 2, 1)
                        pq = ps.tile([P, 512], F32, tag="mm")
                        for d in range(DC):
                            nc.tensor.matmul(pq[:, :w0],
                                             w_sb[:, d, p * P : (p + 1) * P],
                                             zTx[:, d, s0 : s0 + w0],
                                             start=(d == 0), stop=(d == DC - 1))
                        if bi % 2 == 0:
                            nc.scalar.activation(dst[:, dcol : dcol + w],
                                                 pq[:, keep : keep + w], AF.Identity,
                                                 bias=b_sb[:, p : p + 1])
                        else:
                            nc.vector.tensor_scalar(dst[:, dcol : dcol + w],
                                                    pq[:, keep : keep + w],
                                                    b_sb[:, p : p + 1], None,
                                                    op0=ALU.add)
                nc.vector.tensor_copy(KTx[p][:, 0:KPF], KTx[p][:, S : S + KPF])

        # ====== Phase C/D/E/F interleaved: band + global + norm + out_proj + LN2
        with tc.tile_pool(name="bandp", bufs=3) as band_pool, \
             tc.tile_pool(name="ln2z", bufs=2) as z2_pool, \
             tc.tile_pool(name="st2", bufs=4) as stat2_pool:

            # band-phase scratch: quarter-partition l tiles, global-path
            # per-pair scratch, zero-padded stationaries
            ltA = band_pool.tile([P, NQB, QB], BF16, bufs=1)
            ltB = band_pool.tile([P, NQB, QB], BF16, bufs=1)
            lrA = band_pool.tile([P, NQB, QB], BF16, bufs=1)
            lrB = band_pool.tile([P, NQB, QB], BF16, bufs=1)
            lq = band_pool.tile([P, NQB, QB], BF16, bufs=1)
            egpp = [band_pool.tile([2, S], F32, tag=f"egpp{p}", bufs=1,
                                   name=f"egpp{p}") for p in range(NPAIR)]
            laP = [band_pool.tile([2, 4], F32, tag=f"laP{p}", bufs=1,
                                  name=f"laP{p}") for p in range(NPAIR)]
            egp8 = band_pool.tile([8, S], F32, bufs=1)
            la8 = band_pool.tile([8, 4], F32, bufs=1)
            ga8 = band_pool.tile([8, 1], F32, bufs=1)
            larec = band_pool.tile([8, 1], F32, bufs=1)

            # col j of q2g[p] holds head (2p+j)'s global-query column in its
            # own 64 contraction rows; k0h[h] is the zero-padded global-key
            # column; Vg packs each head's global-key V row (+ ones col) at
            # quarter-partition bases so band PV matmuls stay legal
            q2g = [band_pool.tile([P, 2], BF16, tag=f"q2g{p}", bufs=1,
                                  name=f"q2g{p}") for p in range(NPAIR)]
            for pr in range(NPAIR):
                nc.gpsimd.memset(q2g[pr], 0.0)
                for j in range(2):
                    sub = j * HD
                    h = 2 * pr + j
                    nc.gpsimd.tensor_copy(
                        q2g[pr][sub : sub + HD, j : j + 1],
                        QT[pr][sub : sub + HD, SQ : SQ + 1])
                    nc.gpsimd.memset(k0h[h], 0.0)
                    nc.gpsimd.tensor_copy(
                        k0h[h][sub : sub + HD, 0:1],
                        KTx[pr][sub : sub + HD, KPF + S : KPF + S + 1])
                    m = 32 * (h % 3)
                    nc.gpsimd.tensor_copy(Vg[m : m + 1, h, :],
                                          Vsbx[0:1, 16, h, :])

            # global-KEY column scores for all band queries (all 8 heads)
            for h in range(H):
                pr = h // 2
                m = 32 * (h % 3)
                pgX = pgh3[h // 3]
                for half in range(2):
                    gq = ps.tile([P, 512], F32, tag="mm")
                    nc.tensor.matmul(gq[0:1, :], k0h[h],
                                     QT[pr][:, half * 512 : (half + 1) * 512],
                                     start=True, stop=True)
                    nc.scalar.activation(
                        pgX[m : m + 1, half * 512 : (half + 1) * 512],
                        gq[0:1, :], AF.Exp)
                nc.vector.tensor_tensor(pgX[m : m + 1, :], pgX[m : m + 1, :],
                                        mgc_sb[m : m + 1, :], ALU.mult)

            def ln2(t):
                rows = P if t < 8 else 1
                src = x1[:rows, t, :]
                st = stat2_pool.tile([P, 6], F32, tag="bnst2")
                nc.vector.bn_stats(st[:rows], src)
                mv = stat2_pool.tile([P, 2], F32, tag="bnmv2")
                nc.vector.bn_aggr(mv[:rows], st[:rows])
                rstd = stat2_pool.tile([P, 1], F32, tag="rstd2")
                nc.scalar.activation(rstd[:rows], mv[:rows, 1:2], AF.Sqrt,
                                     bias=eps_t[:rows], scale=1.0)
                nc.vector.reciprocal(rstd[:rows], rstd[:rows])
                z2 = z2_pool.tile([P, D], F32, tag="z2")
                nc.vector.tensor_scalar(z2[:rows], src, mv[:rows, 0:1],
                                        rstd[:rows],
                                        op0=ALU.subtract, op1=ALU.mult)
                ptt = ps.tile([P, DC, P], F32, tag="mm")
                for d in range(DC):
                    nc.tensor.transpose(ptt[:, d, :rows],
                                        z2[:rows, d * P : (d + 1) * P],
                                        ident[:rows, :rows])
                nc.scalar.activation(z2T[:, :, t * P : t * P + rows],
                                     ptt[:, :, :rows], AF.Copy)

            def out_proj(t):
                w = P if t < 8 else 1
                yp = ps.tile([P, D], F32, tag="mm")
                if w > 1:
                    for p in range(NPAIR):
                        nc.tensor.matmul(yp[:w], _r(oT[p][:, t * P : t * P + w]),
                                         _r(wo_sb[:, p, :]),
                                         start=(p == 0), stop=False)
                else:
                    for p in range(NPAIR):
                        nc.tensor.matmul(yp[:w], _r(oT[p][:, SQ : SQ + 1]),
                                         _r(wo_sb[:, p, :]),
                                         start=(p == 0), stop=False)
                nc.tensor.matmul(yp[:w], _r(ones_row[:1, :w]), _r(bo_sb),
                                 start=False, stop=True)
                nc.vector.tensor_tensor(x1[:w, t, :], yp[:w], x1[:w, t, :], ALU.add)

            def global_scores(pr, tcb):
                gs = ps.tile([P, 512], F32, tag="mm")
                nc.tensor.matmul(gs[0:2, :], q2g[pr],
                                 KTx[pr][:, tcb * 512 : (tcb + 1) * 512],
                                 start=True, stop=False)
                # additive key mask is head-independent -> rank-1 matmul add
                nc.tensor.matmul(gs[0:2, :], _r(onesP[0:1, 0:2]),
                                 mgrow_sb[0:1, tcb * 512 : (tcb + 1) * 512],
                                 start=False, stop=True)
                nc.scalar.activation(
                    egpp[pr][:, tcb * 512 : (tcb + 1) * 512],
                    gs[0:2, :], AF.Exp,
                    accum_out=laP[pr][:, tcb : tcb + 1])

            def global_gather():
                for pr in range(NPAIR):
                    nc.gpsimd.dma_start(egp8[2 * pr : 2 * pr + 2, :], egpp[pr][:])
                    nc.gpsimd.dma_start(la8[2 * pr : 2 * pr + 2, :], laP[pr][:])

            def global_transposes():
                nc.vector.tensor_reduce(ga8, la8, axis=AXL.X, op=ALU.add)
                nc.vector.reciprocal(larec, ga8)
                for c in range(16):
                    ptb = ps.tile([P, QB], F32, tag="sm")
                    nc.tensor.transpose(ptb[:, 0:8], egp8[0:8, c * P : (c + 1) * P],
                                        ident[0:8, 0:8])
                    nc.scalar.activation(pgT[:, c, :], ptb[:, 0:8], AF.Copy)

            def global_pv():
                for g in range(2):
                    pog = ps.tile([P, 512], F32, tag="mm")
                    for c in range(16):
                        nc.tensor.matmul(pog[0:8, 0 : 4 * HD], pgT[:, c, :],
                                         Vsbx[:, c, 4 * g : 4 * g + 4, 0:HD],
                                         start=(c == 0), stop=(c == 15))
                    pog_sb = band_pool.tile([8, 4 * HD], F32, tag="pog_sb")
                    nc.scalar.activation(pog_sb, pog[0:8, 0 : 4 * HD], AF.Copy,
                                         scale=larec[0:8, 0:1])
                    for j in range(2):
                        ptj = ps.tile([P, QB], F32, tag="sm")
                        nc.tensor.transpose(ptj[:, 0:8],
                                            pog_sb[0:8, j * P : (j + 1) * P],
                                            ident[0:8, 0:8])
                        for hh in (2 * j, 2 * j + 1):
                            h = 4 * g + hh
                            rlo = (hh % 2) * HD
                            nc.scalar.activation(
                                oT[h // 2][rlo : rlo + HD, SQ : SQ + 1].bitcast(F32R),
                                ptj[rlo : rlo + HD, h : h + 1], AF.Copy)

            for i in range(NQB):
                for h in range(H):
                    pr, sub = h // 2, (h % 2) * HD
                    q_ap = QT[pr][sub : sub + HD, i * QB : (i + 1) * QB]
                    sc = ps.tile([P, NKC, QB], F32, tag="sc", bufs=2)
                    for c in range(NKC):
                        nc.tensor.matmul(
                            sc[:, c, :],
                            KTx[pr][sub : sub + HD,
                                    QB * i + c * P : QB * i + c * P + P],
                            q_ap, start=True, stop=True)
                    pT = band_pool.tile([P, NKC, QB], BF16, tag="pT")
                    nc.scalar.activation(pT, sc, AF.Exp)
                    nc.vector.tensor_tensor(pT, pT, mb_sb[:, i, :, :], ALU.mult)
                    po = ps.tile([P, QB], F32, tag="sm")
                    for c in range(NKC):
                        nc.tensor.matmul(po[0 : HD + 1, :], Vsbx[:, 2 * i + c, h, :],
                                         pT[:, c, :], start=(c == 0), stop=False)
                    m = 32 * (h % 3)
                    pgX = pgh3[h // 3]
                    nc.tensor.matmul(po[0 : HD + 1, :], Vg[m : m + 1, h, :],
                                     pgX[m : m + 1, i * QB : (i + 1) * QB],
                                     start=False, stop=True)
                    if h % 2 == 0:
                        nc.vector.tensor_copy(
                            oT[pr][sub : sub + HD, i * QB : (i + 1) * QB].bitcast(F32R),
                            po[0:HD, :])
                    else:
                        nc.scalar.activation(
                            oT[pr][sub : sub + HD, i * QB : (i + 1) * QB].bitcast(F32R),
                            po[0:HD, :], AF.Copy)
                    ml = 32 * (h % 4)
                    ltX = ltA if h < 4 else ltB
                    nc.scalar.activation(
                        ltX[ml : ml + 1, i, :],
                        po[HD : HD + 1, :], AF.Copy)

                # normalize block i: partition-parallel reciprocals over the
                # quarter-row l tiles (DVE allows base 96); matmul operands
                # cannot sit at base 96, so heads 3/7 get moved to lq first
                with nc.allow_low_precision(reason="bf16 softmax sums"):
                    nc.vector.reciprocal(lrA[:, i, :], ltA[:, i, :])
                    nc.vector.reciprocal(lrB[:, i, :], ltB[:, i, :])
                nc.vector.tensor_copy(lq[0:1, i, :], lrA[96:97, i, :])
                nc.vector.tensor_copy(lq[32:33, i, :], lrB[96:97, i, :])

                def lsrc(h):
                    if h % 4 == 3:
                        return lq, 32 * (h // 4)
                    return (lrA if h < 4 else lrB), 32 * (h % 4)

                for p in range(NPAIR):
                    for j in range(2):
                        lt_t, r = lsrc(2 * p + j)
                        lbc = ps.tile([P, QB], F32, tag="sm")
                        nc.tensor.matmul(lbc[0:HD, :],
                                         onesPb[r : r + 1, :],
                                         lt_t[r : r + 1, i, :],
                                         start=True, stop=True)
                        rows = oT[p][j * HD : (j + 1) * HD,
                                     i * QB : (i + 1) * QB]
                        nc.vector.tensor_tensor(rows.bitcast(F32R), rows,
                                                lbc[0:HD, :], ALU.mult)

                # pipeline: out_proj for the two finished token tiles
                out_proj(2 * i)
                out_proj(2 * i + 1)

                # interleave the global-token path under the band blocks
                if i == 0:
                    for tcb in range(4):
                        global_scores(0, tcb)
                        global_scores(1, tcb)
                elif i == 1:
                    for tcb in range(4):
                        global_scores(2, tcb)
                        global_scores(3, tcb)
                    global_gather()
                elif i == 2:
                    global_transposes()
                else:
                    global_pv()

            out_proj(8)
            # LN2 runs as its own pass: keeps the scalar engine's activation
            # table stable (no EXP<->SQRT thrash inside the band loop)
            for t in range(9):
                ln2(t)

        # band scratch released; fetch FFN2 weights under out_proj/FFN1
        ffw = ctx.enter_context(tc.tile_pool(name="ffw", bufs=1))
        w2_sb = ffw.tile([P, FFC, D], BF16)
        nc.sync.dma_start(w2_sb, w2_pc[:])

        # ====== Phase G: FFN + residual -> y ======
        with tc.tile_pool(name="ffn", bufs=1) as ffn_pool, \
             tc.tile_pool(name="ffo", bufs=3) as out_pool:
            for t0, tw in [(0, 512), (512, 512), (SQ, 1)]:
                hT = ffn_pool.tile([P, FFC, 512], BF16, tag="hT")
                s0, w0, keep = (t0, tw, 0) if tw > 1 else (t0 - 1, 2, 1)
                for f in range(FFC):
                    ph = ps.tile([P, 512], F32, tag="mm")
                    for d in range(DC):
                        nc.tensor.matmul(ph[:, :w0],
                                         w1_sb[:, d, f * P : (f + 1) * P],
                                         z2T[:, d, s0 : s0 + w0],
                                         start=(d == 0), stop=(d == DC - 1))
                    nc.scalar.activation(hT[:, f, :tw],
                                         ph[:, keep : keep + tw], AF.Gelu,
                                         bias=b1_sb[:, f : f + 1])
                nsub = 4 if tw == 512 else 1
                for stp in range(nsub):
                    sw = P if tw == 512 else 1
                    ps2 = ps.tile([P, D], F32, tag="mm")
                    for f in range(FFC):
                        nc.tensor.matmul(ps2[:sw],
                                         hT[:, f, stp * P : stp * P + sw],
                                         w2_sb[:, f, :],
                                         start=(f == 0), stop=False)
                    nc.tensor.matmul(ps2[:sw], _r(ones_row[:1, :sw]), _r(b2_sb),
                                     start=False, stop=True)
                    yt = out_pool.tile([P, D], F32, tag="yt")
                    tglob = t0 // P + stp
                    nc.vector.tensor_tensor(yt[:sw], ps2[:sw],
                                            x1[:sw, tglob, :], ALU.add)
                    nc.gpsimd.dma_start(
                        y_out[t0 + stp * P : t0 + stp * P + sw, :], yt[:sw])

    nc.finalize()
    return nc


def make_host_inputs(x, padding_mask, attn_mask, in_proj_w, in_proj_b, out_proj_w,
                     out_proj_b, ln1_g, ln1_b, ln2_g, ln2_b, ff_w1, ff_b1, ff_w2,
                     ff_b2):
    """Build the 8 per-core input maps (numpy only)."""
    import ml_dtypes
    f32 = np.float32
    bf16 = ml_dtypes.bfloat16
    x = np.asarray(x, f32)
    attn_mask = np.asarray(attn_mask, f32)
    padding_mask = np.asarray(padding_mask, bool)

    g1 = np.asarray(ln1_g, f32); b1 = np.asarray(ln1_b, f32)
    g2 = np.asarray(ln2_g, f32); b2 = np.asarray(ln2_b, f32)
    Wq, Wk, Wv = (np.asarray(in_proj_w[i * D:(i + 1) * D], f32) for i in range(3))
    bq0, bk0, bv0 = (np.asarray(in_proj_b[i * D:(i + 1) * D], f32) for i in range(3))
    sc = 1.0 / np.sqrt(HD)

    Wq_ = Wq * g1[None, :] * sc
    bq_ = (Wq @ b1 + bq0) * sc
    Wk_ = Wk * g1[None, :]
    bk_ = Wk @ b1 + bk0
    Wv_ = Wv * g1[None, :]
    bv_ = Wv @ b1 + bv0
    W1_ = np.asarray(ff_w1, f32) * g2[None, :]
    b1f = np.asarray(ff_w1, f32) @ b2 + np.asarray(ff_b1, f32)

    def pc(wt, nchunk):  # [Dout, Din] -> [P, nchunk, Dout] chunked on Din
        return np.ascontiguousarray(
            wt.T.reshape(nchunk, P, wt.shape[0]).transpose(1, 0, 2))

    shared = {
        "wq_pc": pc(Wq_, DC).astype(bf16), "wk_pc": pc(Wk_, DC).astype(bf16),
        "wv_pc": pc(Wv_, DC).astype(bf16),
        "bq_pc": np.ascontiguousarray(bq_.reshape(DC, P).T),
        "bk_pc": np.ascontiguousarray(bk_.reshape(DC, P).T),
        "bv_row": bv_[None, :].copy(),
        "wo_pc": pc(np.asarray(out_proj_w, f32), DC),
        "bo_row": np.asarray(out_proj_b, f32)[None, :].copy(),
        "w1_pc": pc(W1_, DC).astype(bf16),
        "b1_pc": np.ascontiguousarray(b1f.reshape(FFC, P).T),
        "w2_pc": pc(np.asarray(ff_w2, f32), FFC).astype(bf16),
        "b2_row": np.asarray(ff_b2, f32)[None, :].copy(),
        "ones_in": np.ones((1, P), f32),
        "onesp_in": np.ones((P, P), f32),
    }

    in_maps = []
    for core in range(8):
        b = core // 2
        h = core % 2
        rot = np.roll(x[b], -1024 * h, axis=0)
        x_nat = np.ascontiguousarray(np.concatenate([rot, x[b, 0:1]], axis=0))

        # additive mask for this batch -> multiplicative factor
        A = attn_mask + np.where(padding_mask[b], -np.inf, 0.0)[None, :]
        mfac = np.exp(np.minimum(A, 0.0)).astype(f32)  # exp(-inf)=0, exp(0)=1
        mfac[~np.isfinite(A)] = 0.0

        # band masks: [P(t), NQB(i), NKC(c), QB(r)];
        # chunk c of block i covers rotated keys [256i - 64 + 128c, +128)
        i_idx = np.arange(NQB)[:, None, None, None]
        c_idx = np.arange(NKC)[None, :, None, None]
        t_idx = np.arange(P)[None, None, :, None]
        r_idx = np.arange(QB)[None, None, None, :]
        k_rot = (QB * i_idx - KPF + P * c_idx + t_idx) % S
        q_rot = i_idx * QB + r_idx
        gq = (q_rot + 1024 * h) % S
        gk = (k_rot + 1024 * h) % S
        band = mfac[gq, gk]                       # [NQB, NKC, P, QB]
        mask_band = np.ascontiguousarray(band.transpose(2, 0, 1, 3)).astype(bf16)

        # global-key column mask, zeroed when key0 falls inside the window
        key0_rot = (0 - 1024 * h) % S
        gq2 = (np.arange(NQB)[:, None] * QB + np.arange(QB)[None, :] + 1024 * h) % S
        gcol = mfac[gq2, 0].copy()
        for i in range(NQB):
            off = (key0_rot - (QB * i - KPF)) % S
            if off < NKC * P:
                gcol[i, :] = 0.0  # key 0 already inside this block's band window
        mask_gcol1 = np.ascontiguousarray(gcol.reshape(1, -1)).astype(bf16)

        # global-query additive mask row, in SHIFTED key order:
        # score col kappa <-> rotated key (kappa - 64) mod S
        kap = np.arange(S)
        k_act = (((kap - KPF) % S) + 1024 * h) % S
        mask_grow = np.ascontiguousarray(
            np.maximum(A[0, k_act], NEG)[None, :].astype(f32))

        m = dict(shared)
        m.update({
            "x_nat": x_nat,
            "mask_band": mask_band,
            "mask_gcol1": mask_gcol1,
            "mask_grow": mask_grow,
        })
        in_maps.append(m)
    return in_maps


def assemble_output(results):
    """results: list of 8 dicts with 'y' [NT, D] -> full [B, S, D]."""
    out = np.empty((B, S, D), np.float32)
    for b in range(B):
        y0 = results[2 * b]["y"]
        y1 = results[2 * b + 1]["y"]
        out[b, 0] = y0[SQ]
        out[b, 1:SQ] = y0[1:SQ]
        out[b, SQ:] = y1[0:SQ]
    return out


_CACHED_NC = None


def kernel(**inputs) -> np.ndarray:
    global _CACHED_NC
    from concourse.bass_utils import run_bass_kernel_spmd

    in_maps = make_host_inputs(**inputs)
    if _CACHED_NC is None:
        _CACHED_NC = build_module()
    res = run_bass_kernel_spmd(_CACHED_NC, in_maps, core_ids=list(range(8)))
    return assemble_output(res.results)


if __name__ == "__main__":
    nc = build_module()
    print("build + compile OK")


# revision 40
# speedup vs baseline: 1.3454x; 1.0398x over previous
"""LocalGlobalTransformerEncoderBlock on 8 Trainium2 NeuronCores.

Sharding: core = (batch b = core//2, sequence half h = core%2). Each core
computes the full encoder block for 1024 query rows of one batch plus the
global token (sequence position 0). The per-core sequence is ROTATED by
1024*h so the core's query rows are always rotated rows [0, 1024), and
x[b, 0] (the global token) is appended as row 2048.

v2 redesign vs the 481us baseline:
  - Band attention uses 3 unaligned 128-key chunks per 256-query block
    (window [256i-64, 256i+320) mod 2048) via a 64-col wrapped prefix on
    the transposed K / z buffers and half-shifted V key tiles.
  - Attention tensors (Q^T, K^T, V, probs, masks) and FFN weights/hidden
    are bf16: same PE rate as fp32r but half the SBUF/DMA and 2x DVE.
  - Softmax denominators are gathered into a [32, 256] tile so ONE
    partition-parallel reciprocal replaces 16 serial [64,512] ones.
  - The global-token path is per-head wide matmuls + one fused softmax,
    emitted interleaved with the band blocks so it hides under them.
  - Normalize/out_proj/LN2 are emitted per band block (software pipeline);
    FFN weights prefetch during attention; x is loaded once into the
    residual buffer.

Self-contained: only imports from /opt/trn_rl_repo (the installed bass
runtime), numpy, and stdlib.
"""

import sys
from contextlib import ExitStack

if "/opt/trn_rl_repo" not in sys.path:
    sys.path.insert(0, "/opt/trn_rl_repo")

import numpy as np

import concourse.bass as bass
import concourse.bacc as bacc_mod
import concourse.mybir as mybir
import concourse.tile as tile
from concourse.masks import make_identity

P = 128
B, S, D, H, FF = 4, 2048, 512, 8, 2048
HD = D // H            # 64
DC = D // P            # 4 chunks of the model dim
FFC = FF // P          # 16 chunks of the FF dim
SK = S + 1             # 2049 tokens (2048 rotated + appended global token)
SQ = 1024              # band queries per core
NT = SQ + 1            # 1025 output tokens
QB = 256               # band query block
NQB = SQ // QB         # 4
NKC = 3                # 128-key chunks per band window
KPF = 64               # wrapped key/token prefix columns
NPAIR = H // 2         # 4 head-pair tiles
EPS = 1e-5
NEG = -1e30

F32 = mybir.dt.float32
F32R = mybir.dt.float32r
BF16 = mybir.dt.bfloat16
AF = mybir.ActivationFunctionType
ALU = mybir.AluOpType
AXL = mybir.AxisListType


def _r(ap):
    """Reinterpret an fp32 AP as float32r for full-rate PE matmuls."""
    return ap.bitcast(F32R)


def build_module():
    nc = bacc_mod.Bacc("TRN2", target_bir_lowering=False)

    x_nat = nc.dram_tensor("x_nat", [SK, D], F32, kind="ExternalInput")
    wq_pc = nc.dram_tensor("wq_pc", [P, DC, D], BF16, kind="ExternalInput")
    wk_pc = nc.dram_tensor("wk_pc", [P, DC, D], BF16, kind="ExternalInput")
    wv_pc = nc.dram_tensor("wv_pc", [P, DC, D], BF16, kind="ExternalInput")
    bq_pc = nc.dram_tensor("bq_pc", [P, DC], F32, kind="ExternalInput")
    bk_pc = nc.dram_tensor("bk_pc", [P, DC], F32, kind="ExternalInput")
    bv_row = nc.dram_tensor("bv_row", [1, D], F32R, kind="ExternalInput")
    wo_pc = nc.dram_tensor("wo_pc", [P, DC, D], F32R, kind="ExternalInput")
    bo_row = nc.dram_tensor("bo_row", [1, D], F32R, kind="ExternalInput")
    w1_pc = nc.dram_tensor("w1_pc", [P, DC, FF], BF16, kind="ExternalInput")
    b1_pc = nc.dram_tensor("b1_pc", [P, FFC], F32, kind="ExternalInput")
    w2_pc = nc.dram_tensor("w2_pc", [P, FFC, D], BF16, kind="ExternalInput")
    b2_row = nc.dram_tensor("b2_row", [1, D], F32R, kind="ExternalInput")
    mask_band = nc.dram_tensor("mask_band", [P, NQB, NKC, QB], BF16, kind="ExternalInput")
    mask_gcol1 = nc.dram_tensor("mask_gcol1", [1, SQ], BF16, kind="ExternalInput")
    mask_grow = nc.dram_tensor("mask_grow", [1, S], F32R, kind="ExternalInput")
    ones_in = nc.dram_tensor("ones_in", [1, P], F32R, kind="ExternalInput")
    onesp_in = nc.dram_tensor("onesp_in", [P, P], F32R, kind="ExternalInput")
    y_out = nc.dram_tensor("y", [NT, D], F32, kind="ExternalOutput")

    with tile.TileContext(nc) as tc, ExitStack() as ctx:
        # ---- persistent state ----
        persist = ctx.enter_context(tc.tile_pool(name="persist", bufs=1))
        x1 = persist.tile([P, 9, D], F32)
        nc.gpsimd.dma_start(x1[:, 0, :], x_nat[0:P, :])
        nc.gpsimd.dma_start(
            x1[:, 1:4, :], x_nat[P:512, :].rearrange("(t p) d -> p t d", t=3))
        nc.gpsimd.dma_start(
            x1[:, 4:8, :], x_nat[512:1024, :].rearrange("(t p) d -> p t d", t=4))
        nc.gpsimd.dma_start(x1[0:1, 8, :], x_nat[S : S + 1, :])
        ident = persist.tile([P, P], F32)
        make_identity(nc, ident)
        ones_row = persist.tile([1, P], F32R)
        nc.sync.dma_start(ones_row, ones_in[:])
        eps_t = persist.tile([P, 1], F32)
        nc.vector.memset(eps_t, EPS)
        bo_sb = persist.tile([1, D], F32R)
        nc.sync.dma_start(bo_sb, bo_row[:])
        b1_sb = persist.tile([P, FFC], F32)
        nc.sync.dma_start(b1_sb, b1_pc[:])
        b2_sb = persist.tile([1, D], F32R)
        nc.sync.dma_start(b2_sb, b2_row[:])

        oT = [persist.tile([P, NT], F32, name=f"oT{p}") for p in range(NPAIR)]
        z2T = persist.tile([P, DC, NT], BF16)
        pgT = persist.tile([P, 16, 8], BF16)
        onesP = persist.tile([P, P], F32R)
        nc.sync.dma_start(onesP, onesp_in[:])
        onesPb = persist.tile([P, HD], BF16)
        nc.gpsimd.memset(onesPb, 1.0)

        # ---- PSUM pool: mm(2) + sm(2) + sc(2) + smb(2) = 8 banks ----
        ps = ctx.enter_context(tc.tile_pool(name="ps", bufs=2, space="PSUM"))

        # ---- attention state (lives through band phase) ----
        attn = ctx.enter_context(tc.tile_pool(name="attn", bufs=1))
        QT = [attn.tile([P, NT], BF16, name=f"QT{p}") for p in range(NPAIR)]
        KTx = [attn.tile([P, KPF + SK], BF16, name=f"KTx{p}") for p in range(NPAIR)]
        Vsbx = attn.tile([P, 17, H, HD + 1], BF16)
        nc.gpsimd.memset(Vsbx[:, :, :, HD], 1.0)
        # per-head global-key V rows / probs at quarter-partition bases
        Vg = attn.tile([P, H, HD + 1], BF16)
        pgh3 = [attn.tile([P, SQ], BF16, name=f"pgh3_{t}") for t in range(3)]
        k0h = [attn.tile([P, 1], BF16, name=f"k0h{h}") for h in range(H)]
        mb_sb = attn.tile([P, NQB, NKC, QB], BF16)
        mgc_sb = attn.tile([P, SQ], BF16)
        mgrow_sb = attn.tile([1, S], F32R)

        wo_sb = attn.tile([P, DC, D], F32R)
        w1_sb = attn.tile([P, DC, FF], BF16)

        # x loads on the gpsimd queue; weights/masks on the sync queue
        nc.sync.dma_start(mb_sb, mask_band[:])
        nc.sync.dma_start(mgc_sb[0:1, :], mask_gcol1[:])
        nc.sync.dma_start(mgrow_sb, mask_grow[:])
        for mrow in (32, 64):
            nc.vector.tensor_copy(mgc_sb[mrow : mrow + 1, :], mgc_sb[0:1, :])

        # ====== Phase A+B: LN1 -> zTx, V (interleaved), Q/K ======
        with tc.tile_pool(name="phA", bufs=1) as pha, \
             tc.tile_pool(name="lnz", bufs=2) as z_pool, \
             tc.tile_pool(name="st1", bufs=4) as stat_pool:
            zTx = pha.tile([P, DC, KPF + SK], BF16)
            wv_sb = pha.tile([P, DC, D], BF16)
            wq_sb = pha.tile([P, DC, D], BF16)
            wk_sb = pha.tile([P, DC, D], BF16)
            bq_sb = pha.tile([P, DC], F32)
            bk_sb = pha.tile([P, DC], F32)
            bv_sb = pha.tile([1, D], F32R)

            nc.sync.dma_start(wv_sb, wv_pc[:])
            nc.sync.dma_start(bv_sb, bv_row[:])
            nc.sync.dma_start(wq_sb, wq_pc[:])
            nc.sync.dma_start(bq_sb, bq_pc[:])
            nc.sync.dma_start(wk_sb, wk_pc[:])
            nc.sync.dma_start(bk_sb, bk_pc[:])
            nc.sync.dma_start(wo_sb, wo_pc[:])
            nc.sync.dma_start(w1_sb, w1_pc[:])

            def vproj_mm(j):
                """V projection for shifted key tile j (tokens [128j-64,128j+64))."""
                rows = P if j < 16 else 1
                col0 = 128 * j if j < 16 else KPF + S
                pv = ps.tile([P, D], F32, tag="mm")
                for d in range(DC):
                    nc.tensor.matmul(pv[:rows], zTx[:, d, col0 : col0 + rows],
                                     wv_sb[:, d, :],
                                     start=(d == 0), stop=False)
                nc.tensor.matmul(pv[:rows], _r(ones_row[:1, :rows]), _r(bv_sb),
                                 start=False, stop=True)
                return pv, rows

            def vproj_copy(j, pv, rows):
                nc.scalar.activation(
                    Vsbx[:rows, j, :, 0:HD],
                    pv[:rows].rearrange("p (h e) -> p h e", h=H), AF.Copy)

            def ln1(t):
                rows = P if t < 16 else 1
                if t < 8:
                    src = x1[:rows, t, :]
                elif t < 16:
                    xt = z_pool.tile([P, D], F32, tag="xt")
                    nc.gpsimd.dma_start(xt, x_nat[t * P : (t + 1) * P, :])
                    src = xt[:rows]
                else:
                    src = x1[0:1, 8, :]
                st = stat_pool.tile([P, 6], F32, tag="bnst")
                nc.vector.bn_stats(st[:rows], src)
                mv = stat_pool.tile([P, 2], F32, tag="bnmv")
                nc.vector.bn_aggr(mv[:rows], st[:rows])
                rstd = stat_pool.tile([P, 1], F32, tag="rstd")
                nc.scalar.activation(rstd[:rows], mv[:rows, 1:2], AF.Sqrt,
                                     bias=eps_t[:rows], scale=1.0)
                nc.vector.reciprocal(rstd[:rows], rstd[:rows])
                z = z_pool.tile([P, D], F32, tag="z")
                nc.vector.tensor_scalar(z[:rows], src, mv[:rows, 0:1],
                                        rstd[:rows],
                                        op0=ALU.subtract, op1=ALU.mult)
                ptt = ps.tile([P, DC, P], F32, tag="mm")
                for d in range(DC):
                    nc.tensor.transpose(ptt[:, d, :rows],
                                        z[:rows, d * P : (d + 1) * P],
                                        ident[:rows, :rows])
                return ptt

            def zcopy(t, ptt):
                rows = P if t < 16 else 1
                nc.scalar.activation(
                    zTx[:, :, KPF + t * P : KPF + t * P + rows],
                    ptt[:, :, :rows], AF.Copy)
                if t == 15:
                    # wrapped prefix: tokens 1984..2048 = local rows 64..128
                    nc.vector.tensor_copy(zTx[:, :, 0:KPF], ptt[:, :, HD:P])

            # one-tile software pipeline: tile t's PSUM->SBUF copies run
            # behind tile t+1's LN in the scalar stream; V matmuls for key
            # tile j follow zcopy(j), their copy one step later again
            zpend = {}
            vpend = {}
            for t in range(19):
                if t <= 16:
                    zpend[t] = ln1(t)
                if 1 <= t <= 17:
                    zcopy(t - 1, zpend.pop(t - 1))
                vjs = []
                if 2 <= t <= 16:
                    vjs.append(t - 1)
                elif t == 17:
                    vjs += [0, 16]
                for j in vjs:
                    vpend[j] = vproj_mm(j)
                for j in list(vpend):
                    if (t >= 3 and j <= t - 2) or t >= 18:
                        vproj_copy(j, *vpend.pop(j))

            # ---- Q/K projections ----
            q_blocks = [(KPF, 0, 512), (KPF + 512, 512, 512), (KPF + S, SQ, 1)]
            k_blocks = [(KPF + i * 512, KPF + i * 512, 512) for i in range(4)] \
                + [(KPF + S, KPF + S, 1)]
            for p in range(NPAIR):
                for w_sb, b_sb, dst, blocks in (
                        (wq_sb, bq_sb, QT[p], q_blocks),
                        (wk_sb, bk_sb, KTx[p], k_blocks)):
                    for bi, (src, dcol, w) in enumerate(blocks):
                        s0, w0, keep = (src, w, 0) if w > 1 else (src - 1, 2, 1)
                        pq = ps.tile([P, 512], F32, tag="mm")
                        for d in range(DC):
                            nc.tensor.matmul(pq[:, :w0],
                                             w_sb[:, d, p * P : (p + 1) * P],
                                             zTx[:, d, s0 : s0 + w0],
                                             start=(d == 0), stop=(d == DC - 1))
                        if bi % 2 == 0:
                            nc.scalar.activation(dst[:, dcol : dcol + w],
                                                 pq[:, keep : keep + w], AF.Identity,
                                                 bias=b_sb[:, p : p + 1])
                        else:
                            nc.vector.tensor_scalar(dst[:, dcol : dcol + w],
                                                    pq[:, keep : keep + w],
                                                    b_sb[:, p : p + 1], None,
                                                    op0=ALU.add)
                nc.vector.tensor_copy(KTx[p][:, 0:KPF], KTx[p][:, S : S + KPF])

        # ====== Phase C/D/E/F interleaved: band + global + norm + out_proj + LN2
        with tc.tile_pool(name="bandp", bufs=3) as band_pool, \
             tc.tile_pool(name="ln2z", bufs=2) as z2_pool, \
             tc.tile_pool(name="st2", bufs=4) as stat2_pool:

            # band-phase scratch: quarter-partition l tiles, global-path
            # per-pair scratch, zero-padded stationaries
            ltA = band_pool.tile([P, NQB, QB], BF16, bufs=1)
            ltB = band_pool.tile([P, NQB, QB], BF16, bufs=1)
            lrA = band_pool.tile([P, NQB, QB], BF16, bufs=1)
            lrB = band_pool.tile([P, NQB, QB], BF16, bufs=1)
            lq = band_pool.tile([P, NQB, QB], BF16, bufs=1)
            egpp = [band_pool.tile([2, S], F32, tag=f"egpp{p}", bufs=1,
                                   name=f"egpp{p}") for p in range(NPAIR)]
            laP = [band_pool.tile([2, 4], F32, tag=f"laP{p}", bufs=1,
                                  name=f"laP{p}") for p in range(NPAIR)]
            egp8 = band_pool.tile([8, S], F32, bufs=1)
            la8 = band_pool.tile([8, 4], F32, bufs=1)
            ga8 = band_pool.tile([8, 1], F32, bufs=1)
            larec = band_pool.tile([8, 1], F32, bufs=1)

            # col j of q2g[p] holds head (2p+j)'s global-query column in its
            # own 64 contraction rows; k0h[h] is the zero-padded global-key
            # column; Vg packs each head's global-key V row (+ ones col) at
            # quarter-partition bases so band PV matmuls stay legal
            q2g = [band_pool.tile([P, 2], BF16, tag=f"q2g{p}", bufs=1,
                                  name=f"q2g{p}") for p in range(NPAIR)]
            for pr in range(NPAIR):
                nc.gpsimd.memset(q2g[pr], 0.0)
                for j in range(2):
                    sub = j * HD
                    h = 2 * pr + j
                    nc.gpsimd.tensor_copy(
                        q2g[pr][sub : sub + HD, j : j + 1],
                        QT[pr][sub : sub + HD, SQ : SQ + 1])
                    nc.gpsimd.memset(k0h[h], 0.0)
                    nc.gpsimd.tensor_copy(
                        k0h[h][sub : sub + HD, 0:1],
                        KTx[pr][sub : sub + HD, KPF + S : KPF + S + 1])
                    m = 32 * (h % 3)
                    nc.gpsimd.tensor_copy(Vg[m : m + 1, h, :],
                                          Vsbx[0:1, 16, h, :])

            # global-KEY column scores for all band queries (all 8 heads)
            for h in range(H):
                pr = h // 2
                m = 32 * (h % 3)
                pgX = pgh3[h // 3]
                for half in range(2):
                    gq = ps.tile([P, 512], F32, tag="mm")
                    nc.tensor.matmul(gq[0:1, :], k0h[h],
                                     QT[pr][:, half * 512 : (half + 1) * 512],
                                     start=True, stop=True)
                    nc.scalar.activation(
                        pgX[m : m + 1, half * 512 : (half + 1) * 512],
                        gq[0:1, :], AF.Exp)
                nc.vector.tensor_tensor(pgX[m : m + 1, :], pgX[m : m + 1, :],
                                        mgc_sb[m : m + 1, :], ALU.mult)

            def ln2(t):
                rows = P if t < 8 else 1
                src = x1[:rows, t, :]
                st = stat2_pool.tile([P, 6], F32, tag="bnst2")
                nc.vector.bn_stats(st[:rows], src)
                mv = stat2_pool.tile([P, 2], F32, tag="bnmv2")
                nc.vector.bn_aggr(mv[:rows], st[:rows])
                rstd = stat2_pool.tile([P, 1], F32, tag="rstd2")
                nc.scalar.activation(rstd[:rows], mv[:rows, 1:2], AF.Sqrt,
                                     bias=eps_t[:rows], scale=1.0)
                nc.vector.reciprocal(rstd[:rows], rstd[:rows])
                z2 = z2_pool.tile([P, D], F32, tag="z2")
                nc.vector.tensor_scalar(z2[:rows], src, mv[:rows, 0:1],
                                        rstd[:rows],
                                        op0=ALU.subtract, op1=ALU.mult)
                ptt = ps.tile([P, DC, P], F32, tag="mm")
                for d in range(DC):
                    nc.tensor.transpose(ptt[:, d, :rows],
                                        z2[:rows, d * P : (d + 1) * P],
                                        ident[:rows, :rows])
                nc.scalar.activation(z2T[:, :, t * P : t * P + rows],
                                     ptt[:, :, :rows], AF.Copy)

            def out_proj(t):
                w = P if t < 8 else 1
                yp = ps.tile([P, D], F32, tag="mm")
                if w > 1:
                    for p in range(NPAIR):
                        nc.tensor.matmul(yp[:w], _r(oT[p][:, t * P : t * P + w]),
                                         _r(wo_sb[:, p, :]),
                                         start=(p == 0), stop=False)
                else:
                    for p in range(NPAIR):
                        nc.tensor.matmul(yp[:w], _r(oT[p][:, SQ : SQ + 1]),
                                         _r(wo_sb[:, p, :]),
                                         start=(p == 0), stop=False)
                nc.tensor.matmul(yp[:w], _r(ones_row[:1, :w]), _r(bo_sb),
                                 start=False, stop=True)
                nc.vector.tensor_tensor(x1[:w, t, :], yp[:w], x1[:w, t, :], ALU.add)

            def global_scores(pr, tcb):
                gs = ps.tile([P, 512], F32, tag="mm")
                nc.tensor.matmul(gs[0:2, :], q2g[pr],
                                 KTx[pr][:, tcb * 512 : (tcb + 1) * 512],
                                 start=True, stop=False)
                # additive key mask is head-independent -> rank-1 matmul add
                nc.tensor.matmul(gs[0:2, :], _r(onesP[0:1, 0:2]),
                                 mgrow_sb[0:1, tcb * 512 : (tcb + 1) * 512],
                                 start=False, stop=True)
                nc.scalar.activation(
                    egpp[pr][:, tcb * 512 : (tcb + 1) * 512],
                    gs[0:2, :], AF.Exp,
                    accum_out=laP[pr][:, tcb : tcb + 1])

            def global_gather():
                for pr in range(NPAIR):
                    nc.gpsimd.dma_start(egp8[2 * pr : 2 * pr + 2, :], egpp[pr][:])
                    nc.gpsimd.dma_start(la8[2 * pr : 2 * pr + 2, :], laP[pr][:])

            def global_transposes():
                nc.vector.tensor_reduce(ga8, la8, axis=AXL.X, op=ALU.add)
                nc.vector.reciprocal(larec, ga8)
                for c in range(16):
                    ptb = ps.tile([P, QB], F32, tag="sm")
                    nc.tensor.transpose(ptb[:, 0:8], egp8[0:8, c * P : (c + 1) * P],
                                        ident[0:8, 0:8])
                    nc.scalar.activation(pgT[:, c, :], ptb[:, 0:8], AF.Copy)

            def global_pv():
                for g in range(2):
                    pog = ps.tile([P, 512], F32, tag="mm")
                    for c in range(16):
                        nc.tensor.matmul(pog[0:8, 0 : 4 * HD], pgT[:, c, :],
                                         Vsbx[:, c, 4 * g : 4 * g + 4, 0:HD],
                                         start=(c == 0), stop=(c == 15))
                    pog_sb = band_pool.tile([8, 4 * HD], F32, tag="pog_sb")
                    nc.scalar.activation(pog_sb, pog[0:8, 0 : 4 * HD], AF.Copy,
                                         scale=larec[0:8, 0:1])
                    for j in range(2):
                        ptj = ps.tile([P, QB], F32, tag="sm")
                        nc.tensor.transpose(ptj[:, 0:8],
                                            pog_sb[0:8, j * P : (j + 1) * P],
                                            ident[0:8, 0:8])
                        for hh in (2 * j, 2 * j + 1):
                            h = 4 * g + hh
                            rlo = (hh % 2) * HD
                            nc.scalar.activation(
                                oT[h // 2][rlo : rlo + HD, SQ : SQ + 1].bitcast(F32R),
                                ptj[rlo : rlo + HD, h : h + 1], AF.Copy)

            for i in range(NQB):
                def po_copies(h, po):
                    pr, sub = h // 2, (h % 2) * HD
                    if h % 2 == 0:
                        nc.vector.tensor_copy(
                            oT[pr][sub : sub + HD,
                                   i * QB : (i + 1) * QB].bitcast(F32R),
                            po[0:HD, :])
                    else:
                        nc.scalar.activation(
                            oT[pr][sub : sub + HD,
                                   i * QB : (i + 1) * QB].bitcast(F32R),
                            po[0:HD, :], AF.Copy)
                    ml = 32 * (h % 4)
                    ltX = ltA if h < 4 else ltB
                    nc.scalar.activation(ltX[ml : ml + 1, i, :],
                                         po[HD : HD + 1, :], AF.Copy)

                pending = None
                for h in range(H):
                    pr, sub = h // 2, (h % 2) * HD
                    q_ap = QT[pr][sub : sub + HD, i * QB : (i + 1) * QB]
                    sc = ps.tile([P, NKC, QB], F32, tag="sc", bufs=2)
                    for c in range(NKC):
                        nc.tensor.matmul(
                            sc[:, c, :],
                            KTx[pr][sub : sub + HD,
                                    QB * i + c * P : QB * i + c * P + P],
                            q_ap, start=True, stop=True)
                    pT = band_pool.tile([P, NKC, QB], BF16, tag="pT")
                    nc.scalar.activation(pT, sc, AF.Exp)
                    nc.vector.tensor_tensor(pT, pT, mb_sb[:, i, :, :], ALU.mult)
                    po = ps.tile([P, QB], F32, tag="sm")
                    for c in range(NKC):
                        nc.tensor.matmul(po[0 : HD + 1, :], Vsbx[:, 2 * i + c, h, :],
                                         pT[:, c, :], start=(c == 0), stop=False)
                    m = 32 * (h % 3)
                    pgX = pgh3[h // 3]
                    nc.tensor.matmul(po[0 : HD + 1, :], Vg[m : m + 1, h, :],
                                     pgX[m : m + 1, i * QB : (i + 1) * QB],
                                     start=False, stop=True)
                    # one-head software pipeline: copies for head h-1 sit
                    # behind head h's EXP in the scalar stream, so the
                    # in-order scalar engine never stalls on the PV matmuls
                    if pending is not None:
                        po_copies(*pending)
                    pending = (h, po)
                po_copies(*pending)

                # normalize block i: partition-parallel reciprocals over the
                # quarter-row l tiles (DVE allows base 96); matmul operands
                # cannot sit at base 96, so heads 3/7 get moved to lq first
                with nc.allow_low_precision(reason="bf16 softmax sums"):
                    nc.vector.reciprocal(lrA[:, i, :], ltA[:, i, :])
                    nc.vector.reciprocal(lrB[:, i, :], ltB[:, i, :])
                nc.vector.tensor_copy(lq[0:1, i, :], lrA[96:97, i, :])
                nc.vector.tensor_copy(lq[32:33, i, :], lrB[96:97, i, :])

                def lsrc(h):
                    if h % 4 == 3:
                        return lq, 32 * (h // 4)
                    return (lrA if h < 4 else lrB), 32 * (h % 4)

                for p in range(NPAIR):
                    for j in range(2):
                        lt_t, r = lsrc(2 * p + j)
                        lbc = ps.tile([P, QB], F32, tag="sm")
                        nc.tensor.matmul(lbc[0:HD, :],
                                         onesPb[r : r + 1, :],
                                         lt_t[r : r + 1, i, :],
                                         start=True, stop=True)
                        rows = oT[p][j * HD : (j + 1) * HD,
                                     i * QB : (i + 1) * QB]
                        nc.vector.tensor_tensor(rows.bitcast(F32R), rows,
                                                lbc[0:HD, :], ALU.mult)

                # pipeline: out_proj for the two finished token tiles
                out_proj(2 * i)
                out_proj(2 * i + 1)

                # interleave the global-token path under the band blocks
                if i == 0:
                    for tcb in range(4):
                        global_scores(0, tcb)
                        global_scores(1, tcb)
                elif i == 1:
                    for tcb in range(4):
                        global_scores(2, tcb)
                        global_scores(3, tcb)
                    global_gather()
                elif i == 2:
                    global_transposes()
                else:
                    global_pv()

            out_proj(8)
            # LN2 runs as its own pass: keeps the scalar engine's activation
            # table stable (no EXP<->SQRT thrash inside the band loop)
            for t in range(9):
                ln2(t)

        # band scratch released; fetch FFN2 weights under out_proj/FFN1
        ffw = ctx.enter_context(tc.tile_pool(name="ffw", bufs=1))
        w2_sb = ffw.tile([P, FFC, D], BF16)
        nc.sync.dma_start(w2_sb, w2_pc[:])

        # ====== Phase G: FFN + residual -> y ======
        with tc.tile_pool(name="ffn", bufs=1) as ffn_pool, \
             tc.tile_pool(name="ffo", bufs=3) as out_pool:
            for t0, tw in [(0, 512), (512, 512), (SQ, 1)]:
                hT = ffn_pool.tile([P, FFC, 512], BF16, tag="hT")
                s0, w0, keep = (t0, tw, 0) if tw > 1 else (t0 - 1, 2, 1)
                for f in range(FFC):
                    ph = ps.tile([P, 512], F32, tag="mm")
                    for d in range(DC):
                        nc.tensor.matmul(ph[:, :w0],
                                         w1_sb[:, d, f * P : (f + 1) * P],
                                         z2T[:, d, s0 : s0 + w0],
                                         start=(d == 0), stop=(d == DC - 1))
                    nc.scalar.activation(hT[:, f, :tw],
                                         ph[:, keep : keep + tw], AF.Gelu,
                                         bias=b1_sb[:, f : f + 1])
                nsub = 4 if tw == 512 else 1
                for stp in range(nsub):
                    sw = P if tw == 512 else 1
                    ps2 = ps.tile([P, D], F32, tag="mm")
                    for f in range(FFC):
                        nc.tensor.matmul(ps2[:sw],
                                         hT[:, f, stp * P : stp * P + sw],
                                         w2_sb[:, f, :],
                                         start=(f == 0), stop=False)
                    nc.tensor.matmul(ps2[:sw], _r(ones_row[:1, :sw]), _r(b2_sb),
                                     start=False, stop=True)
                    yt = out_pool.tile([P, D], F32, tag="yt")
                    tglob = t0 // P + stp
                    nc.vector.tensor_tensor(yt[:sw], ps2[:sw],
                                            x1[:sw, tglob, :], ALU.add)
                    nc.gpsimd.dma_start(
                        y_out[t0 + stp * P : t0 + stp * P + sw, :], yt[:sw])

    nc.finalize()
    return nc


def make_host_inputs(x, padding_mask, attn_mask, in_proj_w, in_proj_b, out_proj_w,
                     out_proj_b, ln1_g, ln1_b, ln2_g, ln2_b, ff_w1, ff_b1, ff_w2,
                     ff_b2):
    """Build the 8 per-core input maps (numpy only)."""
    import ml_dtypes
    f32 = np.float32
    bf16 = ml_dtypes.bfloat16
    x = np.asarray(x, f32)
    attn_mask = np.asarray(attn_mask, f32)
    padding_mask = np.asarray(padding_mask, bool)

    g1 = np.asarray(ln1_g, f32); b1 = np.asarray(ln1_b, f32)
    g2 = np.asarray(ln2_g, f32); b2 = np.asarray(ln2_b, f32)
    Wq, Wk, Wv = (np.asarray(in_proj_w[i * D:(i + 1) * D], f32) for i in range(3))
    bq0, bk0, bv0 = (np.asarray(in_proj_b[i * D:(i + 1) * D], f32) for i in range(3))
    sc = 1.0 / np.sqrt(HD)

    Wq_ = Wq * g1[None, :] * sc
    bq_ = (Wq @ b1 + bq0) * sc
    Wk_ = Wk * g1[None, :]
    bk_ = Wk @ b1 + bk0
    Wv_ = Wv * g1[None, :]
    bv_ = Wv @ b1 + bv0
    W1_ = np.asarray(ff_w1, f32) * g2[None, :]
    b1f = np.asarray(ff_w1, f32) @ b2 + np.asarray(ff_b1, f32)

    def pc(wt, nchunk):  # [Dout, Din] -> [P, nchunk, Dout] chunked on Din
        return np.ascontiguousarray(
            wt.T.reshape(nchunk, P, wt.shape[0]).transpose(1, 0, 2))

    shared = {
        "wq_pc": pc(Wq_, DC).astype(bf16), "wk_pc": pc(Wk_, DC).astype(bf16),
        "wv_pc": pc(Wv_, DC).astype(bf16),
        "bq_pc": np.ascontiguousarray(bq_.reshape(DC, P).T),
        "bk_pc": np.ascontiguousarray(bk_.reshape(DC, P).T),
        "bv_row": bv_[None, :].copy(),
        "wo_pc": pc(np.asarray(out_proj_w, f32), DC),
        "bo_row": np.asarray(out_proj_b, f32)[None, :].copy(),
        "w1_pc": pc(W1_, DC).astype(bf16),
        "b1_pc": np.ascontiguousarray(b1f.reshape(FFC, P).T),
        "w2_pc": pc(np.asarray(ff_w2, f32), FFC).astype(bf16),
        "b2_row": np.asarray(ff_b2, f32)[None, :].copy(),
        "ones_in": np.ones((1, P), f32),
        "onesp_in": np.ones((P, P), f32),
    }

    in_maps = []
    for core in range(8):
        b = core // 2
        h = core % 2
        rot = np.roll(x[b], -1024 * h, axis=0)
        x_nat = np.ascontiguousarray(np.concatenate([rot, x[b, 0:1]], axis=0))

        # additive mask for this batch -> multiplicative factor
        A = attn_mask + np.where(padding_mask[b], -np.inf, 0.0)[None, :]
        mfac = np.exp(np.minimum(A, 0.0)).astype(f32)  # exp(-inf)=0, exp(0)=1
        mfac[~np.isfinite(A)] = 0.0

        # band masks: [P(t), NQB(i), NKC(c), QB(r)];
        # chunk c of block i covers rotated keys [256i - 64 + 128c, +128)
        i_idx = np.arange(NQB)[:, None, None, None]
        c_idx = np.arange(NKC)[None, :, None, None]
        t_idx = np.arange(P)[None, None, :, None]
        r_idx = np.arange(QB)[None, None, None, :]
        k_rot = (QB * i_idx - KPF + P * c_idx + t_idx) % S
        q_rot = i_idx * QB + r_idx
        gq = (q_rot + 1024 * h) % S
        gk = (k_rot + 1024 * h) % S
        band = mfac[gq, gk]                       # [NQB, NKC, P, QB]
        mask_band = np.ascontiguousarray(band.transpose(2, 0, 1, 3)).astype(bf16)

        # global-key column mask, zeroed when key0 falls inside the window
        key0_rot = (0 - 1024 * h) % S
        gq2 = (np.arange(NQB)[:, None] * QB + np.arange(QB)[None, :] + 1024 * h) % S
        gcol = mfac[gq2, 0].copy()
        for i in range(NQB):
            off = (key0_rot - (QB * i - KPF)) % S
            if off < NKC * P:
                gcol[i, :] = 0.0  # key 0 already inside this block's band window
        mask_gcol1 = np.ascontiguousarray(gcol.reshape(1, -1)).astype(bf16)

        # global-query additive mask row, in SHIFTED key order:
        # score col kappa <-> rotated key (kappa - 64) mod S
        kap = np.arange(S)
        k_act = (((kap - KPF) % S) + 1024 * h) % S
        mask_grow = np.ascontiguousarray(
            np.maximum(A[0, k_act], NEG)[None, :].astype(f32))

        m = dict(shared)
        m.update({
            "x_nat": x_nat,
            "mask_band": mask_band,
            "mask_gcol1": mask_gcol1,
            "mask_grow": mask_grow,
        })
        in_maps.append(m)
    return in_maps


def assemble_output(results):
    """results: list of 8 dicts with 'y' [NT, D] -> full [B, S, D]."""
    out = np.empty((B, S, D), np.float32)
    for b in range(B):
        y0 = results[2 * b]["y"]
        y1 = results[2 * b + 1]["y"]
        out[b, 0] = y0[SQ]
        out[b, 1:SQ] = y0[1:SQ]
        out[b, SQ:] = y1[0:SQ]
    return out


_CACHED_NC = None


def kernel(**inputs) -> np.ndarray:
    global _CACHED_NC
    from concourse.bass_utils import run_bass_kernel_spmd

    in_maps = make_host_inputs(**inputs)
    if _CACHED_NC is None:
        _CACHED_NC = build_module()
    res = run_bass_kernel_spmd(_CACHED_NC, in_maps, core_ids=list(range(8)))
    return assemble_output(res.results)


if __name__ == "__main__":
    nc = build_module()
    print("build + compile OK")


# revision 42
# speedup vs baseline: 1.3860x; 1.0301x over previous
"""LocalGlobalTransformerEncoderBlock on 8 Trainium2 NeuronCores.

Sharding: core = (batch b = core//2, sequence half h = core%2). Each core
computes the full encoder block for 1024 query rows of one batch plus the
global token (sequence position 0). The per-core sequence is ROTATED by
1024*h so the core's query rows are always rotated rows [0, 1024), and
x[b, 0] (the global token) is appended as row 2048.

v2 redesign vs the 481us baseline:
  - Band attention uses 3 unaligned 128-key chunks per 256-query block
    (window [256i-64, 256i+320) mod 2048) via a 64-col wrapped prefix on
    the transposed K / z buffers and half-shifted V key tiles.
  - Attention tensors (Q^T, K^T, V, probs, masks) and FFN weights/hidden
    are bf16: same PE rate as fp32r but half the SBUF/DMA and 2x DVE.
  - Softmax denominators are gathered into a [32, 256] tile so ONE
    partition-parallel reciprocal replaces 16 serial [64,512] ones.
  - The global-token path is per-head wide matmuls + one fused softmax,
    emitted interleaved with the band blocks so it hides under them.
  - Normalize/out_proj/LN2 are emitted per band block (software pipeline);
    FFN weights prefetch during attention; x is loaded once into the
    residual buffer.

Self-contained: only imports from /opt/trn_rl_repo (the installed bass
runtime), numpy, and stdlib.
"""

import sys
from contextlib import ExitStack

if "/opt/trn_rl_repo" not in sys.path:
    sys.path.insert(0, "/opt/trn_rl_repo")

import numpy as np

import concourse.bass as bass
import concourse.bacc as bacc_mod
import concourse.mybir as mybir
import concourse.tile as tile
from concourse.masks import make_identity

P = 128
B, S, D, H, FF = 4, 2048, 512, 8, 2048
HD = D // H            # 64
DC = D // P            # 4 chunks of the model dim
FFC = FF // P          # 16 chunks of the FF dim
SK = S + 1             # 2049 tokens (2048 rotated + appended global token)
SQ = 1024              # band queries per core
NT = SQ + 1            # 1025 output tokens
QB = 256               # band query block
NQB = SQ // QB         # 4
NKC = 3                # 128-key chunks per band window
KPF = 64               # wrapped key/token prefix columns
NPAIR = H // 2         # 4 head-pair tiles
EPS = 1e-5
NEG = -1e30

F32 = mybir.dt.float32
F32R = mybir.dt.float32r
BF16 = mybir.dt.bfloat16
AF = mybir.ActivationFunctionType
ALU = mybir.AluOpType
AXL = mybir.AxisListType


def _r(ap):
    """Reinterpret an fp32 AP as float32r for full-rate PE matmuls."""
    return ap.bitcast(F32R)


def build_module():
    nc = bacc_mod.Bacc("TRN2", target_bir_lowering=False)

    x_nat = nc.dram_tensor("x_nat", [SK, D], F32, kind="ExternalInput")
    wq_pc = nc.dram_tensor("wq_pc", [P, DC, D], BF16, kind="ExternalInput")
    wk_pc = nc.dram_tensor("wk_pc", [P, DC, D], BF16, kind="ExternalInput")
    wv_pc = nc.dram_tensor("wv_pc", [P, DC, D], BF16, kind="ExternalInput")
    bq_pc = nc.dram_tensor("bq_pc", [P, DC], F32, kind="ExternalInput")
    bk_pc = nc.dram_tensor("bk_pc", [P, DC], F32, kind="ExternalInput")
    bv_row = nc.dram_tensor("bv_row", [1, D], F32R, kind="ExternalInput")
    wo_pc = nc.dram_tensor("wo_pc", [P, DC, D], F32R, kind="ExternalInput")
    bo_row = nc.dram_tensor("bo_row", [1, D], F32R, kind="ExternalInput")
    w1_pc = nc.dram_tensor("w1_pc", [P, DC, FF], BF16, kind="ExternalInput")
    b1_pc = nc.dram_tensor("b1_pc", [P, FFC], F32, kind="ExternalInput")
    w2_pc = nc.dram_tensor("w2_pc", [P, FFC, D], BF16, kind="ExternalInput")
    b2_row = nc.dram_tensor("b2_row", [1, D], F32R, kind="ExternalInput")
    mask_band = nc.dram_tensor("mask_band", [P, NQB, NKC, QB], BF16, kind="ExternalInput")
    mask_gcol1 = nc.dram_tensor("mask_gcol1", [1, SQ], BF16, kind="ExternalInput")
    mask_grow = nc.dram_tensor("mask_grow", [1, S], F32R, kind="ExternalInput")
    ones_in = nc.dram_tensor("ones_in", [1, P], F32R, kind="ExternalInput")
    onesp_in = nc.dram_tensor("onesp_in", [P, P], F32R, kind="ExternalInput")
    y_out = nc.dram_tensor("y", [NT, D], F32, kind="ExternalOutput")

    with tile.TileContext(nc) as tc, ExitStack() as ctx:
        # ---- persistent state ----
        persist = ctx.enter_context(tc.tile_pool(name="persist", bufs=1))
        x1 = persist.tile([P, 9, D], F32)
        nc.gpsimd.dma_start(x1[:, 0, :], x_nat[0:P, :])
        nc.gpsimd.dma_start(
            x1[:, 1:4, :], x_nat[P:512, :].rearrange("(t p) d -> p t d", t=3))
        nc.gpsimd.dma_start(
            x1[:, 4:8, :], x_nat[512:1024, :].rearrange("(t p) d -> p t d", t=4))
        nc.gpsimd.dma_start(x1[0:1, 8, :], x_nat[S : S + 1, :])
        ident = persist.tile([P, P], F32)
        make_identity(nc, ident)
        ones_row = persist.tile([1, P], F32R)
        nc.sync.dma_start(ones_row, ones_in[:])
        eps_t = persist.tile([P, 1], F32)
        nc.vector.memset(eps_t, EPS)
        bo_sb = persist.tile([1, D], F32R)
        nc.sync.dma_start(bo_sb, bo_row[:])
        b1_sb = persist.tile([P, FFC], F32)
        nc.sync.dma_start(b1_sb, b1_pc[:])
        b2_sb = persist.tile([1, D], F32R)
        nc.sync.dma_start(b2_sb, b2_row[:])

        oT = [persist.tile([P, NT], F32, name=f"oT{p}") for p in range(NPAIR)]
        z2T = persist.tile([P, DC, NT], BF16)
        pgT = persist.tile([P, 16, 8], BF16)
        onesP = persist.tile([P, P], F32R)
        nc.sync.dma_start(onesP, onesp_in[:])
        onesPb = persist.tile([P, HD], BF16)
        nc.gpsimd.memset(onesPb, 1.0)

        # ---- PSUM pool: mm(2) + sm(2) + sc(2) + smb(2) = 8 banks ----
        ps = ctx.enter_context(tc.tile_pool(name="ps", bufs=2, space="PSUM"))

        # ---- attention state (lives through band phase) ----
        attn = ctx.enter_context(tc.tile_pool(name="attn", bufs=1))
        QT = [attn.tile([P, NT], BF16, name=f"QT{p}") for p in range(NPAIR)]
        KTx = [attn.tile([P, KPF + SK], BF16, name=f"KTx{p}") for p in range(NPAIR)]
        Vsbx = attn.tile([P, 17, H, HD + 1], BF16)
        nc.gpsimd.memset(Vsbx[:, :, :, HD], 1.0)
        # per-head global-key V rows / probs at quarter-partition bases
        Vg = attn.tile([P, H, HD + 1], BF16)
        pgh3 = [attn.tile([P, SQ], BF16, name=f"pgh3_{t}") for t in range(3)]
        k0h = [attn.tile([P, 1], BF16, name=f"k0h{h}") for h in range(H)]
        mb_sb = attn.tile([P, NQB, NKC, QB], BF16)
        mgc_sb = attn.tile([P, SQ], BF16)
        mgrow_sb = attn.tile([1, S], F32R)

        wo_sb = attn.tile([P, DC, D], F32R)
        w1_sb = attn.tile([P, DC, FF], BF16)

        # x loads on the gpsimd queue; weights/masks on the sync queue
        nc.sync.dma_start(mb_sb, mask_band[:])
        nc.sync.dma_start(mgc_sb[0:1, :], mask_gcol1[:])
        nc.sync.dma_start(mgrow_sb, mask_grow[:])
        for mrow in (32, 64):
            nc.vector.tensor_copy(mgc_sb[mrow : mrow + 1, :], mgc_sb[0:1, :])

        # ====== Phase A+B: LN1 -> zTx, V (interleaved), Q/K ======
        with tc.tile_pool(name="phA", bufs=1) as pha, \
             tc.tile_pool(name="lnz", bufs=2) as z_pool, \
             tc.tile_pool(name="st1", bufs=4) as stat_pool:
            zTx = pha.tile([P, DC, KPF + SK], BF16)
            wv_sb = pha.tile([P, DC, D], BF16)
            wq_sb = pha.tile([P, DC, D], BF16)
            wk_sb = pha.tile([P, DC, D], BF16)
            bq_sb = pha.tile([P, DC], F32)
            bk_sb = pha.tile([P, DC], F32)
            bv_sb = pha.tile([1, D], F32R)

            nc.sync.dma_start(wv_sb, wv_pc[:])
            nc.sync.dma_start(bv_sb, bv_row[:])
            nc.sync.dma_start(wq_sb, wq_pc[:])
            nc.sync.dma_start(bq_sb, bq_pc[:])
            nc.sync.dma_start(wk_sb, wk_pc[:])
            nc.sync.dma_start(bk_sb, bk_pc[:])
            nc.sync.dma_start(wo_sb, wo_pc[:])
            nc.sync.dma_start(w1_sb, w1_pc[:])

            def vproj_mm(j):
                """V projection for shifted key tile j (tokens [128j-64,128j+64))."""
                rows = P if j < 16 else 1
                col0 = 128 * j if j < 16 else KPF + S
                pv = ps.tile([P, D], F32, tag="mm")
                for d in range(DC):
                    nc.tensor.matmul(pv[:rows], zTx[:, d, col0 : col0 + rows],
                                     wv_sb[:, d, :],
                                     start=(d == 0), stop=False)
                nc.tensor.matmul(pv[:rows], _r(ones_row[:1, :rows]), _r(bv_sb),
                                 start=False, stop=True)
                return pv, rows

            def vproj_copy(j, pv, rows):
                nc.scalar.activation(
                    Vsbx[:rows, j, :, 0:HD],
                    pv[:rows].rearrange("p (h e) -> p h e", h=H), AF.Copy)

            def ln1(t):
                rows = P if t < 16 else 1
                if t < 8:
                    src = x1[:rows, t, :]
                elif t < 16:
                    xt = z_pool.tile([P, D], F32, tag="xt")
                    nc.gpsimd.dma_start(xt, x_nat[t * P : (t + 1) * P, :])
                    src = xt[:rows]
                else:
                    src = x1[0:1, 8, :]
                st = stat_pool.tile([P, 6], F32, tag="bnst")
                nc.vector.bn_stats(st[:rows], src)
                mv = stat_pool.tile([P, 2], F32, tag="bnmv")
                nc.vector.bn_aggr(mv[:rows], st[:rows])
                rstd = stat_pool.tile([P, 1], F32, tag="rstd")
                nc.scalar.activation(rstd[:rows], mv[:rows, 1:2], AF.Sqrt,
                                     bias=eps_t[:rows], scale=1.0)
                nc.vector.reciprocal(rstd[:rows], rstd[:rows])
                z = z_pool.tile([P, D], F32, tag="z")
                nc.vector.tensor_scalar(z[:rows], src, mv[:rows, 0:1],
                                        rstd[:rows],
                                        op0=ALU.subtract, op1=ALU.mult)
                ptt = ps.tile([P, DC, P], F32, tag="mm")
                for d in range(DC):
                    nc.tensor.transpose(ptt[:, d, :rows],
                                        z[:rows, d * P : (d + 1) * P],
                                        ident[:rows, :rows])
                return ptt

            def zcopy(t, ptt):
                rows = P if t < 16 else 1
                nc.scalar.activation(
                    zTx[:, :, KPF + t * P : KPF + t * P + rows],
                    ptt[:, :, :rows], AF.Copy)
                if t == 15:
                    # wrapped prefix: tokens 1984..2048 = local rows 64..128
                    nc.vector.tensor_copy(zTx[:, :, 0:KPF], ptt[:, :, HD:P])

            # one-tile software pipeline: tile t's PSUM->SBUF copies run
            # behind tile t+1's LN in the scalar stream; V matmuls for key
            # tile j follow zcopy(j), their copy one step later again
            zpend = {}
            vpend = {}
            for t in range(19):
                if t <= 16:
                    zpend[t] = ln1(t)
                if 1 <= t <= 17:
                    zcopy(t - 1, zpend.pop(t - 1))
                vjs = []
                if 2 <= t <= 16:
                    vjs.append(t - 1)
                elif t == 17:
                    vjs += [0, 16]
                for j in vjs:
                    vpend[j] = vproj_mm(j)
                for j in list(vpend):
                    if (t >= 3 and j <= t - 2) or t >= 18:
                        vproj_copy(j, *vpend.pop(j))

            # ---- Q/K projections ----
            q_blocks = [(KPF, 0, 512), (KPF + 512, 512, 512), (KPF + S, SQ, 1)]
            k_blocks = [(KPF + i * 512, KPF + i * 512, 512) for i in range(4)] \
                + [(KPF + S, KPF + S, 1)]
            for p in range(NPAIR):
                for w_sb, b_sb, dst, blocks in (
                        (wq_sb, bq_sb, QT[p], q_blocks),
                        (wk_sb, bk_sb, KTx[p], k_blocks)):
                    for bi, (src, dcol, w) in enumerate(blocks):
                        s0, w0, keep = (src, w, 0) if w > 1 else (src - 1, 2, 1)
                        pq = ps.tile([P, 512], F32, tag="mm")
                        for d in range(DC):
                            nc.tensor.matmul(pq[:, :w0],
                                             w_sb[:, d, p * P : (p + 1) * P],
                                             zTx[:, d, s0 : s0 + w0],
                                             start=(d == 0), stop=(d == DC - 1))
                        if bi % 2 == 0:
                            nc.scalar.activation(dst[:, dcol : dcol + w],
                                                 pq[:, keep : keep + w], AF.Identity,
                                                 bias=b_sb[:, p : p + 1])
                        else:
                            nc.vector.tensor_scalar(dst[:, dcol : dcol + w],
                                                    pq[:, keep : keep + w],
                                                    b_sb[:, p : p + 1], None,
                                                    op0=ALU.add)
                nc.vector.tensor_copy(KTx[p][:, 0:KPF], KTx[p][:, S : S + KPF])

        # ====== Phase C/D/E/F interleaved: band + global + norm + out_proj + LN2
        with tc.tile_pool(name="bandp", bufs=3) as band_pool, \
             tc.tile_pool(name="ln2z", bufs=2) as z2_pool, \
             tc.tile_pool(name="st2", bufs=4) as stat2_pool:

            # band-phase scratch: quarter-partition l tiles, global-path
            # per-pair scratch, zero-padded stationaries
            ltA = band_pool.tile([P, NQB, QB], BF16, bufs=1)
            ltB = band_pool.tile([P, NQB, QB], BF16, bufs=1)
            lrA = band_pool.tile([P, NQB, QB], BF16, bufs=1)
            lrB = band_pool.tile([P, NQB, QB], BF16, bufs=1)
            lq = band_pool.tile([P, NQB, QB], BF16, bufs=1)
            egpp = [band_pool.tile([2, S], F32, tag=f"egpp{p}", bufs=1,
                                   name=f"egpp{p}") for p in range(NPAIR)]
            laP = [band_pool.tile([2, 4], F32, tag=f"laP{p}", bufs=1,
                                  name=f"laP{p}") for p in range(NPAIR)]
            egp8 = band_pool.tile([8, S], F32, bufs=1)
            la8 = band_pool.tile([8, 4], F32, bufs=1)
            ga8 = band_pool.tile([8, 1], F32, bufs=1)
            larec = band_pool.tile([8, 1], F32, bufs=1)

            # col j of q2g[p] holds head (2p+j)'s global-query column in its
            # own 64 contraction rows; k0h[h] is the zero-padded global-key
            # column; Vg packs each head's global-key V row (+ ones col) at
            # quarter-partition bases so band PV matmuls stay legal
            q2g = [band_pool.tile([P, 2], BF16, tag=f"q2g{p}", bufs=1,
                                  name=f"q2g{p}") for p in range(NPAIR)]
            for pr in range(NPAIR):
                nc.gpsimd.memset(q2g[pr], 0.0)
                for j in range(2):
                    sub = j * HD
                    h = 2 * pr + j
                    nc.gpsimd.tensor_copy(
                        q2g[pr][sub : sub + HD, j : j + 1],
                        QT[pr][sub : sub + HD, SQ : SQ + 1])
                    nc.gpsimd.memset(k0h[h], 0.0)
                    nc.gpsimd.tensor_copy(
                        k0h[h][sub : sub + HD, 0:1],
                        KTx[pr][sub : sub + HD, KPF + S : KPF + S + 1])
                    m = 32 * (h % 3)
                    nc.gpsimd.tensor_copy(Vg[m : m + 1, h, :],
                                          Vsbx[0:1, 16, h, :])

            # global-KEY column scores for all band queries (all 8 heads)
            for h in range(H):
                pr = h // 2
                m = 32 * (h % 3)
                pgX = pgh3[h // 3]
                for half in range(2):
                    gq = ps.tile([P, 512], F32, tag="mm")
                    nc.tensor.matmul(gq[0:1, :], k0h[h],
                                     QT[pr][:, half * 512 : (half + 1) * 512],
                                     start=True, stop=True)
                    nc.scalar.activation(
                        pgX[m : m + 1, half * 512 : (half + 1) * 512],
                        gq[0:1, :], AF.Exp)
                nc.vector.tensor_tensor(pgX[m : m + 1, :], pgX[m : m + 1, :],
                                        mgc_sb[m : m + 1, :], ALU.mult)

            def ln2(t):
                rows = P if t < 8 else 1
                src = x1[:rows, t, :]
                st = stat2_pool.tile([P, 6], F32, tag="bnst2")
                nc.vector.bn_stats(st[:rows], src)
                mv = stat2_pool.tile([P, 2], F32, tag="bnmv2")
                nc.vector.bn_aggr(mv[:rows], st[:rows])
                rstd = stat2_pool.tile([P, 1], F32, tag="rstd2")
                nc.scalar.activation(rstd[:rows], mv[:rows, 1:2], AF.Sqrt,
                                     bias=eps_t[:rows], scale=1.0)
                nc.vector.reciprocal(rstd[:rows], rstd[:rows])
                z2 = z2_pool.tile([P, D], F32, tag="z2")
                nc.vector.tensor_scalar(z2[:rows], src, mv[:rows, 0:1],
                                        rstd[:rows],
                                        op0=ALU.subtract, op1=ALU.mult)
                ptt = ps.tile([P, DC, P], F32, tag="mm")
                for d in range(DC):
                    nc.tensor.transpose(ptt[:, d, :rows],
                                        z2[:rows, d * P : (d + 1) * P],
                                        ident[:rows, :rows])
                nc.scalar.activation(z2T[:, :, t * P : t * P + rows],
                                     ptt[:, :, :rows], AF.Copy)

            def out_proj(t):
                w = P if t < 8 else 1
                yp = ps.tile([P, D], F32, tag="mm")
                if w > 1:
                    for p in range(NPAIR):
                        nc.tensor.matmul(yp[:w], _r(oT[p][:, t * P : t * P + w]),
                                         _r(wo_sb[:, p, :]),
                                         start=(p == 0), stop=False)
                else:
                    for p in range(NPAIR):
                        nc.tensor.matmul(yp[:w], _r(oT[p][:, SQ : SQ + 1]),
                                         _r(wo_sb[:, p, :]),
                                         start=(p == 0), stop=False)
                nc.tensor.matmul(yp[:w], _r(ones_row[:1, :w]), _r(bo_sb),
                                 start=False, stop=True)
                nc.vector.tensor_tensor(x1[:w, t, :], yp[:w], x1[:w, t, :], ALU.add)

            def global_scores(pr, tcb):
                gs = ps.tile([P, 512], F32, tag="mm")
                nc.tensor.matmul(gs[0:2, :], q2g[pr],
                                 KTx[pr][:, tcb * 512 : (tcb + 1) * 512],
                                 start=True, stop=False)
                # additive key mask is head-independent -> rank-1 matmul add
                nc.tensor.matmul(gs[0:2, :], _r(onesP[0:1, 0:2]),
                                 mgrow_sb[0:1, tcb * 512 : (tcb + 1) * 512],
                                 start=False, stop=True)
                nc.scalar.activation(
                    egpp[pr][:, tcb * 512 : (tcb + 1) * 512],
                    gs[0:2, :], AF.Exp,
                    accum_out=laP[pr][:, tcb : tcb + 1])

            def global_gather():
                for pr in range(NPAIR):
                    nc.gpsimd.dma_start(egp8[2 * pr : 2 * pr + 2, :], egpp[pr][:])
                    nc.gpsimd.dma_start(la8[2 * pr : 2 * pr + 2, :], laP[pr][:])

            def global_transposes():
                nc.vector.tensor_reduce(ga8, la8, axis=AXL.X, op=ALU.add)
                nc.vector.reciprocal(larec, ga8)
                for c in range(16):
                    ptb = ps.tile([P, QB], F32, tag="sm")
                    nc.tensor.transpose(ptb[:, 0:8], egp8[0:8, c * P : (c + 1) * P],
                                        ident[0:8, 0:8])
                    nc.scalar.activation(pgT[:, c, :], ptb[:, 0:8], AF.Copy)

            def global_pv():
                for g in range(2):
                    pog = ps.tile([P, 512], F32, tag="mm")
                    for c in range(16):
                        nc.tensor.matmul(pog[0:8, 0 : 4 * HD], pgT[:, c, :],
                                         Vsbx[:, c, 4 * g : 4 * g + 4, 0:HD],
                                         start=(c == 0), stop=(c == 15))
                    pog_sb = band_pool.tile([8, 4 * HD], F32, tag="pog_sb")
                    nc.scalar.activation(pog_sb, pog[0:8, 0 : 4 * HD], AF.Copy,
                                         scale=larec[0:8, 0:1])
                    for j in range(2):
                        ptj = ps.tile([P, QB], F32, tag="sm")
                        nc.tensor.transpose(ptj[:, 0:8],
                                            pog_sb[0:8, j * P : (j + 1) * P],
                                            ident[0:8, 0:8])
                        for hh in (2 * j, 2 * j + 1):
                            h = 4 * g + hh
                            rlo = (hh % 2) * HD
                            nc.scalar.activation(
                                oT[h // 2][rlo : rlo + HD, SQ : SQ + 1].bitcast(F32R),
                                ptj[rlo : rlo + HD, h : h + 1], AF.Copy)

            def lsrc(h):
                if h % 4 == 3:
                    return lq, 32 * (h // 4)
                return (lrA if h < 4 else lrB), 32 * (h % 4)

            for i in range(NQB):
                def po_copies(h, po):
                    pr, sub = h // 2, (h % 2) * HD
                    if h % 2 == 0:
                        nc.vector.tensor_copy(
                            oT[pr][sub : sub + HD,
                                   i * QB : (i + 1) * QB].bitcast(F32R),
                            po[0:HD, :])
                    else:
                        nc.scalar.activation(
                            oT[pr][sub : sub + HD,
                                   i * QB : (i + 1) * QB].bitcast(F32R),
                            po[0:HD, :], AF.Copy)
                    ml = 32 * (h % 4)
                    ltX = ltA if h < 4 else ltB
                    nc.scalar.activation(ltX[ml : ml + 1, i, :],
                                         po[HD : HD + 1, :], AF.Copy)

                def sc_mm(h):
                    pr, sub = h // 2, (h % 2) * HD
                    q_ap = QT[pr][sub : sub + HD, i * QB : (i + 1) * QB]
                    sc = ps.tile([P, NKC, QB], F32, tag="sc", bufs=2)
                    for c in range(NKC):
                        nc.tensor.matmul(
                            sc[:, c, :],
                            KTx[pr][sub : sub + HD,
                                    QB * i + c * P : QB * i + c * P + P],
                            q_ap, start=True, stop=True)
                    return sc

                def po_mm(h, pT):
                    po = ps.tile([P, QB], F32, tag="sm")
                    for c in range(NKC):
                        nc.tensor.matmul(po[0 : HD + 1, :],
                                         Vsbx[:, 2 * i + c, h, :],
                                         pT[:, c, :], start=(c == 0), stop=False)
                    m = 32 * (h % 3)
                    pgX = pgh3[h // 3]
                    nc.tensor.matmul(po[0 : HD + 1, :], Vg[m : m + 1, h, :],
                                     pgX[m : m + 1, i * QB : (i + 1) * QB],
                                     start=False, stop=True)
                    return po

                def norm_half(half):
                    lrX, ltX = (lrA, ltA) if half == 0 else (lrB, ltB)
                    with nc.allow_low_precision(reason="bf16 softmax sums"):
                        nc.vector.reciprocal(lrX[:, i, :], ltX[:, i, :])
                    nc.vector.tensor_copy(lq[32 * half : 32 * half + 1, i, :],
                                          lrX[96:97, i, :])
                    for p in (2 * half, 2 * half + 1):
                        for j in range(2):
                            lt_t, r = lsrc(2 * p + j)
                            lbc = ps.tile([P, QB], F32, tag="sm")
                            nc.tensor.matmul(lbc[0:HD, :], onesPb[r : r + 1, :],
                                             lt_t[r : r + 1, i, :],
                                             start=True, stop=True)
                            rows = oT[p][j * HD : (j + 1) * HD,
                                         i * QB : (i + 1) * QB]
                            nc.vector.tensor_tensor(rows.bitcast(F32R), rows,
                                                    lbc[0:HD, :], ALU.mult)

                # two-deep software pipeline: PE never waits on the softmax
                # (sc for head h+1 precedes po for head h), and the in-order
                # scalar stream sees copies one head late
                sc_t = {0: sc_mm(0)}
                po_t = {}
                for h in range(H):
                    pT = band_pool.tile([P, NKC, QB], BF16, tag="pT")
                    nc.scalar.activation(pT, sc_t.pop(h), AF.Exp)
                    nc.vector.tensor_tensor(pT, pT, mb_sb[:, i, :, :], ALU.mult)
                    if h + 1 < H:
                        sc_t[h + 1] = sc_mm(h + 1)
                    po_t[h] = po_mm(h, pT)
                    if h - 1 in po_t:
                        po_copies(h - 1, po_t.pop(h - 1))
                    if h == 5:
                        norm_half(0)
                po_copies(7, po_t.pop(7))
                norm_half(1)


                # pipeline: out_proj for the two finished token tiles
                out_proj(2 * i)
                out_proj(2 * i + 1)

                # interleave the global-token path under the band blocks
                if i == 0:
                    for tcb in range(4):
                        global_scores(0, tcb)
                        global_scores(1, tcb)
                elif i == 1:
                    for tcb in range(4):
                        global_scores(2, tcb)
                        global_scores(3, tcb)
                    global_gather()
                elif i == 2:
                    global_transposes()
                else:
                    global_pv()

            out_proj(8)
            # LN2 runs as its own pass: keeps the scalar engine's activation
            # table stable (no EXP<->SQRT thrash inside the band loop)
            for t in range(9):
                ln2(t)

        # band scratch released; fetch FFN2 weights under out_proj/FFN1
        ffw = ctx.enter_context(tc.tile_pool(name="ffw", bufs=1))
        w2_sb = ffw.tile([P, FFC, D], BF16)
        nc.sync.dma_start(w2_sb, w2_pc[:])

        # ====== Phase G: FFN + residual -> y ======
        with tc.tile_pool(name="ffn", bufs=1) as ffn_pool, \
             tc.tile_pool(name="ffo", bufs=3) as out_pool:
            for t0, tw in [(0, 512), (512, 512), (SQ, 1)]:
                hT = ffn_pool.tile([P, FFC, 512], BF16, tag="hT")
                s0, w0, keep = (t0, tw, 0) if tw > 1 else (t0 - 1, 2, 1)
                for f in range(FFC):
                    ph = ps.tile([P, 512], F32, tag="mm")
                    for d in range(DC):
                        nc.tensor.matmul(ph[:, :w0],
                                         w1_sb[:, d, f * P : (f + 1) * P],
                                         z2T[:, d, s0 : s0 + w0],
                                         start=(d == 0), stop=(d == DC - 1))
                    nc.scalar.activation(hT[:, f, :tw],
                                         ph[:, keep : keep + tw], AF.Gelu,
                                         bias=b1_sb[:, f : f + 1])
                nsub = 4 if tw == 512 else 1
                for stp in range(nsub):
                    sw = P if tw == 512 else 1
                    ps2 = ps.tile([P, D], F32, tag="mm")
                    for f in range(FFC):
                        nc.tensor.matmul(ps2[:sw],
                                         hT[:, f, stp * P : stp * P + sw],
                                         w2_sb[:, f, :],
                                         start=(f == 0), stop=False)
                    nc.tensor.matmul(ps2[:sw], _r(ones_row[:1, :sw]), _r(b2_sb),
                                     start=False, stop=True)
                    yt = out_pool.tile([P, D], F32, tag="yt")
                    tglob = t0 // P + stp
                    nc.vector.tensor_tensor(yt[:sw], ps2[:sw],
                                            x1[:sw, tglob, :], ALU.add)
                    nc.gpsimd.dma_start(
                        y_out[t0 + stp * P : t0 + stp * P + sw, :], yt[:sw])

    nc.finalize()
    return nc


def make_host_inputs(x, padding_mask, attn_mask, in_proj_w, in_proj_b, out_proj_w,
                     out_proj_b, ln1_g, ln1_b, ln2_g, ln2_b, ff_w1, ff_b1, ff_w2,
                     ff_b2):
    """Build the 8 per-core input maps (numpy only)."""
    import ml_dtypes
    f32 = np.float32
    bf16 = ml_dtypes.bfloat16
    x = np.asarray(x, f32)
    attn_mask = np.asarray(attn_mask, f32)
    padding_mask = np.asarray(padding_mask, bool)

    g1 = np.asarray(ln1_g, f32); b1 = np.asarray(ln1_b, f32)
    g2 = np.asarray(ln2_g, f32); b2 = np.asarray(ln2_b, f32)
    Wq, Wk, Wv = (np.asarray(in_proj_w[i * D:(i + 1) * D], f32) for i in range(3))
    bq0, bk0, bv0 = (np.asarray(in_proj_b[i * D:(i + 1) * D], f32) for i in range(3))
    sc = 1.0 / np.sqrt(HD)

    Wq_ = Wq * g1[None, :] * sc
    bq_ = (Wq @ b1 + bq0) * sc
    Wk_ = Wk * g1[None, :]
    bk_ = Wk @ b1 + bk0
    Wv_ = Wv * g1[None, :]
    bv_ = Wv @ b1 + bv0
    W1_ = np.asarray(ff_w1, f32) * g2[None, :]
    b1f = np.asarray(ff_w1, f32) @ b2 + np.asarray(ff_b1, f32)

    def pc(wt, nchunk):  # [Dout, Din] -> [P, nchunk, Dout] chunked on Din
        return np.ascontiguousarray(
            wt.T.reshape(nchunk, P, wt.shape[0]).transpose(1, 0, 2))

    shared = {
        "wq_pc": pc(Wq_, DC).astype(bf16), "wk_pc": pc(Wk_, DC).astype(bf16),
        "wv_pc": pc(Wv_, DC).astype(bf16),
        "bq_pc": np.ascontiguousarray(bq_.reshape(DC, P).T),
        "bk_pc": np.ascontiguousarray(bk_.reshape(DC, P).T),
        "bv_row": bv_[None, :].copy(),
        "wo_pc": pc(np.asarray(out_proj_w, f32), DC),
        "bo_row": np.asarray(out_proj_b, f32)[None, :].copy(),
        "w1_pc": pc(W1_, DC).astype(bf16),
        "b1_pc": np.ascontiguousarray(b1f.reshape(FFC, P).T),
        "w2_pc": pc(np.asarray(ff_w2, f32), FFC).astype(bf16),
        "b2_row": np.asarray(ff_b2, f32)[None, :].copy(),
        "ones_in": np.ones((1, P), f32),
        "onesp_in": np.ones((P, P), f32),
    }

    in_maps = []
    for core in range(8):
        b = core // 2
        h = core % 2
        rot = np.roll(x[b], -1024 * h, axis=0)
        x_nat = np.ascontiguousarray(np.concatenate([rot, x[b, 0:1]], axis=0))

        # additive mask for this batch -> multiplicative factor
        A = attn_mask + np.where(padding_mask[b], -np.inf, 0.0)[None, :]
        mfac = np.exp(np.minimum(A, 0.0)).astype(f32)  # exp(-inf)=0, exp(0)=1
        mfac[~np.isfinite(A)] = 0.0

        # band masks: [P(t), NQB(i), NKC(c), QB(r)];
        # chunk c of block i covers rotated keys [256i - 64 + 128c, +128)
        i_idx = np.arange(NQB)[:, None, None, None]
        c_idx = np.arange(NKC)[None, :, None, None]
        t_idx = np.arange(P)[None, None, :, None]
        r_idx = np.arange(QB)[None, None, None, :]
        k_rot = (QB * i_idx - KPF + P * c_idx + t_idx) % S
        q_rot = i_idx * QB + r_idx
        gq = (q_rot + 1024 * h) % S
        gk = (k_rot + 1024 * h) % S
        band = mfac[gq, gk]                       # [NQB, NKC, P, QB]
        mask_band = np.ascontiguousarray(band.transpose(2, 0, 1, 3)).astype(bf16)

        # global-key column mask, zeroed when key0 falls inside the window
        key0_rot = (0 - 1024 * h) % S
        gq2 = (np.arange(NQB)[:, None] * QB + np.arange(QB)[None, :] + 1024 * h) % S
        gcol = mfac[gq2, 0].copy()
        for i in range(NQB):
            off = (key0_rot - (QB * i - KPF)) % S
            if off < NKC * P:
                gcol[i, :] = 0.0  # key 0 already inside this block's band window
        mask_gcol1 = np.ascontiguousarray(gcol.reshape(1, -1)).astype(bf16)

        # global-query additive mask row, in SHIFTED key order:
        # score col kappa <-> rotated key (kappa - 64) mod S
        kap = np.arange(S)
        k_act = (((kap - KPF) % S) + 1024 * h) % S
        mask_grow = np.ascontiguousarray(
            np.maximum(A[0, k_act], NEG)[None, :].astype(f32))

        m = dict(shared)
        m.update({
            "x_nat": x_nat,
            "mask_band": mask_band,
            "mask_gcol1": mask_gcol1,
            "mask_grow": mask_grow,
        })
        in_maps.append(m)
    return in_maps


def assemble_output(results):
    """results: list of 8 dicts with 'y' [NT, D] -> full [B, S, D]."""
    out = np.empty((B, S, D), np.float32)
    for b in range(B):
        y0 = results[2 * b]["y"]
        y1 = results[2 * b + 1]["y"]
        out[b, 0] = y0[SQ]
        out[b, 1:SQ] = y0[1:SQ]
        out[b, SQ:] = y1[0:SQ]
    return out


_CACHED_NC = None


def kernel(**inputs) -> np.ndarray:
    global _CACHED_NC
    from concourse.bass_utils import run_bass_kernel_spmd

    in_maps = make_host_inputs(**inputs)
    if _CACHED_NC is None:
        _CACHED_NC = build_module()
    res = run_bass_kernel_spmd(_CACHED_NC, in_maps, core_ids=list(range(8)))
    return assemble_output(res.results)


if __name__ == "__main__":
    nc = build_module()
    print("build + compile OK")


# revision 47
# speedup vs baseline: 1.4761x; 1.0650x over previous
"""LocalGlobalTransformerEncoderBlock on 8 Trainium2 NeuronCores.

Sharding: core = (batch b = core//2, sequence half h = core%2). Each core
computes the full encoder block for 1024 query rows of one batch plus the
global token (sequence position 0). The per-core sequence is ROTATED by
1024*h so the core's query rows are always rotated rows [0, 1024), and
x[b, 0] (the global token) is appended as row 2048.

v2 redesign vs the 481us baseline:
  - Band attention uses 3 unaligned 128-key chunks per 256-query block
    (window [256i-64, 256i+320) mod 2048) via a 64-col wrapped prefix on
    the transposed K / z buffers and half-shifted V key tiles.
  - Attention tensors (Q^T, K^T, V, probs, masks) and FFN weights/hidden
    are bf16: same PE rate as fp32r but half the SBUF/DMA and 2x DVE.
  - Softmax denominators are gathered into a [32, 256] tile so ONE
    partition-parallel reciprocal replaces 16 serial [64,512] ones.
  - The global-token path is per-head wide matmuls + one fused softmax,
    emitted interleaved with the band blocks so it hides under them.
  - Normalize/out_proj/LN2 are emitted per band block (software pipeline);
    FFN weights prefetch during attention; x is loaded once into the
    residual buffer.

Self-contained: only imports from /opt/trn_rl_repo (the installed bass
runtime), numpy, and stdlib.
"""

import sys
from contextlib import ExitStack

if "/opt/trn_rl_repo" not in sys.path:
    sys.path.insert(0, "/opt/trn_rl_repo")

import numpy as np

import concourse.bass as bass
import concourse.bacc as bacc_mod
import concourse.mybir as mybir
import concourse.tile as tile
from concourse.masks import make_identity

P = 128
B, S, D, H, FF = 4, 2048, 512, 8, 2048
HD = D // H            # 64
DC = D // P            # 4 chunks of the model dim
FFC = FF // P          # 16 chunks of the FF dim
SK = S + 1             # 2049 tokens (2048 rotated + appended global token)
SQ = 1024              # band queries per core
NT = SQ + 1            # 1025 output tokens
QB = 256               # band query block
NQB = SQ // QB         # 4
NKC = 3                # 128-key chunks per band window
KPF = 64               # wrapped key/token prefix columns
NPAIR = H // 2         # 4 head-pair tiles
EPS = 1e-5
NEG = -1e30

F32 = mybir.dt.float32
F32R = mybir.dt.float32r
BF16 = mybir.dt.bfloat16
AF = mybir.ActivationFunctionType
ALU = mybir.AluOpType
AXL = mybir.AxisListType


def _r(ap):
    """Reinterpret an fp32 AP as float32r for full-rate PE matmuls."""
    return ap.bitcast(F32R)


def build_module():
    nc = bacc_mod.Bacc("TRN2", target_bir_lowering=False)

    x_nat = nc.dram_tensor("x_nat", [SK, D], F32, kind="ExternalInput")
    wq_pc = nc.dram_tensor("wq_pc", [P, DC, D], BF16, kind="ExternalInput")
    wk_pc = nc.dram_tensor("wk_pc", [P, DC, D], BF16, kind="ExternalInput")
    wv_pc = nc.dram_tensor("wv_pc", [P, DC, D], BF16, kind="ExternalInput")
    bq_pc = nc.dram_tensor("bq_pc", [P, DC], F32, kind="ExternalInput")
    bk_pc = nc.dram_tensor("bk_pc", [P, DC], F32, kind="ExternalInput")
    wo_pc = nc.dram_tensor("wo_pc", [P, DC, D], F32R, kind="ExternalInput")
    bo_row = nc.dram_tensor("bo_row", [1, D], F32R, kind="ExternalInput")
    w1_pc = nc.dram_tensor("w1_pc", [P, DC, FF], BF16, kind="ExternalInput")
    b1_pc = nc.dram_tensor("b1_pc", [P, FFC], F32, kind="ExternalInput")
    w2_pc = nc.dram_tensor("w2_pc", [P, FFC, D], BF16, kind="ExternalInput")
    b2_row = nc.dram_tensor("b2_row", [1, D], F32R, kind="ExternalInput")
    mask_band = nc.dram_tensor("mask_band", [P, NQB, NKC, QB], BF16, kind="ExternalInput")
    mask_gcol1 = nc.dram_tensor("mask_gcol1", [1, SQ], BF16, kind="ExternalInput")
    mask_grow = nc.dram_tensor("mask_grow", [1, S], F32R, kind="ExternalInput")
    ones_in = nc.dram_tensor("ones_in", [1, P], F32R, kind="ExternalInput")
    onesp_in = nc.dram_tensor("onesp_in", [P, P], F32R, kind="ExternalInput")
    y_out = nc.dram_tensor("y", [NT, D], F32, kind="ExternalOutput")

    with tile.TileContext(nc) as tc, ExitStack() as ctx:
        # ---- persistent state ----
        persist = ctx.enter_context(tc.tile_pool(name="persist", bufs=1))
        x1 = persist.tile([P, 9, D], F32)
        nc.gpsimd.dma_start(x1[:, 0, :], x_nat[0:P, :])
        nc.gpsimd.dma_start(
            x1[:, 1:4, :], x_nat[P:512, :].rearrange("(t p) d -> p t d", t=3))
        nc.gpsimd.dma_start(
            x1[:, 4:8, :], x_nat[512:1024, :].rearrange("(t p) d -> p t d", t=4))
        nc.gpsimd.dma_start(x1[0:1, 8, :], x_nat[S : S + 1, :])
        ident = persist.tile([P, P], F32)
        make_identity(nc, ident)
        ones_row = persist.tile([1, P], F32R)
        nc.sync.dma_start(ones_row, ones_in[:])
        eps_t = persist.tile([P, 1], F32)
        nc.vector.memset(eps_t, EPS)
        bo_sb = persist.tile([1, D], F32R)
        nc.sync.dma_start(bo_sb, bo_row[:])
        b1_sb = persist.tile([P, FFC], F32)
        nc.sync.dma_start(b1_sb, b1_pc[:])
        b2_sb = persist.tile([1, D], F32R)
        nc.sync.dma_start(b2_sb, b2_row[:])

        oT = [persist.tile([P, NT], F32, name=f"oT{p}") for p in range(NPAIR)]
        z2T = persist.tile([P, DC, NT], BF16)
        pgT = persist.tile([P, 16, 8], BF16)
        onesP = persist.tile([P, P], F32R)
        nc.sync.dma_start(onesP, onesp_in[:])
        onesPb = persist.tile([P, HD], BF16)
        nc.gpsimd.memset(onesPb, 1.0)

        # ---- PSUM pool: mm(2) + sm(2) + sc(2) + smb(2) = 8 banks ----
        ps = ctx.enter_context(tc.tile_pool(name="ps", bufs=2, space="PSUM"))

        # ---- attention state (lives through band phase) ----
        attn = ctx.enter_context(tc.tile_pool(name="attn", bufs=1))
        QT = [attn.tile([P, NT], BF16, name=f"QT{p}") for p in range(NPAIR)]
        KTx = [attn.tile([P, KPF + SK], BF16, name=f"KTx{p}") for p in range(NPAIR)]
        Vsbx = attn.tile([P, 17, H, HD + 1], BF16)
        nc.gpsimd.memset(Vsbx[:, :, :, HD], 1.0)
        # per-head global-key V rows / probs at quarter-partition bases
        Vg = attn.tile([P, H, HD + 1], BF16)
        pgh3 = [attn.tile([P, SQ], BF16, name=f"pgh3_{t}") for t in range(3)]
        k0h = [attn.tile([P, 1], BF16, name=f"k0h{h}") for h in range(H)]
        mb_sb = attn.tile([P, NQB, NKC, QB], BF16)
        mgc_sb = attn.tile([P, SQ], BF16)
        mgrow_sb = attn.tile([1, S], F32R)

        wo_sb = attn.tile([P, DC, D], F32R)
        w1_sb = attn.tile([P, DC, FF], BF16)

        # x loads on the gpsimd queue; weights/masks on the sync queue
        nc.sync.dma_start(mb_sb, mask_band[:])
        nc.sync.dma_start(mgc_sb[0:1, :], mask_gcol1[:])
        nc.sync.dma_start(mgrow_sb, mask_grow[:])
        for mrow in (32, 64):
            nc.vector.tensor_copy(mgc_sb[mrow : mrow + 1, :], mgc_sb[0:1, :])

        # ====== Phase A+B: LN1 -> zTx, V (interleaved), Q/K ======
        with tc.tile_pool(name="phA", bufs=1) as pha, \
             tc.tile_pool(name="lnz", bufs=2) as z_pool, \
             tc.tile_pool(name="st1", bufs=4) as stat_pool:
            zTx = pha.tile([P, DC, KPF + SK], BF16)
            wv_sb = pha.tile([P, DC, D], BF16)
            wq_sb = pha.tile([P, DC, D], BF16)
            wk_sb = pha.tile([P, DC, D], BF16)
            bq_sb = pha.tile([P, DC], F32)
            bk_sb = pha.tile([P, DC], F32)

            nc.sync.dma_start(wv_sb, wv_pc[:])
            nc.sync.dma_start(wq_sb, wq_pc[:])
            nc.sync.dma_start(bq_sb, bq_pc[:])
            nc.sync.dma_start(wk_sb, wk_pc[:])
            nc.sync.dma_start(bk_sb, bk_pc[:])
            nc.sync.dma_start(wo_sb, wo_pc[:])
            nc.sync.dma_start(w1_sb, w1_pc[:])

            def vproj_mm(j):
                """V projection for shifted key tile j (tokens [128j-64,128j+64))."""
                rows = P if j < 16 else 1
                col0 = 128 * j if j < 16 else KPF + S
                pv = ps.tile([P, D], F32, tag="mm")
                for d in range(DC):
                    nc.tensor.matmul(pv[:rows], zTx[:, d, col0 : col0 + rows],
                                     wv_sb[:, d, :],
                                     start=(d == 0), stop=(d == DC - 1))
                return pv, rows

            def vproj_copy(j, pv, rows):
                nc.scalar.activation(
                    Vsbx[:rows, j, :, 0:HD],
                    pv[:rows].rearrange("p (h e) -> p h e", h=H), AF.Copy)

            def ln1(t):
                rows = P if t < 16 else 1
                if t < 8:
                    src = x1[:rows, t, :]
                elif t < 16:
                    xt = z_pool.tile([P, D], F32, tag="xt")
                    nc.gpsimd.dma_start(xt, x_nat[t * P : (t + 1) * P, :])
                    src = xt[:rows]
                else:
                    src = x1[0:1, 8, :]
                st = stat_pool.tile([P, 6], F32, tag="bnst")
                nc.vector.bn_stats(st[:rows], src)
                mv = stat_pool.tile([P, 2], F32, tag="bnmv")
                nc.vector.bn_aggr(mv[:rows], st[:rows])
                rstd = stat_pool.tile([P, 1], F32, tag="rstd")
                nc.scalar.activation(rstd[:rows], mv[:rows, 1:2], AF.Sqrt,
                                     bias=eps_t[:rows], scale=1.0)
                nc.vector.reciprocal(rstd[:rows], rstd[:rows])
                z = z_pool.tile([P, D], F32, tag="z")
                nc.vector.tensor_scalar(z[:rows], src, mv[:rows, 0:1],
                                        rstd[:rows],
                                        op0=ALU.subtract, op1=ALU.mult)
                ptt = ps.tile([P, DC, P], F32, tag="mm")
                for d in range(DC):
                    nc.tensor.transpose(ptt[:, d, :rows],
                                        z[:rows, d * P : (d + 1) * P],
                                        ident[:rows, :rows])
                return ptt

            def zcopy(t, ptt):
                rows = P if t < 16 else 1
                nc.scalar.activation(
                    zTx[:, :, KPF + t * P : KPF + t * P + rows],
                    ptt[:, :, :rows], AF.Copy)
                if t == 15:
                    # wrapped prefix: tokens 1984..2048 = local rows 64..128
                    nc.vector.tensor_copy(zTx[:, :, 0:KPF], ptt[:, :, HD:P])

            # one-tile software pipeline: tile t's PSUM->SBUF copies run
            # behind tile t+1's LN in the scalar stream; V matmuls for key
            # tile j follow zcopy(j), their copy one step later again
            zpend = {}
            vpend = {}
            for t in range(19):
                if t <= 16:
                    zpend[t] = ln1(t)
                if 1 <= t <= 17:
                    zcopy(t - 1, zpend.pop(t - 1))
                vjs = []
                if 2 <= t <= 16:
                    vjs.append(t - 1)
                elif t == 17:
                    vjs += [0, 16]
                for j in vjs:
                    vpend[j] = vproj_mm(j)
                for j in list(vpend):
                    if (t >= 3 and j <= t - 2) or t >= 18:
                        vproj_copy(j, *vpend.pop(j))

            # ---- Q/K projections ----
            q_blocks = [(KPF, 0, 512), (KPF + 512, 512, 512), (KPF + S, SQ, 1)]
            k_blocks = [(KPF + i * 512, KPF + i * 512, 512) for i in range(4)] \
                + [(KPF + S, KPF + S, 1)]
            for p in range(NPAIR):
                for w_sb, b_sb, dst, blocks in (
                        (wq_sb, bq_sb, QT[p], q_blocks),
                        (wk_sb, bk_sb, KTx[p], k_blocks)):
                    for bi, (src, dcol, w) in enumerate(blocks):
                        s0, w0, keep = (src, w, 0) if w > 1 else (src - 1, 2, 1)
                        pq = ps.tile([P, 512], F32, tag="mm")
                        for d in range(DC):
                            nc.tensor.matmul(pq[:, :w0],
                                             w_sb[:, d, p * P : (p + 1) * P],
                                             zTx[:, d, s0 : s0 + w0],
                                             start=(d == 0), stop=(d == DC - 1))
                        if bi % 2 == 0:
                            nc.scalar.activation(dst[:, dcol : dcol + w],
                                                 pq[:, keep : keep + w], AF.Identity,
                                                 bias=b_sb[:, p : p + 1])
                        else:
                            nc.vector.tensor_scalar(dst[:, dcol : dcol + w],
                                                    pq[:, keep : keep + w],
                                                    b_sb[:, p : p + 1], None,
                                                    op0=ALU.add)
                nc.vector.tensor_copy(KTx[p][:, 0:KPF], KTx[p][:, S : S + KPF])

        # ====== Phase C/D/E/F interleaved: band + global + norm + out_proj + LN2
        with tc.tile_pool(name="bandp", bufs=3) as band_pool, \
             tc.tile_pool(name="ln2z", bufs=2) as z2_pool, \
             tc.tile_pool(name="st2", bufs=4) as stat2_pool:

            # band-phase scratch: quarter-partition l tiles, global-path
            # per-pair scratch, zero-padded stationaries
            ltA = band_pool.tile([P, NQB, QB], BF16, bufs=1)
            ltB = band_pool.tile([P, NQB, QB], BF16, bufs=1)
            lrA = band_pool.tile([P, NQB, QB], BF16, bufs=1)
            lrB = band_pool.tile([P, NQB, QB], BF16, bufs=1)
            lq = band_pool.tile([P, NQB, QB], BF16, bufs=1)
            egpp = [band_pool.tile([2, S], F32, tag=f"egpp{p}", bufs=1,
                                   name=f"egpp{p}") for p in range(NPAIR)]
            laP = [band_pool.tile([2, 4], F32, tag=f"laP{p}", bufs=1,
                                  name=f"laP{p}") for p in range(NPAIR)]
            egp8 = band_pool.tile([8, S], F32, bufs=1)
            la8 = band_pool.tile([8, 4], F32, bufs=1)
            ga8 = band_pool.tile([8, 1], F32, bufs=1)
            larec = band_pool.tile([8, 1], F32, bufs=1)

            # col j of q2g[p] holds head (2p+j)'s global-query column in its
            # own 64 contraction rows; k0h[h] is the zero-padded global-key
            # column; Vg packs each head's global-key V row (+ ones col) at
            # quarter-partition bases so band PV matmuls stay legal
            q2g = [band_pool.tile([P, 2], BF16, tag=f"q2g{p}", bufs=1,
                                  name=f"q2g{p}") for p in range(NPAIR)]
            for pr in range(NPAIR):
                nc.gpsimd.memset(q2g[pr], 0.0)
                for j in range(2):
                    sub = j * HD
                    h = 2 * pr + j
                    nc.gpsimd.tensor_copy(
                        q2g[pr][sub : sub + HD, j : j + 1],
                        QT[pr][sub : sub + HD, SQ : SQ + 1])
                    nc.gpsimd.memset(k0h[h], 0.0)
                    nc.gpsimd.tensor_copy(
                        k0h[h][sub : sub + HD, 0:1],
                        KTx[pr][sub : sub + HD, KPF + S : KPF + S + 1])
                    m = 32 * (h % 3)
                    nc.gpsimd.tensor_copy(Vg[m : m + 1, h, :],
                                          Vsbx[0:1, 16, h, :])

            # global-KEY column scores for all band queries (all 8 heads)
            for h in range(H):
                pr = h // 2
                m = 32 * (h % 3)
                pgX = pgh3[h // 3]
                for half in range(2):
                    gq = ps.tile([P, 512], F32, tag="mm")
                    nc.tensor.matmul(gq[0:1, :], k0h[h],
                                     QT[pr][:, half * 512 : (half + 1) * 512],
                                     start=True, stop=True)
                    nc.scalar.activation(
                        pgX[m : m + 1, half * 512 : (half + 1) * 512],
                        gq[0:1, :], AF.Exp)
                nc.vector.tensor_tensor(pgX[m : m + 1, :], pgX[m : m + 1, :],
                                        mgc_sb[m : m + 1, :], ALU.mult)

            def ln2(t):
                rows = P if t < 8 else 1
                src = x1[:rows, t, :]
                st = stat2_pool.tile([P, 6], F32, tag="bnst2")
                nc.vector.bn_stats(st[:rows], src)
                mv = stat2_pool.tile([P, 2], F32, tag="bnmv2")
                nc.vector.bn_aggr(mv[:rows], st[:rows])
                rstd = stat2_pool.tile([P, 1], F32, tag="rstd2")
                nc.scalar.activation(rstd[:rows], mv[:rows, 1:2], AF.Sqrt,
                                     bias=eps_t[:rows], scale=1.0)
                nc.vector.reciprocal(rstd[:rows], rstd[:rows])
                z2 = z2_pool.tile([P, D], F32, tag="z2")
                nc.vector.tensor_scalar(z2[:rows], src, mv[:rows, 0:1],
                                        rstd[:rows],
                                        op0=ALU.subtract, op1=ALU.mult)
                ptt = ps.tile([P, DC, P], F32, tag="mm")
                for d in range(DC):
                    nc.tensor.transpose(ptt[:, d, :rows],
                                        z2[:rows, d * P : (d + 1) * P],
                                        ident[:rows, :rows])
                nc.scalar.activation(z2T[:, :, t * P : t * P + rows],
                                     ptt[:, :, :rows], AF.Copy)

            def out_proj(t):
                w = P if t < 8 else 1
                yp = ps.tile([P, D], F32, tag="mm")
                if w > 1:
                    for p in range(NPAIR):
                        nc.tensor.matmul(yp[:w], _r(oT[p][:, t * P : t * P + w]),
                                         _r(wo_sb[:, p, :]),
                                         start=(p == 0), stop=False)
                else:
                    for p in range(NPAIR):
                        nc.tensor.matmul(yp[:w], _r(oT[p][:, SQ : SQ + 1]),
                                         _r(wo_sb[:, p, :]),
                                         start=(p == 0), stop=False)
                nc.tensor.matmul(yp[:w], _r(ones_row[:1, :w]), _r(bo_sb),
                                 start=False, stop=True)
                nc.vector.tensor_tensor(x1[:w, t, :], yp[:w], x1[:w, t, :], ALU.add)

            def global_scores(pr, tcb):
                gs = ps.tile([P, 512], F32, tag="mm")
                nc.tensor.matmul(gs[0:2, :], q2g[pr],
                                 KTx[pr][:, tcb * 512 : (tcb + 1) * 512],
                                 start=True, stop=False)
                # additive key mask is head-independent -> rank-1 matmul add
                nc.tensor.matmul(gs[0:2, :], _r(onesP[0:1, 0:2]),
                                 mgrow_sb[0:1, tcb * 512 : (tcb + 1) * 512],
                                 start=False, stop=True)
                nc.scalar.activation(
                    egpp[pr][:, tcb * 512 : (tcb + 1) * 512],
                    gs[0:2, :], AF.Exp,
                    accum_out=laP[pr][:, tcb : tcb + 1])

            def global_gather():
                for pr in range(NPAIR):
                    nc.gpsimd.dma_start(egp8[2 * pr : 2 * pr + 2, :], egpp[pr][:])
                    nc.gpsimd.dma_start(la8[2 * pr : 2 * pr + 2, :], laP[pr][:])

            def global_transposes():
                nc.vector.tensor_reduce(ga8, la8, axis=AXL.X, op=ALU.add)
                nc.vector.reciprocal(larec, ga8)
                for c in range(16):
                    ptb = ps.tile([P, QB], F32, tag="sm", bufs=2)
                    nc.tensor.transpose(ptb[:, 0:8], egp8[0:8, c * P : (c + 1) * P],
                                        ident[0:8, 0:8])
                    nc.vector.tensor_copy(pgT[:, c, :], ptb[:, 0:8])

            def global_pv():
                for g in range(2):
                    pog = ps.tile([P, 512], F32, tag="mm")
                    for c in range(16):
                        nc.tensor.matmul(pog[0:8, 0 : 4 * HD], pgT[:, c, :],
                                         Vsbx[:, c, 4 * g : 4 * g + 4, 0:HD],
                                         start=(c == 0), stop=(c == 15))
                    pog_sb = band_pool.tile([8, 4 * HD], F32, tag="pog_sb")
                    nc.scalar.activation(pog_sb, pog[0:8, 0 : 4 * HD], AF.Copy,
                                         scale=larec[0:8, 0:1])
                    for j in range(2):
                        ptj = ps.tile([P, QB], F32, tag="sm", bufs=2)
                        nc.tensor.transpose(ptj[:, 0:8],
                                            pog_sb[0:8, j * P : (j + 1) * P],
                                            ident[0:8, 0:8])
                        for hh in (2 * j, 2 * j + 1):
                            h = 4 * g + hh
                            rlo = (hh % 2) * HD
                            nc.scalar.activation(
                                oT[h // 2][rlo : rlo + HD, SQ : SQ + 1].bitcast(F32R),
                                ptj[rlo : rlo + HD, h : h + 1], AF.Copy)

            def lsrc(h):
                if h % 4 == 3:
                    return lq, 32 * (h // 4)
                return (lrA if h < 4 else lrB), 32 * (h % 4)

            for i in range(NQB):
                # interleave the global-token path under the band blocks;
                # for the last block it runs FIRST so out_proj(8) overlaps
                if i == 2:
                    global_transposes()
                elif i == 3:
                    global_pv()

                def po_copies(h, po):
                    pr, sub = h // 2, (h % 2) * HD
                    ml = 32 * (h % 4)
                    ltX = ltA if h < 4 else ltB
                    oT_dst = oT[pr][sub : sub + HD,
                                    i * QB : (i + 1) * QB].bitcast(F32R)
                    if h % 2 == 0:
                        nc.vector.tensor_copy(oT_dst, po[0:HD, :])
                        nc.scalar.activation(ltX[ml : ml + 1, i, :],
                                             po[HD : HD + 1, :], AF.Copy)
                    else:
                        nc.scalar.activation(oT_dst, po[0:HD, :], AF.Copy)
                        nc.vector.tensor_copy(ltX[ml : ml + 1, i, :],
                                              po[HD : HD + 1, :])

                def sc_mm(h):
                    pr, sub = h // 2, (h % 2) * HD
                    q_ap = QT[pr][sub : sub + HD, i * QB : (i + 1) * QB]
                    sc = ps.tile([P, NKC, QB], F32, tag="sc", bufs=2)
                    for c in range(NKC):
                        nc.tensor.matmul(
                            sc[:, c, :],
                            KTx[pr][sub : sub + HD,
                                    QB * i + c * P : QB * i + c * P + P],
                            q_ap, start=True, stop=True)
                    return sc

                def po_mm(h, pT):
                    po = ps.tile([P, QB], F32, tag="sm", bufs=2)
                    for c in range(NKC):
                        nc.tensor.matmul(po[0 : HD + 1, :],
                                         Vsbx[:, 2 * i + c, h, :],
                                         pT[:, c, :], start=(c == 0), stop=False)
                    m = 32 * (h % 3)
                    pgX = pgh3[h // 3]
                    nc.tensor.matmul(po[0 : HD + 1, :], Vg[m : m + 1, h, :],
                                     pgX[m : m + 1, i * QB : (i + 1) * QB],
                                     start=False, stop=True)
                    return po

                def norm_half(half):
                    lrX, ltX = (lrA, ltA) if half == 0 else (lrB, ltB)
                    with nc.allow_low_precision(reason="bf16 softmax sums"):
                        nc.vector.reciprocal(lrX[:, i, :], ltX[:, i, :])
                    nc.vector.tensor_copy(lq[32 * half : 32 * half + 1, i, :],
                                          lrX[96:97, i, :])
                    for p in (2 * half, 2 * half + 1):
                        for j in range(2):
                            lt_t, r = lsrc(2 * p + j)
                            lbc = ps.tile([P, QB], F32, tag="sm", bufs=2)
                            nc.tensor.matmul(lbc[0:HD, :], onesPb[r : r + 1, :],
                                             lt_t[r : r + 1, i, :],
                                             start=True, stop=True)
                            rows = oT[p][j * HD : (j + 1) * HD,
                                         i * QB : (i + 1) * QB]
                            nc.vector.tensor_tensor(rows.bitcast(F32R), rows,
                                                    lbc[0:HD, :], ALU.mult)

                # two-deep software pipeline: PE never waits on the softmax
                # (sc for head h+1 precedes po for head h), and the in-order
                # scalar stream sees copies one head late
                sc_t = {0: sc_mm(0)}
                po_t = {}
                for h in range(H):
                    pT = band_pool.tile([P, NKC, QB], BF16, tag="pT")
                    nc.scalar.activation(pT, sc_t.pop(h), AF.Exp)
                    nc.vector.tensor_tensor(pT, pT, mb_sb[:, i, :, :], ALU.mult)
                    if h + 1 < H:
                        sc_t[h + 1] = sc_mm(h + 1)
                    po_t[h] = po_mm(h, pT)
                    if h - 1 in po_t:
                        po_copies(h - 1, po_t.pop(h - 1))
                    if h == 5:
                        norm_half(0)
                po_copies(7, po_t.pop(7))
                norm_half(1)

                if i == 0:
                    for tcb in range(4):
                        global_scores(0, tcb)
                        global_scores(1, tcb)
                elif i == 1:
                    for tcb in range(4):
                        global_scores(2, tcb)
                        global_scores(3, tcb)
                    global_gather()


                # pipeline: out_proj for the two finished token tiles
                out_proj(2 * i)
                out_proj(2 * i + 1)


            out_proj(8)
            # LN2 runs as its own pass AFTER all band EXPs: the wait hint
            # stops the scheduler re-interleaving its SQRTs into the band
            # stream (each EXP<->SQRT switch costs a 1.3us table load)
            with tc.tile_wait_until(0.26):
                for t in range(9):
                    ln2(t)

        # band scratch released; fetch FFN2 weights under out_proj/FFN1
        ffw = ctx.enter_context(tc.tile_pool(name="ffw", bufs=1))
        w2_sb = ffw.tile([P, FFC, D], BF16)
        nc.sync.dma_start(w2_sb, w2_pc[:])

        # ====== Phase G: FFN + residual -> y ======
        with tc.tile_pool(name="ffn", bufs=1) as ffn_pool, \
             tc.tile_pool(name="ffo", bufs=3) as out_pool:
            for t0, tw in [(0, 512), (512, 512), (SQ, 1)]:
                hT = ffn_pool.tile([P, FFC, 512], BF16, tag="hT")
                s0, w0, keep = (t0, tw, 0) if tw > 1 else (t0 - 1, 2, 1)
                for f in range(FFC):
                    ph = ps.tile([P, 512], F32, tag="mm")
                    for d in range(DC):
                        nc.tensor.matmul(ph[:, :w0],
                                         w1_sb[:, d, f * P : (f + 1) * P],
                                         z2T[:, d, s0 : s0 + w0],
                                         start=(d == 0), stop=(d == DC - 1))
                    nc.scalar.activation(hT[:, f, :tw],
                                         ph[:, keep : keep + tw], AF.Gelu,
                                         bias=b1_sb[:, f : f + 1])
                nsub = 4 if tw == 512 else 1
                for stp in range(nsub):
                    sw = P if tw == 512 else 1
                    ps2 = ps.tile([P, D], F32, tag="mm")
                    for f in range(FFC):
                        nc.tensor.matmul(ps2[:sw],
                                         hT[:, f, stp * P : stp * P + sw],
                                         w2_sb[:, f, :],
                                         start=(f == 0), stop=False)
                    nc.tensor.matmul(ps2[:sw], _r(ones_row[:1, :sw]), _r(b2_sb),
                                     start=False, stop=True)
                    yt = out_pool.tile([P, D], F32, tag="yt")
                    tglob = t0 // P + stp
                    nc.vector.tensor_tensor(yt[:sw], ps2[:sw],
                                            x1[:sw, tglob, :], ALU.add)
                    nc.gpsimd.dma_start(
                        y_out[t0 + stp * P : t0 + stp * P + sw, :], yt[:sw])

    nc.finalize()
    return nc


def make_host_inputs(x, padding_mask, attn_mask, in_proj_w, in_proj_b, out_proj_w,
                     out_proj_b, ln1_g, ln1_b, ln2_g, ln2_b, ff_w1, ff_b1, ff_w2,
                     ff_b2):
    """Build the 8 per-core input maps (numpy only)."""
    import ml_dtypes
    f32 = np.float32
    bf16 = ml_dtypes.bfloat16
    x = np.asarray(x, f32)
    attn_mask = np.asarray(attn_mask, f32)
    padding_mask = np.asarray(padding_mask, bool)

    g1 = np.asarray(ln1_g, f32); b1 = np.asarray(ln1_b, f32)
    g2 = np.asarray(ln2_g, f32); b2 = np.asarray(ln2_b, f32)
    Wq, Wk, Wv = (np.asarray(in_proj_w[i * D:(i + 1) * D], f32) for i in range(3))
    bq0, bk0, bv0 = (np.asarray(in_proj_b[i * D:(i + 1) * D], f32) for i in range(3))
    sc = 1.0 / np.sqrt(HD)

    Wq_ = Wq * g1[None, :] * sc
    bq_ = (Wq @ b1 + bq0) * sc
    Wk_ = Wk * g1[None, :]
    bk_ = Wk @ b1 + bk0
    Wv_ = Wv * g1[None, :]
    bv_ = Wv @ b1 + bv0
    W1_ = np.asarray(ff_w1, f32) * g2[None, :]
    b1f = np.asarray(ff_w1, f32) @ b2 + np.asarray(ff_b1, f32)

    def pc(wt, nchunk):  # [Dout, Din] -> [P, nchunk, Dout] chunked on Din
        return np.ascontiguousarray(
            wt.T.reshape(nchunk, P, wt.shape[0]).transpose(1, 0, 2))

    shared = {
        "wq_pc": pc(Wq_, DC).astype(bf16), "wk_pc": pc(Wk_, DC).astype(bf16),
        "wv_pc": pc(Wv_, DC).astype(bf16),
        "bq_pc": np.ascontiguousarray(bq_.reshape(DC, P).T),
        "bk_pc": np.ascontiguousarray(bk_.reshape(DC, P).T),
        "wo_pc": pc(np.asarray(out_proj_w, f32), DC),
        "bo_row": (np.asarray(out_proj_w, f32) @ bv_
                   + np.asarray(out_proj_b, f32))[None, :].copy(),
        "w1_pc": pc(W1_, DC).astype(bf16),
        "b1_pc": np.ascontiguousarray(b1f.reshape(FFC, P).T),
        "w2_pc": pc(np.asarray(ff_w2, f32), FFC).astype(bf16),
        "b2_row": np.asarray(ff_b2, f32)[None, :].copy(),
        "ones_in": np.ones((1, P), f32),
        "onesp_in": np.ones((P, P), f32),
    }

    in_maps = []
    for core in range(8):
        b = core // 2
        h = core % 2
        rot = np.roll(x[b], -1024 * h, axis=0)
        x_nat = np.ascontiguousarray(np.concatenate([rot, x[b, 0:1]], axis=0))

        # additive mask for this batch -> multiplicative factor
        A = attn_mask + np.where(padding_mask[b], -np.inf, 0.0)[None, :]
        mfac = np.exp(np.minimum(A, 0.0)).astype(f32)  # exp(-inf)=0, exp(0)=1
        mfac[~np.isfinite(A)] = 0.0

        # band masks: [P(t), NQB(i), NKC(c), QB(r)];
        # chunk c of block i covers rotated keys [256i - 64 + 128c, +128)
        i_idx = np.arange(NQB)[:, None, None, None]
        c_idx = np.arange(NKC)[None, :, None, None]
        t_idx = np.arange(P)[None, None, :, None]
        r_idx = np.arange(QB)[None, None, None, :]
        k_rot = (QB * i_idx - KPF + P * c_idx + t_idx) % S
        q_rot = i_idx * QB + r_idx
        gq = (q_rot + 1024 * h) % S
        gk = (k_rot + 1024 * h) % S
        band = mfac[gq, gk]                       # [NQB, NKC, P, QB]
        mask_band = np.ascontiguousarray(band.transpose(2, 0, 1, 3)).astype(bf16)

        # global-key column mask, zeroed when key0 falls inside the window
        key0_rot = (0 - 1024 * h) % S
        gq2 = (np.arange(NQB)[:, None] * QB + np.arange(QB)[None, :] + 1024 * h) % S
        gcol = mfac[gq2, 0].copy()
        for i in range(NQB):
            off = (key0_rot - (QB * i - KPF)) % S
            if off < NKC * P:
                gcol[i, :] = 0.0  # key 0 already inside this block's band window
        mask_gcol1 = np.ascontiguousarray(gcol.reshape(1, -1)).astype(bf16)

        # global-query additive mask row, in SHIFTED key order:
        # score col kappa <-> rotated key (kappa - 64) mod S
        kap = np.arange(S)
        k_act = (((kap - KPF) % S) + 1024 * h) % S
        mask_grow = np.ascontiguousarray(
            np.maximum(A[0, k_act], NEG)[None, :].astype(f32))

        m = dict(shared)
        m.update({
            "x_nat": x_nat,
            "mask_band": mask_band,
            "mask_gcol1": mask_gcol1,
            "mask_grow": mask_grow,
        })
        in_maps.append(m)
    return in_maps


def assemble_output(results):
    """results: list of 8 dicts with 'y' [NT, D] -> full [B, S, D]."""
    out = np.empty((B, S, D), np.float32)
    for b in range(B):
        y0 = results[2 * b]["y"]
        y1 = results[2 * b + 1]["y"]
        out[b, 0] = y0[SQ]
        out[b, 1:SQ] = y0[1:SQ]
        out[b, SQ:] = y1[0:SQ]
    return out


_CACHED_NC = None


def kernel(**inputs) -> np.ndarray:
    global _CACHED_NC
    from concourse.bass_utils import run_bass_kernel_spmd

    in_maps = make_host_inputs(**inputs)
    if _CACHED_NC is None:
        _CACHED_NC = build_module()
    res = run_bass_kernel_spmd(_CACHED_NC, in_maps, core_ids=list(range(8)))
    return assemble_output(res.results)


if __name__ == "__main__":
    nc = build_module()
    print("build + compile OK")
